# revision 51
# baseline (speedup 1.0000x reference)
"""RoIAlign (scale_and_translate, linear, antialias) Trainium2 kernel.

Channel-sharded across 8 NeuronCores: each core keeps a [512, 512, 8]
slice of the feature map resident in SBUF and computes all 512 boxes
for its 8 channels, one box at a time (no box grouping):

  stage 1 (PE):  T^T[x, (c, i)] = sum_y F[y, x, c] * Wy[y, i]
                 one matmul per (c, y-tile, x-window), out free = 32.
                 Wide-kernel boxes (ky*kx >= 12) run fp8e4 DoubleRow
                 (two 128-row y-tiles per matmul, 0.5 cycles/row); the
                 rest run bf16.
  evac:          psum_t [128, 512] (two (box, window) slots) -> SBUF
                 bf16, least-loaded assignment over DVE / ACT (GPSIMD
                 cannot read PSUM).
  stage 2 (PE):  out[(c,i), j] = sum_x T^T[x, ci] * Wx[x, j],
                 operand-swapped (lhsT = T^T chunk, moving = 32 Wx
                 cols) so out free = 32 instead of 256.
  out:           psum2 [128, 512] (8 boxes) -> bf16 staging -> DRAM.

Boxes with wide x-kernels use device-resident 2x/4x x-pooled copies of
the feature map (hat-function pooling; per-box Wx is least-squares
refit onto the hat basis, with delta columns at the image edge). This
shrinks the x-band, cutting stage-1 work and - critically - the
PSUM-evacuation volume, which is the binding engine resource.

Host side computes exact per-box dense resampling weights, extracts
nonzero bands, zero-pads to 128-row tiles/windows, and streams them as
flat [128, cols] arrays in 32-box chunks. fp8 pooled maps are padded
to multiple-of-16 widths (DoubleRow ldweights stride restriction).
"""

import numpy as np

H = 512
W = 512
C = 64
N_BOXES = 512
OUT = 32
N_CORES = 8
C_LOC = C // N_CORES  # 8 channels per core
PART = 128
NT = H // PART        # 4 y partition tiles
GROUP = 32            # boxes per weight-DMA chunk
SLOTS = 4             # (box, window) slots per psum_t tile
FP8_KPROD = 12.0      # use fp8 stage-1 when ky*kx >= this
FP8_KMIN = 2.0        # ... and both ky, kx >= this
XP2_KMIN = 10.0       # use the 2x x-pooled map when kx >= this
XP4_KMIN = 14.0       # use the 4x x-pooled map when kx >= this
WCLIP = 1e-3          # zero fitted pooled weights below this (rel) magnitude
UP2 = W // 2 + 1      # 2x-pooled x size: hats at even x + delta at x=511
UP4 = (W - 1) // 4 + 1 + 3  # 4x pooled: 128 hat nodes + 3 tail deltas = 131
UPS = {2: UP2, 4: UP4}
# fp8 DoubleRow ldweights requires the pair-dim byte stride to be a
# multiple of 16 -> pad the fp8 pooled-map widths up to a multiple of 16
UPS_PAD = {p: (u + 15) // 16 * 16 for p, u in UPS.items()}

# scheduling knobs for _build_program (tuned via TimelineSim)
BUILD_KW = dict(slots=2, p1bufs=6, rbufs=12, evac_pat="auto", wdma="yy",
                odma="y", s2_lag=10)


# ---------------------------------------------------------------------------
# Host-side weight computation (mirrors jax.image.scale_and_translate with
# method="linear", antialias=True)
# ---------------------------------------------------------------------------

def _compute_weight_mat(in_size, out_size, scale, translation):
    inv_scale = 1.0 / scale
    kernel_scale = max(inv_scale, 1.0)
    sample_f = (np.arange(out_size, dtype=np.float64) + 0.5) * inv_scale \
        - translation * inv_scale - 0.5
    x = np.abs(sample_f[None, :] - np.arange(in_size, dtype=np.float64)[:, None]) \
        / kernel_scale
    weights = np.maximum(0.0, 1.0 - x)
    total = weights.sum(axis=0, keepdims=True)
    weights = np.where(
        np.abs(total) > 1000.0 * float(np.finfo(np.float32).eps),
        weights / np.where(total != 0, total, 1.0),
        0.0,
    )
    valid = (sample_f >= -0.5) & (sample_f <= in_size - 0.5)
    return np.where(valid[None, :], weights, 0.0).astype(np.float32)


def host_geometry(boxes):
    """Exact per-box dense weights wy/wx [N, 512, 32] fp32."""
    boxes = np.asarray(boxes, dtype=np.float64)
    wy_all = np.zeros((N_BOXES, H, OUT), np.float32)
    wx_all = np.zeros((N_BOXES, W, OUT), np.float32)
    for n in range(N_BOXES):
        cx, cy, w, h = boxes[n]
        x0 = cx - w / 2
        y0 = cy - h / 2
        w = max(w, 1e-6)
        h = max(h, 1e-6)
        x_scale = OUT / (w * W)
        y_scale = OUT / (h * H)
        wy_all[n] = _compute_weight_mat(H, OUT, y_scale, -y0 * OUT / h)
        wx_all[n] = _compute_weight_mat(W, OUT, x_scale, -x0 * OUT / w)
    return wy_all, wx_all


def xpool_basis(p):
    """Hat basis at stride-p grid nodes plus delta columns for the tail
    pixels past the last node (which a hat grid cannot represent)."""
    nu = (W - 1) // p + 1
    extra = [x for x in range(W) if x > (nu - 1) * p]
    P = np.zeros((W, nu + len(extra)), np.float32)
    for u in range(nu):
        c = p * u
        for x in range(max(0, c - p + 1), min(W, c + p)):
            P[x, u] = 1.0 - abs(x - c) / p
    for j, x in enumerate(extra):
        P[x, :] = 0.0
        P[x, nu + j] = 1.0
    return P


def xpool_fit_mats():
    """Per pooling factor: (P, PINV) with PINV = (P^T P)^-1 P^T."""
    mats = {}
    for p in (2, 4):
        P = xpool_basis(p)
        PINV = np.linalg.solve((P.T @ P).astype(np.float64),
                               P.T.astype(np.float64))
        mats[p] = (P, PINV)
    return mats


def plan_boxes(boxes, wy_all, wx_all):
    """Per-box banded geometry + flat weight packing.

    Returns plan dicts (None for boxes with empty bands) and the packed
    flat weight arrays wyb [128, :] bf16, wy8 [128, :] fp8, wx [128, :]
    bf16 (as float32 here; cast at upload).
    """
    boxes = np.asarray(boxes, dtype=np.float64)
    ky = np.maximum(16.0 * boxes[:, 3], 1.0)  # y kernel halfwidth (px)
    kx = np.maximum(16.0 * boxes[:, 2], 1.0)

    mats = xpool_fit_mats()

    plans = []
    wyb_cols, wy8_cols, wx_cols = [], [], []
    for n in range(N_BOXES):
        ynz = np.nonzero(wy_all[n].any(axis=1))[0]
        xnz = np.nonzero(wx_all[n].any(axis=1))[0]
        if len(ynz) == 0 or len(xnz) == 0:
            plans.append(None)
            continue
        r0, r1 = int(ynz[0]), int(ynz[-1]) + 1
        use_fp8 = (ky[n] * kx[n] >= FP8_KPROD
                   and ky[n] >= FP8_KMIN and kx[n] >= FP8_KMIN)
        xp = 4 if kx[n] >= XP4_KMIN else (2 if kx[n] >= XP2_KMIN else 1)

        if xp > 1:
            # least-squares fit of Wx in the pooled hat basis, clipping
            # the tiny ringing tail of the fit to keep the band compact
            P, PINV = mats[xp]
            wx_n = (PINV @ wx_all[n].astype(np.float64)).astype(np.float32)
            wx_n[np.abs(wx_n) < WCLIP * np.abs(wx_n).max()] = 0.0
            xnz = np.nonzero(wx_n.any(axis=1))[0]
            if len(xnz) == 0:
                plans.append(None)
                continue
            WW = UPS[xp]
        else:
            wx_n = wx_all[n]
            WW = W
        c0, c1 = int(xnz[0]), int(xnz[-1]) + 1

        # x windows (arbitrary free-dim offset, clamped; overlap zeroed)
        nxw = (c1 - c0 + PART - 1) // PART
        xws = [min(c0 + k * PART, max(WW - PART, 0)) for k in range(nxw)]

        p = {"fp8": use_fp8, "xp": xp, "xws": xws}
        if use_fp8:
            p_lo, p_hi = r0 // (2 * PART), (r1 - 1) // (2 * PART)
            p["tp"] = list(range(p_lo, p_hi + 1))
            p["wy8_off"] = len(wy8_cols) and sum(c.shape[1] for c in wy8_cols)
            p["wy8_off"] = sum(c.shape[1] for c in wy8_cols)
            for tp in p["tp"]:
                # [128, (q, i)] with y = tp*256 + q*128 + p
                blk = np.zeros((PART, 2, OUT), np.float32)
                for q in range(2):
                    blk[:, q, :] = wy_all[n][tp * 256 + q * 128: tp * 256 + (q + 1) * 128]
                wy8_cols.append(blk.reshape(PART, 2 * OUT))
        else:
            t_lo, t_hi = r0 // PART, (r1 - 1) // PART
            p["ts"] = list(range(t_lo, t_hi + 1))
            p["wyb_off"] = sum(c.shape[1] for c in wyb_cols)
            for t in p["ts"]:
                wyb_cols.append(wy_all[n][t * PART:(t + 1) * PART].copy())

        p["wx_off"] = sum(c.shape[1] for c in wx_cols)
        prev_end = c0
        for x0 in xws:
            wxw = wx_n[x0:x0 + PART].copy()
            lo = max(prev_end - x0, 0)
            wxw[:lo] = 0.0
            prev_end = max(prev_end, x0 + PART)
            wx_cols.append(wxw)
        plans.append(p)

    def cat(cols, width):
        if not cols:
            return np.zeros((PART, width), np.float32)
        return np.concatenate(cols, axis=1)

    wyb_flat = cat(wyb_cols, OUT)
    wy8_flat = cat(wy8_cols, 2 * OUT)
    wx_flat = cat(wx_cols, OUT)
    return plans, wyb_flat, wy8_flat, wx_flat


# ---------------------------------------------------------------------------
# Device program
# ---------------------------------------------------------------------------

def _split_multiwait_bir(raw: bytes) -> bytes:
    """The walrus build here accepts only one sync wait per instruction.
    Hoist extra waits onto single-wait EventSemaphore instructions inserted
    just before, on the same engine (per-engine order is preserved)."""
    import orjson

    d = orjson.loads(raw)
    ctr = 0
    for fn in d.get("functions", []):
        for bb in fn.get("blocks") or []:
            out = []
            for ins in bb["instructions"]:
                si = ins.get("sync_info")
                ws = (si or {}).get("on_wait") or []
                if len(ws) > 1:
                    for w in ws[:-1]:
                        ctr += 1
                        out.append({
                            "debug": ins.get("debug", 0),
                            "engine": ins["engine"],
                            "ins": [],
                            "outs": [],
                            "name": f"{ins['name']}-xw{ctr}",
                            "opcode": "EventSemaphore",
                            "sync_info": {"on_update": [], "on_wait": [w]},
                        })
                    si["on_wait"] = [ws[-1]]
                out.append(ins)
            bb["instructions"] = out
    return orjson.dumps(d)


def _patch_serialization(nc):
    orig = nc.to_json_bytes

    def patched():
        return _split_multiwait_bir(orig())

    nc.to_json_bytes = patched
    return nc


def _make_tc_class():
    import concourse.tile as tile
    from concourse.vector_clock import ScopedClock
    import bass_rust

    class TC(tile.TileContext):
        """TileContext with the tail drain's multi-sem wait split into
        individual single-wait instructions (this walrus rejects >1 wait
        on a CTRL instruction)."""

        def _drain_and_barrier(self, tick_clock, wait_clock):
            nc = self.nc
            probe = nc.sync.drain()
            wait_clock.add_sem_waits(
                probe.ins, ScopedClock({None: tick_clock.global_clock})
            )
            waits = list(probe.ins.sync_info.on_wait)
            probe.ins.sync_info = bass_rust.SyncInfo(on_wait=[], on_update=[])
            by_name = {hh.name: hh for hh in self.sems.allocated().values()}
            for wt in waits:
                nc.sync.wait_ge(by_name[wt.ant_name], wt.wait_value)
            nc.all_engine_barrier()
            popped = nc._tile_sem_poison_stack.pop()
            assert popped is self._sem_poison
            nc.clear_and_free_semaphores(list(self.sems.allocated().values()))
            nc.all_engine_barrier()

    return TC


def _build_program(plans, wyb_cols_n, wy8_cols_n, wx_cols_n,
                   slots=SLOTS, p1bufs=2, rbufs=6, p2bufs=2, evac_mod=3,
                   evac_pat=None, wdma="gs", odma="y", s2_lag=0):
    import concourse.bass as bass
    import concourse.mybir as mybir
    from contextlib import ExitStack

    FP32 = mybir.dt.float32
    BF16 = mybir.dt.bfloat16
    F8E4 = mybir.dt.float8e4
    DR = mybir.MatmulPerfMode.DoubleRow

    any_fp8 = any(p is not None and p["fp8"] for p in plans)
    # pooled-map variants needed: (pool factor, fp8?)
    variants = sorted({(p["xp"], p["fp8"]) for p in plans
                       if p is not None and p["xp"] > 1})

    nc = bass.Bass()
    # feature map, already in SBUF layout [128, (c, t, x)] / [128, (c, tp, q, x)]
    f_d = nc.dram_tensor("f", [PART, C_LOC * NT * W], BF16, kind="ExternalInput")
    if any_fp8:
        f8_d = nc.dram_tensor("f8", [PART, C_LOC * 2 * 2 * W], F8E4,
                              kind="ExternalInput")
    g_ds = {}
    for (xp, isf8) in variants:
        nm = f"g{xp}{'f8' if isf8 else ''}"
        if isf8:
            g_ds[(xp, True)] = nc.dram_tensor(
                nm, [PART, C_LOC * 2 * 2 * UPS_PAD[xp]], F8E4,
                kind="ExternalInput")
        else:
            g_ds[(xp, False)] = nc.dram_tensor(
                nm, [PART, C_LOC * NT * UPS[xp]], BF16, kind="ExternalInput")
    wyb_d = nc.dram_tensor("wyb", [PART, max(wyb_cols_n, OUT)], BF16,
                           kind="ExternalInput")
    if any_fp8:
        wy8_d = nc.dram_tensor("wy8", [PART, max(wy8_cols_n, 2 * OUT)], F8E4,
                               kind="ExternalInput")
    wx_d = nc.dram_tensor("wx", [PART, max(wx_cols_n, OUT)], BF16,
                          kind="ExternalInput")
    # output: box b = 16*G + g: cols G*1024 + g*64 + hh*32 + j,
    # partition p = c_lh*32 + i, channel c = hh*4 + c_lh
    NG16 = (N_BOXES + 15) // 16
    if odma == "P":
        out_d = nc.dram_tensor("out", [PART, (N_BOXES // 4) * 256], FP32,
                               kind="ExternalOutput")
    else:
        out_d = nc.dram_tensor("out", [PART, NG16 * 1024], BF16,
                               kind="ExternalOutput")

    # per-GROUP chunk column ranges
    def group_ranges(key, width):
        rng = []
        for g0 in range(0, N_BOXES, GROUP):
            los, his = [], []
            for n in range(g0, min(g0 + GROUP, N_BOXES)):
                p = plans[n]
                if p is None or key not in p:
                    continue
                ntiles = len(p["ts"]) if key == "wyb_off" else (
                    len(p["tp"]) if key == "wy8_off" else len(p["xws"]))
                los.append(p[key])
                his.append(p[key] + ntiles * width)
            rng.append((min(los), max(his)) if los else None)
        return rng

    wyb_rng = group_ranges("wyb_off", OUT)
    wy8_rng = group_ranges("wy8_off", 2 * OUT)
    wx_rng = group_ranges("wx_off", OUT)

    TC = _make_tc_class()
    with TC(nc) as tc, ExitStack() as ctx:
        fpool = ctx.enter_context(tc.tile_pool(name="fmap", bufs=1))
        wpool = ctx.enter_context(tc.tile_pool(name="wts", bufs=2))
        rpool = ctx.enter_context(tc.tile_pool(name="rhs2", bufs=rbufs))
        opool = ctx.enter_context(tc.tile_pool(name="osb", bufs=2))
        p1pool = ctx.enter_context(tc.tile_pool(name="psumT", bufs=p1bufs, space="PSUM"))
        p2pool = ctx.enter_context(tc.tile_pool(name="psum2", bufs=p2bufs, space="PSUM"))

        _eng_map = {"g": nc.gpsimd, "s": nc.scalar, "y": nc.sync}

        def _wy_eng():
            return _eng_map[wdma[0]]

        def _wx_eng():
            return _eng_map[wdma[1]]

        # weight chunk state
        wy_tiles = {}  # group index -> {"b": (tile, base), "8": (tile, base)}
        wx_tiles = {}  # group index -> (tile, base)

        def load_chunks(gi):
            if gi in wy_tiles:
                return
            wy_tiles[gi] = {}
            if wyb_rng[gi] is not None:
                lo, hi = wyb_rng[gi]
                tb = wpool.tile([PART, hi - lo], BF16, name="wyb_sb", tag="wyb")
                _wy_eng().dma_start(out=tb[:], in_=wyb_d[:, lo:hi])
                wy_tiles[gi]["b"] = (tb, lo)
            if wy8_rng[gi] is not None:
                lo, hi = wy8_rng[gi]
                t8 = wpool.tile([PART, hi - lo], F8E4, name="wy8_sb", tag="wy8")
                _wy_eng().dma_start(out=t8[:], in_=wy8_d[:, lo:hi])
                wy_tiles[gi]["8"] = (t8, lo)
            if wx_rng[gi] is not None:
                lo, hi = wx_rng[gi]
                tx = wpool.tile([PART, hi - lo], BF16, name="wx_sb", tag="wx")
                _wx_eng().dma_start(out=tx[:], in_=wx_d[:, lo:hi])
                wx_tiles[gi] = (tx, lo)

        # group-0 weights first so PE can start as soon as the first F
        # tiles land (same FIFO queue as the F DMAs below)
        load_chunks(0)

        f_sb = fpool.tile([PART, C_LOC * NT * W], BF16, name="f_sb")
        f_v = f_sb[:].rearrange("p (c t x) -> p c t x", c=C_LOC, t=NT)
        f_dv = f_d[:].rearrange("p (c t x) -> p c t x", c=C_LOC, t=NT)
        for t in range(NT):
            nc.sync.dma_start(out=f_v[:, :, t, :], in_=f_dv[:, :, t, :])
        if any_fp8:
            f8_sb = fpool.tile([PART, C_LOC * 2 * 2 * W], F8E4, name="f8_sb")
            f8_v = f8_sb[:].rearrange("p (c tp q x) -> p c tp q x",
                                      c=C_LOC, tp=2, q=2)
            f8_dv = f8_d[:].rearrange("p (c tp q x) -> p c tp q x",
                                      c=C_LOC, tp=2, q=2)
            for tp in range(2):
                nc.sync.dma_start(out=f8_v[:, :, tp, :, :],
                                  in_=f8_dv[:, :, tp, :, :])
        g_vs = {}
        for (xp, isf8), gd in g_ds.items():
            if isf8:
                upw = UPS_PAD[xp]
                gt = fpool.tile([PART, C_LOC * 2 * 2 * upw], F8E4,
                                name=f"g{xp}f8_sb", tag=f"g{xp}f8")
                gv = gt[:].rearrange("p (c tp q x) -> p c tp q x",
                                     c=C_LOC, tp=2, q=2)
                nc.sync.dma_start(out=gv, in_=gd[:].rearrange(
                    "p (c tp q x) -> p c tp q x", c=C_LOC, tp=2, q=2))
            else:
                upw = UPS[xp]
                gt = fpool.tile([PART, C_LOC * NT * upw], BF16,
                                name=f"g{xp}_sb", tag=f"g{xp}")
                gv = gt[:].rearrange("p (c t x) -> p c t x", c=C_LOC, t=NT)
                nc.sync.dma_start(out=gv, in_=gd[:].rearrange(
                    "p (c t x) -> p c t x", c=C_LOC, t=NT))
            g_vs[(xp, isf8)] = gv

        evac_busy = [0.0, 0.0]  # DVE, ACT modeled busy ns

        def evac(out_ap, in_ap, idx):
            # GPSIMD cannot read PSUM on real HW: DVE / ACT only.
            free = in_ap.free_size()
            costs = (free * 1.0417 + 125.0, free * 0.8333 + 185.0)
            if evac_pat == "auto" or evac_pat is None:
                which = 0 if evac_busy[0] + costs[0] <= evac_busy[1] + costs[1] \
                    else 1
            else:
                which = {"v": 0, "s": 1}[evac_pat[idx % len(evac_pat)]]
            evac_busy[which] += costs[which]
            if which == 0:
                nc.vector.tensor_copy(out=out_ap, in_=in_ap)
            else:
                nc.scalar.copy(out_ap, in_ap)

        evac_ctr = 0
        # (box, window) slot stream state
        cur_p1 = None          # current psum_t tile
        cur_slots = []         # [(box, win_idx)]
        rhs2_of = {}           # (box, win) -> (tile, slot)
        pend_s2 = []           # boxes whose stage-2 is not yet emitted

        o_sb = None
        psum2 = None

        def flush_p1():
            nonlocal cur_p1, cur_slots, evac_ctr
            if cur_p1 is None:
                return
            r_t = rpool.tile([PART, slots * 256], BF16, name="r_t", tag="r")
            evac(r_t[:], cur_p1[:], evac_ctr)
            evac_ctr += 1
            for si, key in enumerate(cur_slots):
                rhs2_of[key] = (r_t, si)
            cur_p1 = None
            cur_slots = []

        def emit_stage2(n):
            """stage 2 + output for box n (requires rhs2 of all windows)."""
            nonlocal psum2, o_sb, evac_ctr
            p = plans[n]
            g16, s16 = n // 16, n % 16
            oct_, sq = s16 // 8, s16 % 8
            if s16 == 0:
                o_sb = opool.tile([PART, 1024], BF16, name="o_sb", tag="o")
            if sq == 0:
                psum2 = p2pool.tile([PART, 512], FP32, name="ps2", tag="ps2")
            if p is None:
                # write *something* finite so the tile is defined
                for hh in range(2):
                    nc.tensor.matmul(
                        out=psum2[:, sq * 64 + hh * 32: sq * 64 + hh * 32 + 32],
                        lhsT=f_v[:, 0, 0, 0:PART], rhs=f_v[:, 0, 0, 0:OUT],
                        start=True, stop=True)
            else:
                nw = len(p["xws"])
                for hh in range(2):
                    for k in range(nw):
                        r_t, si = rhs2_of[(n, k)]
                        wx_sl = wx_sb_view(n, k)
                        nc.tensor.matmul(
                            out=psum2[:, sq * 64 + hh * 32: sq * 64 + hh * 32 + 32],
                            lhsT=r_t[:, si * 256 + hh * 128: si * 256 + (hh + 1) * 128],
                            rhs=wx_sl,
                            start=(k == 0), stop=(k == nw - 1))
                for k in range(nw):
                    rhs2_of.pop((n, k), None)
            if sq == 7:
                evac(o_sb[:, oct_ * 512:(oct_ + 1) * 512], psum2[:], evac_ctr)
                evac_ctr += 1
            if s16 == 15:
                _eng_map[odma].dma_start(
                    out=out_d[:, g16 * 1024:(g16 + 1) * 1024], in_=o_sb[:])

        def wx_sb_view(n, k):
            t, base = wx_tiles[n // GROUP]
            off = plans[n]["wx_off"] + k * OUT - base
            return t[:, off:off + OUT]

        for g0 in range(0, N_BOXES, GROUP):
            gi = g0 // GROUP
            load_chunks(gi)
            if wyb_rng[gi] is not None:
                wyb_sb, wyb_base = wy_tiles[gi]["b"]
            if wy8_rng[gi] is not None:
                wy8_sb, wy8_base = wy_tiles[gi]["8"]

            for n in range(g0, min(g0 + GROUP, N_BOXES)):
                p = plans[n]
                if p is not None:
                    for k, x0 in enumerate(p["xws"]):
                        if cur_p1 is None:
                            cur_p1 = p1pool.tile([PART, slots * 256], FP32,
                                                 name="ps1", tag="ps1")
                        si = len(cur_slots)
                        cur_slots.append((n, k))
                        base = si * 256
                        if p["fp8"]:
                            src = g_vs[(p["xp"], True)] if p["xp"] > 1 else f8_v
                            off = p["wy8_off"] - wy8_base
                            for c in range(C_LOC):
                                for j, tp in enumerate(p["tp"]):
                                    nc.tensor.matmul(
                                        out=cur_p1[:, base + c * OUT: base + (c + 1) * OUT],
                                        lhsT=src[:, c, tp, :, x0:x0 + PART],
                                        rhs=wy8_sb[:, off + j * 64: off + (j + 1) * 64]
                                            .rearrange("p (q i) -> p q i", q=2),
                                        start=(j == 0), stop=(j == len(p["tp"]) - 1),
                                        perf_mode=DR)
                        else:
                            src = g_vs[(p["xp"], False)] if p["xp"] > 1 else f_v
                            off = p["wyb_off"] - wyb_base
                            for c in range(C_LOC):
                                for j, t in enumerate(p["ts"]):
                                    nc.tensor.matmul(
                                        out=cur_p1[:, base + c * OUT: base + (c + 1) * OUT],
                                        lhsT=src[:, c, t, x0:x0 + PART],
                                        rhs=wyb_sb[:, off + j * OUT: off + (j + 1) * OUT],
                                        start=(j == 0), stop=(j == len(p["ts"]) - 1))
                        if len(cur_slots) == slots:
                            flush_p1()
                pend_s2.append(n)
                # emit stage 2 for boxes whose windows are all evacuated
                while pend_s2:
                    b = pend_s2[0]
                    pb = plans[b]
                    if pb is not None and any(
                            (b, k) not in rhs2_of for k in range(len(pb["xws"]))):
                        break
                    if n - b < s2_lag:
                        break
                    emit_stage2(b)
                    pend_s2.pop(0)
        flush_p1()
        while pend_s2:
            emit_stage2(pend_s2.pop(0))

    return _patch_serialization(nc)


# ---------------------------------------------------------------------------
# Entry point
# ---------------------------------------------------------------------------

_LAST = {}


def kernel(feature_map, boxes, output_width):
    from concourse.bass_utils import run_bass_kernel_spmd
    import ml_dtypes

    feature_map = np.asarray(feature_map, dtype=np.float32)
    boxes_np = np.asarray(boxes, dtype=np.float32)
    assert int(output_width) == OUT

    wy_all, wx_all = host_geometry(boxes_np)
    plans, wyb_flat, wy8_flat, wx_flat = plan_boxes(boxes_np, wy_all, wx_all)
    nc = _build_program(plans, wyb_flat.shape[1], wy8_flat.shape[1],
                        wx_flat.shape[1], **BUILD_KW)

    any_fp8 = any(p is not None and p["fp8"] for p in plans)
    variants = sorted({(p["xp"], p["fp8"]) for p in plans
                       if p is not None and p["xp"] > 1})
    pools_needed = sorted({xp for (xp, _) in variants})
    wyb_u = wyb_flat.astype(ml_dtypes.bfloat16)
    wx_u = wx_flat.astype(ml_dtypes.bfloat16)
    if any_fp8:
        wy8_u = wy8_flat.astype(ml_dtypes.float8_e4m3)
    bases = {xp: xpool_basis(xp) for xp in pools_needed}

    in_maps = []
    for kcore in range(N_CORES):
        # f layout [p, (c, t, x)]: y = t*128 + p
        f_k = feature_map[:, :, kcore * C_LOC:(kcore + 1) * C_LOC]  # [y, x, c]
        f_bf = f_k.astype(ml_dtypes.bfloat16).astype(np.float32)
        f_sb = np.ascontiguousarray(
            f_bf.reshape(NT, PART, W, C_LOC).transpose(1, 3, 0, 2)
        ).reshape(PART, C_LOC * NT * W).astype(ml_dtypes.bfloat16)
        m = {"f": f_sb, "wyb": wyb_u, "wx": wx_u}
        if any_fp8:
            # f8 layout [p, (c, tp, q, x)]: y = tp*256 + q*128 + p
            f8_sb = np.ascontiguousarray(
                f_bf.reshape(2, 2, PART, W, C_LOC).transpose(2, 4, 0, 1, 3)
            ).reshape(PART, C_LOC * 2 * 2 * W).astype(ml_dtypes.float8_e4m3)
            m["f8"] = f8_sb
            m["wy8"] = wy8_u
        g_ks = {xp: np.einsum("xu,yxc->yuc", bases[xp], f_bf, optimize=True)
                for xp in pools_needed}
        for (xp, isf8) in variants:
            g_k = g_ks[xp]
            nm = f"g{xp}{'f8' if isf8 else ''}"
            if isf8:
                upw = UPS_PAD[xp]
                g_pad = np.zeros((H, upw, C_LOC), np.float32)
                g_pad[:, :g_k.shape[1], :] = g_k
                m[nm] = np.ascontiguousarray(
                    g_pad.reshape(2, 2, PART, upw, C_LOC).transpose(2, 4, 0, 1, 3)
                ).reshape(PART, C_LOC * 2 * 2 * upw).astype(ml_dtypes.float8_e4m3)
            else:
                upw = g_k.shape[1]
                m[nm] = np.ascontiguousarray(
                    g_k.reshape(NT, PART, upw, C_LOC).transpose(1, 3, 0, 2)
                ).reshape(PART, C_LOC * NT * upw).astype(ml_dtypes.bfloat16)
        in_maps.append(m)

    _LAST["nc"] = nc
    _LAST["in_maps"] = in_maps
    res = run_bass_kernel_spmd(nc, in_maps, list(range(N_CORES)))

    out = np.zeros((N_BOXES, OUT, OUT, C), np.float32)
    for kcore in range(N_CORES):
        dev = np.asarray(res.results[kcore]["out"]).astype(np.float32)
        # [p, (G, g, hh, j)] with p = c_lh*32 + i, box = 16G + g
        v = dev.reshape(4, OUT, N_BOXES // 16, 16, 2, OUT)  # c_lh, i, G, g, hh, j
        v = v.transpose(2, 3, 1, 5, 4, 0)                   # G, g, i, j, hh, c_lh
        v = v.reshape(N_BOXES, OUT, OUT, C_LOC)
        out[:, :, :, kcore * C_LOC:(kcore + 1) * C_LOC] = v
    for n in range(N_BOXES):
        if plans[n] is None:
            out[n] = 0.0
    return out


def estimate_hw_ns():
    """Cost-model estimate of the per-core kernel duration (ns)."""
    from concourse.timeline_sim import TimelineSim
    nc = _LAST.get("nc")
    if nc is None:
        return -1
    sim = TimelineSim(nc)
    sim.simulate()
    return int(sim.time)


def measure_wall(n=5):
    """Wall-clock of repeated dispatches (includes axon round trips)."""
    import time
    from concourse.bass_utils import run_bass_kernel_spmd
    times = []
    for _ in range(n):
        t0 = time.perf_counter()
        run_bass_kernel_spmd(_LAST["nc"], _LAST["in_maps"], list(range(N_CORES)))
        times.append(time.perf_counter() - t0)
    return times


# revision 58
# speedup vs baseline: 1.1201x; 1.1201x over previous
"""RoIAlign (scale_and_translate, linear, antialias) Trainium2 kernel.

Channel-sharded across 8 NeuronCores: each core keeps a [512, 512, 8]
slice of the feature map resident in SBUF and computes all 512 boxes
for its 8 channels, one box at a time (no box grouping):

  stage 1 (PE):  T^T[x, (c, i)] = sum_y F[y, x, c] * Wy[y, i]
                 one matmul per (c, y-tile, x-window), out free = 32.
                 Wide-kernel boxes (ky*kx >= 12) run fp8e4 DoubleRow
                 (two 128-row y-tiles per matmul, 0.5 cycles/row); the
                 rest run bf16.
  evac:          psum_t [128, 512] (two (box, window) slots) -> SBUF
                 bf16, least-loaded assignment over DVE / ACT (GPSIMD
                 cannot read PSUM).
  stage 2 (PE):  out[(c,i), j] = sum_x T^T[x, ci] * Wx[x, j],
                 operand-swapped (lhsT = T^T chunk, moving = 32 Wx
                 cols) so out free = 32 instead of 256.
  out:           psum2 [128, 512] (8 boxes) -> bf16 staging -> DRAM.

Boxes with wide x-kernels use device-resident 2x/4x x-pooled copies of
the feature map (hat-function pooling; per-box Wx is least-squares
refit onto the hat basis, with delta columns at the image edge). This
shrinks the x-band, cutting stage-1 work and - critically - the
PSUM-evacuation volume, which is the binding engine resource.

Host side computes exact per-box dense resampling weights, extracts
nonzero bands, zero-pads to 128-row tiles/windows, and streams them as
flat [128, cols] arrays in 32-box chunks. fp8 pooled maps are padded
to multiple-of-16 widths (DoubleRow ldweights stride restriction).
"""

import numpy as np

H = 512
W = 512
C = 64
N_BOXES = 512
OUT = 32
N_CORES = 8
C_LOC = C // N_CORES  # 8 channels per core
PART = 128
NT = H // PART        # 4 y partition tiles
GROUP = 32            # boxes per weight-DMA chunk
SLOTS = 4             # (box, window) slots per psum_t tile
FP8_KPROD = 12.0      # use fp8 stage-1 when ky*kx >= this
FP8_KMIN = 2.0        # ... and both ky, kx >= this
XP2_KMIN = 10.0       # use the 2x x-pooled map when kx >= this
XP4_KMIN = 14.0       # use the 4x x-pooled map when kx >= this
WCLIP = 1e-3          # zero fitted pooled weights below this (rel) magnitude
UP2 = W // 2 + 1      # 2x-pooled x size: hats at even x + delta at x=511
UP4 = (W - 1) // 4 + 1 + 3  # 4x pooled: 128 hat nodes + 3 tail deltas = 131
UPS = {2: UP2, 4: UP4}
# fp8 DoubleRow ldweights requires the pair-dim byte stride to be a
# multiple of 16 -> pad the fp8 pooled-map widths up to a multiple of 16
UPS_PAD = {p: (u + 15) // 16 * 16 for p, u in UPS.items()}

# scheduling knobs for _build_program (tuned via TimelineSim)
BUILD_KW = dict(slots=2, p1bufs=6, rbufs=12, evac_pat="auto", wdma="yy",
                odma="y", s2_lag=10)


# ---------------------------------------------------------------------------
# Host-side weight computation (mirrors jax.image.scale_and_translate with
# method="linear", antialias=True)
# ---------------------------------------------------------------------------

def _compute_weight_mat(in_size, out_size, scale, translation):
    inv_scale = 1.0 / scale
    kernel_scale = max(inv_scale, 1.0)
    sample_f = (np.arange(out_size, dtype=np.float64) + 0.5) * inv_scale \
        - translation * inv_scale - 0.5
    x = np.abs(sample_f[None, :] - np.arange(in_size, dtype=np.float64)[:, None]) \
        / kernel_scale
    weights = np.maximum(0.0, 1.0 - x)
    total = weights.sum(axis=0, keepdims=True)
    weights = np.where(
        np.abs(total) > 1000.0 * float(np.finfo(np.float32).eps),
        weights / np.where(total != 0, total, 1.0),
        0.0,
    )
    valid = (sample_f >= -0.5) & (sample_f <= in_size - 0.5)
    return np.where(valid[None, :], weights, 0.0).astype(np.float32)


def host_geometry(boxes):
    """Exact per-box dense weights wy/wx [N, 512, 32] fp32."""
    boxes = np.asarray(boxes, dtype=np.float64)
    wy_all = np.zeros((N_BOXES, H, OUT), np.float32)
    wx_all = np.zeros((N_BOXES, W, OUT), np.float32)
    for n in range(N_BOXES):
        cx, cy, w, h = boxes[n]
        x0 = cx - w / 2
        y0 = cy - h / 2
        w = max(w, 1e-6)
        h = max(h, 1e-6)
        x_scale = OUT / (w * W)
        y_scale = OUT / (h * H)
        wy_all[n] = _compute_weight_mat(H, OUT, y_scale, -y0 * OUT / h)
        wx_all[n] = _compute_weight_mat(W, OUT, x_scale, -x0 * OUT / w)
    return wy_all, wx_all


def xpool_basis(p):
    """Hat basis at stride-p grid nodes plus delta columns for the tail
    pixels past the last node (which a hat grid cannot represent)."""
    nu = (W - 1) // p + 1
    extra = [x for x in range(W) if x > (nu - 1) * p]
    P = np.zeros((W, nu + len(extra)), np.float32)
    for u in range(nu):
        c = p * u
        for x in range(max(0, c - p + 1), min(W, c + p)):
            P[x, u] = 1.0 - abs(x - c) / p
    for j, x in enumerate(extra):
        P[x, :] = 0.0
        P[x, nu + j] = 1.0
    return P


def xpool_fit_mats():
    """Per pooling factor: (P, PINV) with PINV = (P^T P)^-1 P^T."""
    mats = {}
    for p in (2, 4):
        P = xpool_basis(p)
        PINV = np.linalg.solve((P.T @ P).astype(np.float64),
                               P.T.astype(np.float64))
        mats[p] = (P, PINV)
    return mats


def plan_boxes(boxes, wy_all, wx_all):
    """Per-box banded geometry + flat weight packing.

    Returns plan dicts (None for boxes with empty bands) and the packed
    flat weight arrays wyb [128, :] bf16, wy8 [128, :] fp8, wx [128, :]
    bf16 (as float32 here; cast at upload).
    """
    boxes = np.asarray(boxes, dtype=np.float64)
    ky = np.maximum(16.0 * boxes[:, 3], 1.0)  # y kernel halfwidth (px)
    kx = np.maximum(16.0 * boxes[:, 2], 1.0)

    mats = xpool_fit_mats()

    # processing order: boxes whose source map arrives earliest on the
    # input DMA queue go first (f -> f8 -> pooled bf16 -> pooled fp8), so
    # compute covers the input-upload stream instead of stalling on it.
    # Within the first class, boxes needing only low y-tiles go first so
    # PE can start right after the first F-tile DMA lands.
    def klass(n):
        f8 = (ky[n] * kx[n] >= FP8_KPROD
              and ky[n] >= FP8_KMIN and kx[n] >= FP8_KMIN)
        xp = 4 if kx[n] >= XP4_KMIN else (2 if kx[n] >= XP2_KMIN else 1)
        if xp == 1:
            return 0 if not f8 else 1
        return 2 if not f8 else 3

    def last_tile(n):
        nz = np.nonzero(wy_all[n].any(axis=1))[0]
        return (int(nz[-1]) // PART) if len(nz) else 0

    perm = sorted(range(N_BOXES),
                  key=lambda n: (klass(n),
                                 last_tile(n) if klass(n) == 0 else 0, n))

    plans = [None] * N_BOXES
    wyb_cols, wy8_cols, wx_cols = [], [], []
    for n in perm:
        ynz = np.nonzero(wy_all[n].any(axis=1))[0]
        xnz = np.nonzero(wx_all[n].any(axis=1))[0]
        if len(ynz) == 0 or len(xnz) == 0:
            continue
        r0, r1 = int(ynz[0]), int(ynz[-1]) + 1
        use_fp8 = (ky[n] * kx[n] >= FP8_KPROD
                   and ky[n] >= FP8_KMIN and kx[n] >= FP8_KMIN)
        xp = 4 if kx[n] >= XP4_KMIN else (2 if kx[n] >= XP2_KMIN else 1)

        if xp > 1:
            # least-squares fit of Wx in the pooled hat basis, clipping
            # the tiny ringing tail of the fit to keep the band compact
            P, PINV = mats[xp]
            wx_n = (PINV @ wx_all[n].astype(np.float64)).astype(np.float32)
            wx_n[np.abs(wx_n) < WCLIP * np.abs(wx_n).max()] = 0.0
            xnz = np.nonzero(wx_n.any(axis=1))[0]
            if len(xnz) == 0:
                continue
            WW = UPS[xp]
        else:
            wx_n = wx_all[n]
            WW = W
        c0, c1 = int(xnz[0]), int(xnz[-1]) + 1

        # x windows (arbitrary free-dim offset, clamped; overlap zeroed)
        nxw = (c1 - c0 + PART - 1) // PART
        xws = [min(c0 + k * PART, max(WW - PART, 0)) for k in range(nxw)]

        p = {"fp8": use_fp8, "xp": xp, "xws": xws}
        if use_fp8:
            p_lo, p_hi = r0 // (2 * PART), (r1 - 1) // (2 * PART)
            p["tp"] = list(range(p_lo, p_hi + 1))
            p["wy8_off"] = len(wy8_cols) and sum(c.shape[1] for c in wy8_cols)
            p["wy8_off"] = sum(c.shape[1] for c in wy8_cols)
            for tp in p["tp"]:
                # [128, (q, i)] with y = tp*256 + q*128 + p
                blk = np.zeros((PART, 2, OUT), np.float32)
                for q in range(2):
                    blk[:, q, :] = wy_all[n][tp * 256 + q * 128: tp * 256 + (q + 1) * 128]
                wy8_cols.append(blk.reshape(PART, 2 * OUT))
        else:
            t_lo, t_hi = r0 // PART, (r1 - 1) // PART
            p["ts"] = list(range(t_lo, t_hi + 1))
            p["wyb_off"] = sum(c.shape[1] for c in wyb_cols)
            for t in p["ts"]:
                wyb_cols.append(wy_all[n][t * PART:(t + 1) * PART].copy())

        p["wx_off"] = sum(c.shape[1] for c in wx_cols)
        prev_end = c0
        for x0 in xws:
            wxw = wx_n[x0:x0 + PART].copy()
            lo = max(prev_end - x0, 0)
            wxw[:lo] = 0.0
            prev_end = max(prev_end, x0 + PART)
            wx_cols.append(wxw)
        plans[n] = p

    def cat(cols, width):
        if not cols:
            return np.zeros((PART, width), np.float32)
        return np.concatenate(cols, axis=1)

    wyb_flat = cat(wyb_cols, OUT)
    wy8_flat = cat(wy8_cols, 2 * OUT)
    wx_flat = cat(wx_cols, OUT)
    return plans, perm, wyb_flat, wy8_flat, wx_flat


# ---------------------------------------------------------------------------
# Device program
# ---------------------------------------------------------------------------

def _split_multiwait_bir(raw: bytes) -> bytes:
    """The walrus build here accepts only one sync wait per instruction.
    Hoist extra waits onto single-wait EventSemaphore instructions inserted
    just before, on the same engine (per-engine order is preserved)."""
    import orjson

    d = orjson.loads(raw)
    ctr = 0
    for fn in d.get("functions", []):
        for bb in fn.get("blocks") or []:
            out = []
            for ins in bb["instructions"]:
                si = ins.get("sync_info")
                ws = (si or {}).get("on_wait") or []
                if len(ws) > 1:
                    for w in ws[:-1]:
                        ctr += 1
                        out.append({
                            "debug": ins.get("debug", 0),
                            "engine": ins["engine"],
                            "ins": [],
                            "outs": [],
                            "name": f"{ins['name']}-xw{ctr}",
                            "opcode": "EventSemaphore",
                            "sync_info": {"on_update": [], "on_wait": [w]},
                        })
                    si["on_wait"] = [ws[-1]]
                out.append(ins)
            bb["instructions"] = out
    return orjson.dumps(d)


def _patch_serialization(nc):
    orig = nc.to_json_bytes

    def patched():
        return _split_multiwait_bir(orig())

    nc.to_json_bytes = patched
    return nc


def _make_tc_class():
    import concourse.tile as tile
    from concourse.vector_clock import ScopedClock
    import bass_rust

    class TC(tile.TileContext):
        """TileContext with the tail drain's multi-sem wait split into
        individual single-wait instructions (this walrus rejects >1 wait
        on a CTRL instruction)."""

        def _drain_and_barrier(self, tick_clock, wait_clock):
            nc = self.nc
            probe = nc.sync.drain()
            wait_clock.add_sem_waits(
                probe.ins, ScopedClock({None: tick_clock.global_clock})
            )
            waits = list(probe.ins.sync_info.on_wait)
            probe.ins.sync_info = bass_rust.SyncInfo(on_wait=[], on_update=[])
            by_name = {hh.name: hh for hh in self.sems.allocated().values()}
            for wt in waits:
                nc.sync.wait_ge(by_name[wt.ant_name], wt.wait_value)
            nc.all_engine_barrier()
            popped = nc._tile_sem_poison_stack.pop()
            assert popped is self._sem_poison
            nc.clear_and_free_semaphores(list(self.sems.allocated().values()))
            nc.all_engine_barrier()

    return TC


def _build_program(plans, perm, wyb_cols_n, wy8_cols_n, wx_cols_n,
                   slots=SLOTS, p1bufs=2, rbufs=6, p2bufs=2, evac_mod=3,
                   evac_pat=None, wdma="gs", odma="y", s2_lag=0):
    import concourse.bass as bass
    import concourse.mybir as mybir
    from contextlib import ExitStack

    FP32 = mybir.dt.float32
    BF16 = mybir.dt.bfloat16
    F8E4 = mybir.dt.float8e4
    DR = mybir.MatmulPerfMode.DoubleRow

    any_fp8 = any(p is not None and p["fp8"] for p in plans)
    # pooled-map variants needed: (pool factor, fp8?)
    variants = sorted({(p["xp"], p["fp8"]) for p in plans
                       if p is not None and p["xp"] > 1})

    nc = bass.Bass()
    # feature map, already in SBUF layout [128, (c, t, x)] / [128, (c, tp, q, x)]
    f_d = nc.dram_tensor("f", [PART, C_LOC * NT * W], BF16, kind="ExternalInput")
    if any_fp8:
        f8_d = nc.dram_tensor("f8", [PART, C_LOC * 2 * 2 * W], F8E4,
                              kind="ExternalInput")
    g_ds = {}
    for (xp, isf8) in variants:
        nm = f"g{xp}{'f8' if isf8 else ''}"
        if isf8:
            g_ds[(xp, True)] = nc.dram_tensor(
                nm, [PART, C_LOC * 2 * 2 * UPS_PAD[xp]], F8E4,
                kind="ExternalInput")
        else:
            g_ds[(xp, False)] = nc.dram_tensor(
                nm, [PART, C_LOC * NT * UPS[xp]], BF16, kind="ExternalInput")
    wyb_d = nc.dram_tensor("wyb", [PART, max(wyb_cols_n, OUT)], BF16,
                           kind="ExternalInput")
    if any_fp8:
        wy8_d = nc.dram_tensor("wy8", [PART, max(wy8_cols_n, 2 * OUT)], F8E4,
                               kind="ExternalInput")
    wx_d = nc.dram_tensor("wx", [PART, max(wx_cols_n, OUT)], BF16,
                          kind="ExternalInput")
    # output: box b = 16*G + g: cols G*1024 + g*64 + hh*32 + j,
    # partition p = c_lh*32 + i, channel c = hh*4 + c_lh
    NG16 = (N_BOXES + 15) // 16
    if odma == "P":
        out_d = nc.dram_tensor("out", [PART, (N_BOXES // 4) * 256], FP32,
                               kind="ExternalOutput")
    else:
        out_d = nc.dram_tensor("out", [PART, NG16 * 1024], BF16,
                               kind="ExternalOutput")

    # per-GROUP chunk column ranges (groups are processing positions)
    def group_ranges(key, width):
        rng = []
        for g0 in range(0, N_BOXES, GROUP):
            los, his = [], []
            for n in perm[g0:g0 + GROUP]:
                p = plans[n]
                if p is None or key not in p:
                    continue
                ntiles = len(p["ts"]) if key == "wyb_off" else (
                    len(p["tp"]) if key == "wy8_off" else len(p["xws"]))
                los.append(p[key])
                his.append(p[key] + ntiles * width)
            rng.append((min(los), max(his)) if los else None)
        return rng

    wyb_rng = group_ranges("wyb_off", OUT)
    wy8_rng = group_ranges("wy8_off", 2 * OUT)
    wx_rng = group_ranges("wx_off", OUT)

    TC = _make_tc_class()
    with TC(nc) as tc, ExitStack() as ctx:
        fpool = ctx.enter_context(tc.tile_pool(name="fmap", bufs=1))
        wpool = ctx.enter_context(tc.tile_pool(name="wts", bufs=2))
        rpool = ctx.enter_context(tc.tile_pool(name="rhs2", bufs=rbufs))
        opool = ctx.enter_context(tc.tile_pool(name="osb", bufs=2))
        p1pool = ctx.enter_context(tc.tile_pool(name="psumT", bufs=p1bufs, space="PSUM"))
        p2pool = ctx.enter_context(tc.tile_pool(name="psum2", bufs=p2bufs, space="PSUM"))

        _eng_map = {"g": nc.gpsimd, "s": nc.scalar, "y": nc.sync}

        def _wy_eng():
            return _eng_map[wdma[0]]

        def _wx_eng():
            return _eng_map[wdma[1]]

        # weight chunk state
        wy_tiles = {}  # group index -> {"b": (tile, base), "8": (tile, base)}
        wx_tiles = {}  # group index -> (tile, base)

        def load_chunks(gi):
            if gi in wy_tiles:
                return
            wy_tiles[gi] = {}
            if wyb_rng[gi] is not None:
                lo, hi = wyb_rng[gi]
                tb = wpool.tile([PART, hi - lo], BF16, name="wyb_sb", tag="wyb")
                _wy_eng().dma_start(out=tb[:], in_=wyb_d[:, lo:hi])
                wy_tiles[gi]["b"] = (tb, lo)
            if wy8_rng[gi] is not None:
                lo, hi = wy8_rng[gi]
                t8 = wpool.tile([PART, hi - lo], F8E4, name="wy8_sb", tag="wy8")
                _wy_eng().dma_start(out=t8[:], in_=wy8_d[:, lo:hi])
                wy_tiles[gi]["8"] = (t8, lo)
            if wx_rng[gi] is not None:
                lo, hi = wx_rng[gi]
                tx = wpool.tile([PART, hi - lo], BF16, name="wx_sb", tag="wx")
                _wx_eng().dma_start(out=tx[:], in_=wx_d[:, lo:hi])
                wx_tiles[gi] = (tx, lo)

        # group-0 weights first so PE can start as soon as the first F
        # tiles land (same FIFO queue as the F DMAs below)
        load_chunks(0)

        f_sb = fpool.tile([PART, C_LOC * NT * W], BF16, name="f_sb")
        f_v = f_sb[:].rearrange("p (c t x) -> p c t x", c=C_LOC, t=NT)
        f_dv = f_d[:].rearrange("p (c t x) -> p c t x", c=C_LOC, t=NT)
        for t in range(NT):
            nc.sync.dma_start(out=f_v[:, :, t, :], in_=f_dv[:, :, t, :])

        # f8 / pooled-map uploads are deferred to just before the first
        # processing group that uses them, so the (FIFO) input queue
        # delivers weights and maps in need order instead of stalling
        # later weight chunks behind maps nobody needs yet.
        map_dmas = {}  # key -> emit thunk
        first_grp = {}  # key -> first processing group using the map

        def _key_of(p):
            if p["xp"] > 1:
                return (p["xp"], p["fp8"])
            return "f8" if p["fp8"] else None

        for pos, n in enumerate(perm):
            p = plans[n]
            if p is None:
                continue
            kkey = _key_of(p)
            if kkey is not None and kkey not in first_grp:
                first_grp[kkey] = pos // GROUP

        if any_fp8:
            f8_sb = fpool.tile([PART, C_LOC * 2 * 2 * W], F8E4, name="f8_sb")
            f8_v = f8_sb[:].rearrange("p (c tp q x) -> p c tp q x",
                                      c=C_LOC, tp=2, q=2)
            f8_dv = f8_d[:].rearrange("p (c tp q x) -> p c tp q x",
                                      c=C_LOC, tp=2, q=2)

            def _emit_f8(f8_v=f8_v, f8_dv=f8_dv):
                for tp in range(2):
                    nc.sync.dma_start(out=f8_v[:, :, tp, :, :],
                                      in_=f8_dv[:, :, tp, :, :])
            map_dmas["f8"] = _emit_f8
        g_vs = {}
        for (xp, isf8), gd in g_ds.items():
            if isf8:
                upw = UPS_PAD[xp]
                gt = fpool.tile([PART, C_LOC * 2 * 2 * upw], F8E4,
                                name=f"g{xp}f8_sb", tag=f"g{xp}f8")
                gv = gt[:].rearrange("p (c tp q x) -> p c tp q x",
                                     c=C_LOC, tp=2, q=2)

                def _emit(gv=gv, gd=gd):
                    nc.sync.dma_start(out=gv, in_=gd[:].rearrange(
                        "p (c tp q x) -> p c tp q x", c=C_LOC, tp=2, q=2))
            else:
                upw = UPS[xp]
                gt = fpool.tile([PART, C_LOC * NT * upw], BF16,
                                name=f"g{xp}_sb", tag=f"g{xp}")
                gv = gt[:].rearrange("p (c t x) -> p c t x", c=C_LOC, t=NT)

                def _emit(gv=gv, gd=gd):
                    nc.sync.dma_start(out=gv, in_=gd[:].rearrange(
                        "p (c t x) -> p c t x", c=C_LOC, t=NT))
            map_dmas[(xp, isf8)] = _emit
            g_vs[(xp, isf8)] = gv

        def emit_maps_due(gi):
            """Emit deferred map DMAs needed by group gi+1 (1-group lead)."""
            for kkey, thunk in list(map_dmas.items()):
                if first_grp.get(kkey, 0) <= gi + 1:
                    thunk()
                    del map_dmas[kkey]
        emit_maps_due(0)

        evac_busy = [0.0, 0.0]  # DVE, ACT modeled busy ns

        def evac(out_ap, in_ap, idx):
            # GPSIMD cannot read PSUM on real HW: DVE / ACT only.
            free = in_ap.free_size()
            costs = (free * 1.0417 + 125.0, free * 0.8333 + 185.0)
            if evac_pat == "auto" or evac_pat is None:
                which = 0 if evac_busy[0] + costs[0] <= evac_busy[1] + costs[1] \
                    else 1
            else:
                which = {"v": 0, "s": 1}[evac_pat[idx % len(evac_pat)]]
            evac_busy[which] += costs[which]
            if which == 0:
                nc.vector.tensor_copy(out=out_ap, in_=in_ap)
            else:
                nc.scalar.copy(out_ap, in_ap)

        evac_ctr = 0
        # (box, window) slot stream state
        cur_p1 = None          # current psum_t tile
        cur_slots = []         # [(box, win_idx)]
        rhs2_of = {}           # (box, win) -> (tile, slot)
        pend_s2 = []           # boxes whose stage-2 is not yet emitted

        o_sb = None
        psum2 = None

        def flush_p1():
            nonlocal cur_p1, cur_slots, evac_ctr
            if cur_p1 is None:
                return
            r_t = rpool.tile([PART, slots * 256], BF16, name="r_t", tag="r")
            evac(r_t[:], cur_p1[:], evac_ctr)
            evac_ctr += 1
            for si, key in enumerate(cur_slots):
                rhs2_of[key] = (r_t, si)
            cur_p1 = None
            cur_slots = []

        def emit_stage2(n):
            """stage 2 + output for box n (requires rhs2 of all windows).
            Output slots are by processing position; host unpermutes."""
            nonlocal psum2, o_sb, evac_ctr
            p = plans[n]
            pos = pos_of[n]
            g16, s16 = pos // 16, pos % 16
            oct_, sq = s16 // 8, s16 % 8
            if s16 == 0:
                o_sb = opool.tile([PART, 1024], BF16, name="o_sb", tag="o")
            if sq == 0:
                psum2 = p2pool.tile([PART, 512], FP32, name="ps2", tag="ps2")
            if p is None:
                # write *something* finite so the tile is defined
                for hh in range(2):
                    nc.tensor.matmul(
                        out=psum2[:, sq * 64 + hh * 32: sq * 64 + hh * 32 + 32],
                        lhsT=f_v[:, 0, 0, 0:PART], rhs=f_v[:, 0, 0, 0:OUT],
                        start=True, stop=True)
            else:
                nw = len(p["xws"])
                for hh in range(2):
                    for k in range(nw):
                        r_t, si = rhs2_of[(n, k)]
                        wx_sl = wx_sb_view(n, k)
                        nc.tensor.matmul(
                            out=psum2[:, sq * 64 + hh * 32: sq * 64 + hh * 32 + 32],
                            lhsT=r_t[:, si * 256 + hh * 128: si * 256 + (hh + 1) * 128],
                            rhs=wx_sl,
                            start=(k == 0), stop=(k == nw - 1))
                for k in range(nw):
                    rhs2_of.pop((n, k), None)
            if sq == 7:
                evac(o_sb[:, oct_ * 512:(oct_ + 1) * 512], psum2[:], evac_ctr)
                evac_ctr += 1
            if s16 == 15:
                _eng_map[odma].dma_start(
                    out=out_d[:, g16 * 1024:(g16 + 1) * 1024], in_=o_sb[:])

        pos_of = {n: i for i, n in enumerate(perm)}

        def wx_sb_view(n, k):
            t, base = wx_tiles[pos_of[n] // GROUP]
            off = plans[n]["wx_off"] + k * OUT - base
            return t[:, off:off + OUT]

        for g0 in range(0, N_BOXES, GROUP):
            gi = g0 // GROUP
            emit_maps_due(gi)
            load_chunks(gi)
            if wyb_rng[gi] is not None:
                wyb_sb, wyb_base = wy_tiles[gi]["b"]
            if wy8_rng[gi] is not None:
                wy8_sb, wy8_base = wy_tiles[gi]["8"]

            for n in perm[g0:g0 + GROUP]:
                p = plans[n]
                if p is not None:
                    for k, x0 in enumerate(p["xws"]):
                        if cur_p1 is None:
                            cur_p1 = p1pool.tile([PART, slots * 256], FP32,
                                                 name="ps1", tag="ps1")
                        si = len(cur_slots)
                        cur_slots.append((n, k))
                        base = si * 256
                        if p["fp8"]:
                            src = g_vs[(p["xp"], True)] if p["xp"] > 1 else f8_v
                            off = p["wy8_off"] - wy8_base
                            for c in range(C_LOC):
                                for j, tp in enumerate(p["tp"]):
                                    nc.tensor.matmul(
                                        out=cur_p1[:, base + c * OUT: base + (c + 1) * OUT],
                                        lhsT=src[:, c, tp, :, x0:x0 + PART],
                                        rhs=wy8_sb[:, off + j * 64: off + (j + 1) * 64]
                                            .rearrange("p (q i) -> p q i", q=2),
                                        start=(j == 0), stop=(j == len(p["tp"]) - 1),
                                        perf_mode=DR)
                        else:
                            src = g_vs[(p["xp"], False)] if p["xp"] > 1 else f_v
                            off = p["wyb_off"] - wyb_base
                            for c in range(C_LOC):
                                for j, t in enumerate(p["ts"]):
                                    nc.tensor.matmul(
                                        out=cur_p1[:, base + c * OUT: base + (c + 1) * OUT],
                                        lhsT=src[:, c, t, x0:x0 + PART],
                                        rhs=wyb_sb[:, off + j * OUT: off + (j + 1) * OUT],
                                        start=(j == 0), stop=(j == len(p["ts"]) - 1))
                        if len(cur_slots) == slots:
                            flush_p1()
                pend_s2.append(n)
                # emit stage 2 for boxes whose windows are all evacuated
                while pend_s2:
                    b = pend_s2[0]
                    pb = plans[b]
                    if pb is not None and any(
                            (b, k) not in rhs2_of for k in range(len(pb["xws"]))):
                        break
                    if pos_of[n] - pos_of[b] < s2_lag:
                        break
                    emit_stage2(b)
                    pend_s2.pop(0)
        flush_p1()
        while pend_s2:
            emit_stage2(pend_s2.pop(0))

    return _patch_serialization(nc)


# ---------------------------------------------------------------------------
# Entry point
# ---------------------------------------------------------------------------

_LAST = {}


def kernel(feature_map, boxes, output_width):
    from concourse.bass_utils import run_bass_kernel_spmd
    import ml_dtypes

    feature_map = np.asarray(feature_map, dtype=np.float32)
    boxes_np = np.asarray(boxes, dtype=np.float32)
    assert int(output_width) == OUT

    wy_all, wx_all = host_geometry(boxes_np)
    plans, perm, wyb_flat, wy8_flat, wx_flat = plan_boxes(
        boxes_np, wy_all, wx_all)
    nc = _build_program(plans, perm, wyb_flat.shape[1], wy8_flat.shape[1],
                        wx_flat.shape[1], **BUILD_KW)

    any_fp8 = any(p is not None and p["fp8"] for p in plans)
    variants = sorted({(p["xp"], p["fp8"]) for p in plans
                       if p is not None and p["xp"] > 1})
    pools_needed = sorted({xp for (xp, _) in variants})
    wyb_u = wyb_flat.astype(ml_dtypes.bfloat16)
    wx_u = wx_flat.astype(ml_dtypes.bfloat16)
    if any_fp8:
        wy8_u = wy8_flat.astype(ml_dtypes.float8_e4m3)
    bases = {xp: xpool_basis(xp) for xp in pools_needed}

    in_maps = []
    for kcore in range(N_CORES):
        # f layout [p, (c, t, x)]: y = t*128 + p
        f_k = feature_map[:, :, kcore * C_LOC:(kcore + 1) * C_LOC]  # [y, x, c]
        f_bf = f_k.astype(ml_dtypes.bfloat16).astype(np.float32)
        f_sb = np.ascontiguousarray(
            f_bf.reshape(NT, PART, W, C_LOC).transpose(1, 3, 0, 2)
        ).reshape(PART, C_LOC * NT * W).astype(ml_dtypes.bfloat16)
        m = {"f": f_sb, "wyb": wyb_u, "wx": wx_u}
        if any_fp8:
            # f8 layout [p, (c, tp, q, x)]: y = tp*256 + q*128 + p
            f8_sb = np.ascontiguousarray(
                f_bf.reshape(2, 2, PART, W, C_LOC).transpose(2, 4, 0, 1, 3)
            ).reshape(PART, C_LOC * 2 * 2 * W).astype(ml_dtypes.float8_e4m3)
            m["f8"] = f8_sb
            m["wy8"] = wy8_u
        g_ks = {xp: np.einsum("xu,yxc->yuc", bases[xp], f_bf, optimize=True)
                for xp in pools_needed}
        for (xp, isf8) in variants:
            g_k = g_ks[xp]
            nm = f"g{xp}{'f8' if isf8 else ''}"
            if isf8:
                upw = UPS_PAD[xp]
                g_pad = np.zeros((H, upw, C_LOC), np.float32)
                g_pad[:, :g_k.shape[1], :] = g_k
                m[nm] = np.ascontiguousarray(
                    g_pad.reshape(2, 2, PART, upw, C_LOC).transpose(2, 4, 0, 1, 3)
                ).reshape(PART, C_LOC * 2 * 2 * upw).astype(ml_dtypes.float8_e4m3)
            else:
                upw = g_k.shape[1]
                m[nm] = np.ascontiguousarray(
                    g_k.reshape(NT, PART, upw, C_LOC).transpose(1, 3, 0, 2)
                ).reshape(PART, C_LOC * NT * upw).astype(ml_dtypes.bfloat16)
        in_maps.append(m)

    _LAST["nc"] = nc
    _LAST["in_maps"] = in_maps
    res = run_bass_kernel_spmd(nc, in_maps, list(range(N_CORES)))

    out = np.zeros((N_BOXES, OUT, OUT, C), np.float32)
    perm_np = np.asarray(perm)
    for kcore in range(N_CORES):
        dev = np.asarray(res.results[kcore]["out"]).astype(np.float32)
        # [p, (G, g, hh, j)] with p = c_lh*32 + i, slot = 16G + g =
        # processing position; unpermute to original box order
        v = dev.reshape(4, OUT, N_BOXES // 16, 16, 2, OUT)  # c_lh, i, G, g, hh, j
        v = v.transpose(2, 3, 1, 5, 4, 0)                   # G, g, i, j, hh, c_lh
        v = v.reshape(N_BOXES, OUT, OUT, C_LOC)
        out[perm_np, :, :, kcore * C_LOC:(kcore + 1) * C_LOC] = v
    for n in range(N_BOXES):
        if plans[n] is None:
            out[n] = 0.0
    return out


def estimate_hw_ns():
    """Cost-model estimate of the per-core kernel duration (ns)."""
    from concourse.timeline_sim import TimelineSim
    nc = _LAST.get("nc")
    if nc is None:
        return -1
    sim = TimelineSim(nc)
    sim.simulate()
    return int(sim.time)


def measure_wall(n=5):
    """Wall-clock of repeated dispatches (includes axon round trips)."""
    import time
    from concourse.bass_utils import run_bass_kernel_spmd
    times = []
    for _ in range(n):
        t0 = time.perf_counter()
        run_bass_kernel_spmd(_LAST["nc"], _LAST["in_maps"], list(range(N_CORES)))
        times.append(time.perf_counter() - t0)
    return times


# revision 59
# speedup vs baseline: 1.1475x; 1.0245x over previous
"""RoIAlign (scale_and_translate, linear, antialias) Trainium2 kernel.

Channel-sharded across 8 NeuronCores: each core keeps a [512, 512, 8]
slice of the feature map resident in SBUF and computes all 512 boxes
for its 8 channels, one box at a time (no box grouping):

  stage 1 (PE):  T^T[x, (c, i)] = sum_y F[y, x, c] * Wy[y, i]
                 one matmul per (c, y-tile, x-window), out free = 32.
                 Wide-kernel boxes (ky*kx >= 12) run fp8e4 DoubleRow
                 (two 128-row y-tiles per matmul, 0.5 cycles/row); the
                 rest run bf16.
  evac:          psum_t [128, 512] (two (box, window) slots) -> SBUF
                 bf16, least-loaded assignment over DVE / ACT (GPSIMD
                 cannot read PSUM).
  stage 2 (PE):  out[(c,i), j] = sum_x T^T[x, ci] * Wx[x, j],
                 operand-swapped (lhsT = T^T chunk, moving = 32 Wx
                 cols) so out free = 32 instead of 256.
  out:           psum2 [128, 512] (8 boxes) -> bf16 staging -> DRAM.

Boxes with wide x-kernels use device-resident 2x/4x x-pooled copies of
the feature map (hat-function pooling; per-box Wx is least-squares
refit onto the hat basis, with delta columns at the image edge). This
shrinks the x-band, cutting stage-1 work and - critically - the
PSUM-evacuation volume, which is the binding engine resource.

Host side computes exact per-box dense resampling weights, extracts
nonzero bands, zero-pads to 128-row tiles/windows, and streams them as
flat [128, cols] arrays in 32-box chunks. fp8 pooled maps are padded
to multiple-of-16 widths (DoubleRow ldweights stride restriction).
"""

import numpy as np

H = 512
W = 512
C = 64
N_BOXES = 512
OUT = 32
N_CORES = 8
C_LOC = C // N_CORES  # 8 channels per core
PART = 128
NT = H // PART        # 4 y partition tiles
GROUP = 32            # boxes per weight-DMA chunk
SLOTS = 4             # (box, window) slots per psum_t tile
FP8_KPROD = 12.0      # use fp8 stage-1 when ky*kx >= this
FP8_KMIN = 2.0        # ... and both ky, kx >= this
XP2_KMIN = 10.0       # use the 2x x-pooled map when kx >= this
XP4_KMIN = 13.0       # use the 4x x-pooled map when kx >= this
WCLIP = 1e-3          # zero fitted pooled weights below this (rel) magnitude
UP2 = W // 2 + 1      # 2x-pooled x size: hats at even x + delta at x=511
UP4 = (W - 1) // 4 + 1 + 3  # 4x pooled: 128 hat nodes + 3 tail deltas = 131
UPS = {2: UP2, 4: UP4}
# fp8 DoubleRow ldweights requires the pair-dim byte stride to be a
# multiple of 16 -> pad the fp8 pooled-map widths up to a multiple of 16
UPS_PAD = {p: (u + 15) // 16 * 16 for p, u in UPS.items()}

# scheduling knobs for _build_program (tuned via TimelineSim)
BUILD_KW = dict(slots=2, p1bufs=6, rbufs=12, evac_pat="auto", wdma="yy",
                odma="y", s2_lag=10)


# ---------------------------------------------------------------------------
# Host-side weight computation (mirrors jax.image.scale_and_translate with
# method="linear", antialias=True)
# ---------------------------------------------------------------------------

def _compute_weight_mat(in_size, out_size, scale, translation):
    inv_scale = 1.0 / scale
    kernel_scale = max(inv_scale, 1.0)
    sample_f = (np.arange(out_size, dtype=np.float64) + 0.5) * inv_scale \
        - translation * inv_scale - 0.5
    x = np.abs(sample_f[None, :] - np.arange(in_size, dtype=np.float64)[:, None]) \
        / kernel_scale
    weights = np.maximum(0.0, 1.0 - x)
    total = weights.sum(axis=0, keepdims=True)
    weights = np.where(
        np.abs(total) > 1000.0 * float(np.finfo(np.float32).eps),
        weights / np.where(total != 0, total, 1.0),
        0.0,
    )
    valid = (sample_f >= -0.5) & (sample_f <= in_size - 0.5)
    return np.where(valid[None, :], weights, 0.0).astype(np.float32)


def host_geometry(boxes):
    """Exact per-box dense weights wy/wx [N, 512, 32] fp32."""
    boxes = np.asarray(boxes, dtype=np.float64)
    wy_all = np.zeros((N_BOXES, H, OUT), np.float32)
    wx_all = np.zeros((N_BOXES, W, OUT), np.float32)
    for n in range(N_BOXES):
        cx, cy, w, h = boxes[n]
        x0 = cx - w / 2
        y0 = cy - h / 2
        w = max(w, 1e-6)
        h = max(h, 1e-6)
        x_scale = OUT / (w * W)
        y_scale = OUT / (h * H)
        wy_all[n] = _compute_weight_mat(H, OUT, y_scale, -y0 * OUT / h)
        wx_all[n] = _compute_weight_mat(W, OUT, x_scale, -x0 * OUT / w)
    return wy_all, wx_all


def xpool_basis(p):
    """Hat basis at stride-p grid nodes plus delta columns for the tail
    pixels past the last node (which a hat grid cannot represent)."""
    nu = (W - 1) // p + 1
    extra = [x for x in range(W) if x > (nu - 1) * p]
    P = np.zeros((W, nu + len(extra)), np.float32)
    for u in range(nu):
        c = p * u
        for x in range(max(0, c - p + 1), min(W, c + p)):
            P[x, u] = 1.0 - abs(x - c) / p
    for j, x in enumerate(extra):
        P[x, :] = 0.0
        P[x, nu + j] = 1.0
    return P


def xpool_fit_mats():
    """Per pooling factor: (P, PINV) with PINV = (P^T P)^-1 P^T."""
    mats = {}
    for p in (2, 4):
        P = xpool_basis(p)
        PINV = np.linalg.solve((P.T @ P).astype(np.float64),
                               P.T.astype(np.float64))
        mats[p] = (P, PINV)
    return mats


def plan_boxes(boxes, wy_all, wx_all):
    """Per-box banded geometry + flat weight packing.

    Returns plan dicts (None for boxes with empty bands) and the packed
    flat weight arrays wyb [128, :] bf16, wy8 [128, :] fp8, wx [128, :]
    bf16 (as float32 here; cast at upload).
    """
    boxes = np.asarray(boxes, dtype=np.float64)
    ky = np.maximum(16.0 * boxes[:, 3], 1.0)  # y kernel halfwidth (px)
    kx = np.maximum(16.0 * boxes[:, 2], 1.0)

    mats = xpool_fit_mats()

    # processing order: boxes whose source map arrives earliest on the
    # input DMA queue go first (f -> f8 -> pooled bf16 -> pooled fp8), so
    # compute covers the input-upload stream instead of stalling on it.
    # Within the first class, boxes needing only low y-tiles go first so
    # PE can start right after the first F-tile DMA lands.
    def klass(n):
        f8 = (ky[n] * kx[n] >= FP8_KPROD
              and ky[n] >= FP8_KMIN and kx[n] >= FP8_KMIN)
        xp = 4 if kx[n] >= XP4_KMIN else (2 if kx[n] >= XP2_KMIN else 1)
        if xp == 1:
            return 0 if not f8 else 1
        return 2 if not f8 else 3

    def last_tile(n):
        nz = np.nonzero(wy_all[n].any(axis=1))[0]
        return (int(nz[-1]) // PART) if len(nz) else 0

    perm = sorted(range(N_BOXES),
                  key=lambda n: (klass(n),
                                 last_tile(n) if klass(n) == 0 else 0, n))

    plans = [None] * N_BOXES
    wyb_cols, wy8_cols, wx_cols = [], [], []
    for n in perm:
        ynz = np.nonzero(wy_all[n].any(axis=1))[0]
        xnz = np.nonzero(wx_all[n].any(axis=1))[0]
        if len(ynz) == 0 or len(xnz) == 0:
            continue
        r0, r1 = int(ynz[0]), int(ynz[-1]) + 1
        use_fp8 = (ky[n] * kx[n] >= FP8_KPROD
                   and ky[n] >= FP8_KMIN and kx[n] >= FP8_KMIN)
        xp = 4 if kx[n] >= XP4_KMIN else (2 if kx[n] >= XP2_KMIN else 1)

        if xp > 1:
            # least-squares fit of Wx in the pooled hat basis, clipping
            # the tiny ringing tail of the fit to keep the band compact
            P, PINV = mats[xp]
            wx_n = (PINV @ wx_all[n].astype(np.float64)).astype(np.float32)
            wx_n[np.abs(wx_n) < WCLIP * np.abs(wx_n).max()] = 0.0
            xnz = np.nonzero(wx_n.any(axis=1))[0]
            if len(xnz) == 0:
                continue
            WW = UPS[xp]
        else:
            wx_n = wx_all[n]
            WW = W
        c0, c1 = int(xnz[0]), int(xnz[-1]) + 1

        # x windows (arbitrary free-dim offset, clamped; overlap zeroed)
        nxw = (c1 - c0 + PART - 1) // PART
        xws = [min(c0 + k * PART, max(WW - PART, 0)) for k in range(nxw)]

        p = {"fp8": use_fp8, "xp": xp, "xws": xws}
        if use_fp8:
            p_lo, p_hi = r0 // (2 * PART), (r1 - 1) // (2 * PART)
            p["tp"] = list(range(p_lo, p_hi + 1))
            p["wy8_off"] = len(wy8_cols) and sum(c.shape[1] for c in wy8_cols)
            p["wy8_off"] = sum(c.shape[1] for c in wy8_cols)
            for tp in p["tp"]:
                # [128, (q, i)] with y = tp*256 + q*128 + p
                blk = np.zeros((PART, 2, OUT), np.float32)
                for q in range(2):
                    blk[:, q, :] = wy_all[n][tp * 256 + q * 128: tp * 256 + (q + 1) * 128]
                wy8_cols.append(blk.reshape(PART, 2 * OUT))
        else:
            t_lo, t_hi = r0 // PART, (r1 - 1) // PART
            p["ts"] = list(range(t_lo, t_hi + 1))
            p["wyb_off"] = sum(c.shape[1] for c in wyb_cols)
            for t in p["ts"]:
                wyb_cols.append(wy_all[n][t * PART:(t + 1) * PART].copy())

        p["wx_off"] = sum(c.shape[1] for c in wx_cols)
        prev_end = c0
        for x0 in xws:
            wxw = wx_n[x0:x0 + PART].copy()
            lo = max(prev_end - x0, 0)
            wxw[:lo] = 0.0
            prev_end = max(prev_end, x0 + PART)
            wx_cols.append(wxw)
        plans[n] = p

    def cat(cols, width):
        if not cols:
            return np.zeros((PART, width), np.float32)
        return np.concatenate(cols, axis=1)

    wyb_flat = cat(wyb_cols, OUT)
    wy8_flat = cat(wy8_cols, 2 * OUT)
    wx_flat = cat(wx_cols, OUT)
    return plans, perm, wyb_flat, wy8_flat, wx_flat


# ---------------------------------------------------------------------------
# Device program
# ---------------------------------------------------------------------------

def _split_multiwait_bir(raw: bytes) -> bytes:
    """The walrus build here accepts only one sync wait per instruction.
    Hoist extra waits onto single-wait EventSemaphore instructions inserted
    just before, on the same engine (per-engine order is preserved)."""
    import orjson

    d = orjson.loads(raw)
    ctr = 0
    for fn in d.get("functions", []):
        for bb in fn.get("blocks") or []:
            out = []
            for ins in bb["instructions"]:
                si = ins.get("sync_info")
                ws = (si or {}).get("on_wait") or []
                if len(ws) > 1:
                    for w in ws[:-1]:
                        ctr += 1
                        out.append({
                            "debug": ins.get("debug", 0),
                            "engine": ins["engine"],
                            "ins": [],
                            "outs": [],
                            "name": f"{ins['name']}-xw{ctr}",
                            "opcode": "EventSemaphore",
                            "sync_info": {"on_update": [], "on_wait": [w]},
                        })
                    si["on_wait"] = [ws[-1]]
                out.append(ins)
            bb["instructions"] = out
    return orjson.dumps(d)


def _patch_serialization(nc):
    orig = nc.to_json_bytes

    def patched():
        return _split_multiwait_bir(orig())

    nc.to_json_bytes = patched
    return nc


def _make_tc_class():
    import concourse.tile as tile
    from concourse.vector_clock import ScopedClock
    import bass_rust

    class TC(tile.TileContext):
        """TileContext with the tail drain's multi-sem wait split into
        individual single-wait instructions (this walrus rejects >1 wait
        on a CTRL instruction)."""

        def _drain_and_barrier(self, tick_clock, wait_clock):
            nc = self.nc
            probe = nc.sync.drain()
            wait_clock.add_sem_waits(
                probe.ins, ScopedClock({None: tick_clock.global_clock})
            )
            waits = list(probe.ins.sync_info.on_wait)
            probe.ins.sync_info = bass_rust.SyncInfo(on_wait=[], on_update=[])
            by_name = {hh.name: hh for hh in self.sems.allocated().values()}
            for wt in waits:
                nc.sync.wait_ge(by_name[wt.ant_name], wt.wait_value)
            nc.all_engine_barrier()
            popped = nc._tile_sem_poison_stack.pop()
            assert popped is self._sem_poison
            nc.clear_and_free_semaphores(list(self.sems.allocated().values()))
            nc.all_engine_barrier()

    return TC


def _build_program(plans, perm, wyb_cols_n, wy8_cols_n, wx_cols_n,
                   slots=SLOTS, p1bufs=2, rbufs=6, p2bufs=2, evac_mod=3,
                   evac_pat=None, wdma="gs", odma="y", s2_lag=0):
    import concourse.bass as bass
    import concourse.mybir as mybir
    from contextlib import ExitStack

    FP32 = mybir.dt.float32
    BF16 = mybir.dt.bfloat16
    F8E4 = mybir.dt.float8e4
    DR = mybir.MatmulPerfMode.DoubleRow

    any_fp8 = any(p is not None and p["fp8"] for p in plans)
    # pooled-map variants needed: (pool factor, fp8?)
    variants = sorted({(p["xp"], p["fp8"]) for p in plans
                       if p is not None and p["xp"] > 1})

    nc = bass.Bass()
    # feature map, already in SBUF layout [128, (c, t, x)] / [128, (c, tp, q, x)]
    f_d = nc.dram_tensor("f", [PART, C_LOC * NT * W], BF16, kind="ExternalInput")
    if any_fp8:
        f8_d = nc.dram_tensor("f8", [PART, C_LOC * 2 * 2 * W], F8E4,
                              kind="ExternalInput")
    g_ds = {}
    for (xp, isf8) in variants:
        nm = f"g{xp}{'f8' if isf8 else ''}"
        if isf8:
            g_ds[(xp, True)] = nc.dram_tensor(
                nm, [PART, C_LOC * 2 * 2 * UPS_PAD[xp]], F8E4,
                kind="ExternalInput")
        else:
            g_ds[(xp, False)] = nc.dram_tensor(
                nm, [PART, C_LOC * NT * UPS[xp]], BF16, kind="ExternalInput")
    wyb_d = nc.dram_tensor("wyb", [PART, max(wyb_cols_n, OUT)], BF16,
                           kind="ExternalInput")
    if any_fp8:
        wy8_d = nc.dram_tensor("wy8", [PART, max(wy8_cols_n, 2 * OUT)], F8E4,
                               kind="ExternalInput")
    wx_d = nc.dram_tensor("wx", [PART, max(wx_cols_n, OUT)], BF16,
                          kind="ExternalInput")
    # output: box b = 16*G + g: cols G*1024 + g*64 + hh*32 + j,
    # partition p = c_lh*32 + i, channel c = hh*4 + c_lh
    NG16 = (N_BOXES + 15) // 16
    if odma == "P":
        out_d = nc.dram_tensor("out", [PART, (N_BOXES // 4) * 256], FP32,
                               kind="ExternalOutput")
    else:
        out_d = nc.dram_tensor("out", [PART, NG16 * 1024], BF16,
                               kind="ExternalOutput")

    # per-GROUP chunk column ranges (groups are processing positions)
    def group_ranges(key, width):
        rng = []
        for g0 in range(0, N_BOXES, GROUP):
            los, his = [], []
            for n in perm[g0:g0 + GROUP]:
                p = plans[n]
                if p is None or key not in p:
                    continue
                ntiles = len(p["ts"]) if key == "wyb_off" else (
                    len(p["tp"]) if key == "wy8_off" else len(p["xws"]))
                los.append(p[key])
                his.append(p[key] + ntiles * width)
            rng.append((min(los), max(his)) if los else None)
        return rng

    wyb_rng = group_ranges("wyb_off", OUT)
    wy8_rng = group_ranges("wy8_off", 2 * OUT)
    wx_rng = group_ranges("wx_off", OUT)

    TC = _make_tc_class()
    with TC(nc) as tc, ExitStack() as ctx:
        fpool = ctx.enter_context(tc.tile_pool(name="fmap", bufs=1))
        wpool = ctx.enter_context(tc.tile_pool(name="wts", bufs=2))
        rpool = ctx.enter_context(tc.tile_pool(name="rhs2", bufs=rbufs))
        opool = ctx.enter_context(tc.tile_pool(name="osb", bufs=2))
        p1pool = ctx.enter_context(tc.tile_pool(name="psumT", bufs=p1bufs, space="PSUM"))
        p2pool = ctx.enter_context(tc.tile_pool(name="psum2", bufs=p2bufs, space="PSUM"))

        _eng_map = {"g": nc.gpsimd, "s": nc.scalar, "y": nc.sync}

        def _wy_eng():
            return _eng_map[wdma[0]]

        def _wx_eng():
            return _eng_map[wdma[1]]

        # weight chunk state
        wy_tiles = {}  # group index -> {"b": (tile, base), "8": (tile, base)}
        wx_tiles = {}  # group index -> (tile, base)

        def load_chunks(gi):
            if gi in wy_tiles:
                return
            wy_tiles[gi] = {}
            if wyb_rng[gi] is not None:
                lo, hi = wyb_rng[gi]
                tb = wpool.tile([PART, hi - lo], BF16, name="wyb_sb", tag="wyb")
                _wy_eng().dma_start(out=tb[:], in_=wyb_d[:, lo:hi])
                wy_tiles[gi]["b"] = (tb, lo)
            if wy8_rng[gi] is not None:
                lo, hi = wy8_rng[gi]
                t8 = wpool.tile([PART, hi - lo], F8E4, name="wy8_sb", tag="wy8")
                _wy_eng().dma_start(out=t8[:], in_=wy8_d[:, lo:hi])
                wy_tiles[gi]["8"] = (t8, lo)
            if wx_rng[gi] is not None:
                lo, hi = wx_rng[gi]
                tx = wpool.tile([PART, hi - lo], BF16, name="wx_sb", tag="wx")
                _wx_eng().dma_start(out=tx[:], in_=wx_d[:, lo:hi])
                wx_tiles[gi] = (tx, lo)

        # group-0 weights first so PE can start as soon as the first F
        # tiles land (same FIFO queue as the F DMAs below)
        load_chunks(0)

        f_sb = fpool.tile([PART, C_LOC * NT * W], BF16, name="f_sb")
        f_v = f_sb[:].rearrange("p (c t x) -> p c t x", c=C_LOC, t=NT)
        f_dv = f_d[:].rearrange("p (c t x) -> p c t x", c=C_LOC, t=NT)
        for t in range(NT):
            nc.sync.dma_start(out=f_v[:, :, t, :], in_=f_dv[:, :, t, :])

        # f8 / pooled-map uploads are deferred to just before the first
        # processing group that uses them, so the (FIFO) input queue
        # delivers weights and maps in need order instead of stalling
        # later weight chunks behind maps nobody needs yet.
        map_dmas = {}  # key -> emit thunk
        first_grp = {}  # key -> first processing group using the map

        def _key_of(p):
            if p["xp"] > 1:
                return (p["xp"], p["fp8"])
            return "f8" if p["fp8"] else None

        for pos, n in enumerate(perm):
            p = plans[n]
            if p is None:
                continue
            kkey = _key_of(p)
            if kkey is not None and kkey not in first_grp:
                first_grp[kkey] = pos // GROUP

        if any_fp8:
            f8_sb = fpool.tile([PART, C_LOC * 2 * 2 * W], F8E4, name="f8_sb")
            f8_v = f8_sb[:].rearrange("p (c tp q x) -> p c tp q x",
                                      c=C_LOC, tp=2, q=2)
            f8_dv = f8_d[:].rearrange("p (c tp q x) -> p c tp q x",
                                      c=C_LOC, tp=2, q=2)

            def _emit_f8(f8_v=f8_v, f8_dv=f8_dv):
                for tp in range(2):
                    nc.sync.dma_start(out=f8_v[:, :, tp, :, :],
                                      in_=f8_dv[:, :, tp, :, :])
            map_dmas["f8"] = _emit_f8
        g_vs = {}
        for (xp, isf8), gd in g_ds.items():
            if isf8:
                upw = UPS_PAD[xp]
                gt = fpool.tile([PART, C_LOC * 2 * 2 * upw], F8E4,
                                name=f"g{xp}f8_sb", tag=f"g{xp}f8")
                gv = gt[:].rearrange("p (c tp q x) -> p c tp q x",
                                     c=C_LOC, tp=2, q=2)

                def _emit(gv=gv, gd=gd):
                    nc.sync.dma_start(out=gv, in_=gd[:].rearrange(
                        "p (c tp q x) -> p c tp q x", c=C_LOC, tp=2, q=2))
            else:
                upw = UPS[xp]
                gt = fpool.tile([PART, C_LOC * NT * upw], BF16,
                                name=f"g{xp}_sb", tag=f"g{xp}")
                gv = gt[:].rearrange("p (c t x) -> p c t x", c=C_LOC, t=NT)

                def _emit(gv=gv, gd=gd):
                    nc.sync.dma_start(out=gv, in_=gd[:].rearrange(
                        "p (c t x) -> p c t x", c=C_LOC, t=NT))
            map_dmas[(xp, isf8)] = _emit
            g_vs[(xp, isf8)] = gv

        def emit_maps_due(gi):
            """Emit deferred map DMAs needed by group gi+1 (1-group lead)."""
            for kkey, thunk in list(map_dmas.items()):
                if first_grp.get(kkey, 0) <= gi + 1:
                    thunk()
                    del map_dmas[kkey]
        emit_maps_due(0)

        evac_busy = [0.0, 0.0]  # DVE, ACT modeled busy ns

        def evac(out_ap, in_ap, idx):
            # GPSIMD cannot read PSUM on real HW: DVE / ACT only.
            free = in_ap.free_size()
            costs = (free * 1.0417 + 125.0, free * 0.8333 + 185.0)
            if evac_pat == "auto" or evac_pat is None:
                which = 0 if evac_busy[0] + costs[0] <= evac_busy[1] + costs[1] \
                    else 1
            else:
                which = {"v": 0, "s": 1}[evac_pat[idx % len(evac_pat)]]
            evac_busy[which] += costs[which]
            if which == 0:
                nc.vector.tensor_copy(out=out_ap, in_=in_ap)
            else:
                nc.scalar.copy(out_ap, in_ap)

        evac_ctr = 0
        # (box, window) slot stream state
        cur_p1 = None          # current psum_t tile
        cur_slots = []         # [(box, win_idx)]
        rhs2_of = {}           # (box, win) -> (tile, slot)
        pend_s2 = []           # boxes whose stage-2 is not yet emitted

        o_sb = None
        psum2 = None

        def flush_p1():
            nonlocal cur_p1, cur_slots, evac_ctr
            if cur_p1 is None:
                return
            r_t = rpool.tile([PART, slots * 256], BF16, name="r_t", tag="r")
            evac(r_t[:], cur_p1[:], evac_ctr)
            evac_ctr += 1
            for si, key in enumerate(cur_slots):
                rhs2_of[key] = (r_t, si)
            cur_p1 = None
            cur_slots = []

        def emit_stage2(n):
            """stage 2 + output for box n (requires rhs2 of all windows).
            Output slots are by processing position; host unpermutes."""
            nonlocal psum2, o_sb, evac_ctr
            p = plans[n]
            pos = pos_of[n]
            g16, s16 = pos // 16, pos % 16
            oct_, sq = s16 // 8, s16 % 8
            if s16 == 0:
                o_sb = opool.tile([PART, 1024], BF16, name="o_sb", tag="o")
            if sq == 0:
                psum2 = p2pool.tile([PART, 512], FP32, name="ps2", tag="ps2")
            if p is None:
                # write *something* finite so the tile is defined
                for hh in range(2):
                    nc.tensor.matmul(
                        out=psum2[:, sq * 64 + hh * 32: sq * 64 + hh * 32 + 32],
                        lhsT=f_v[:, 0, 0, 0:PART], rhs=f_v[:, 0, 0, 0:OUT],
                        start=True, stop=True)
            else:
                nw = len(p["xws"])
                for hh in range(2):
                    for k in range(nw):
                        r_t, si = rhs2_of[(n, k)]
                        wx_sl = wx_sb_view(n, k)
                        nc.tensor.matmul(
                            out=psum2[:, sq * 64 + hh * 32: sq * 64 + hh * 32 + 32],
                            lhsT=r_t[:, si * 256 + hh * 128: si * 256 + (hh + 1) * 128],
                            rhs=wx_sl,
                            start=(k == 0), stop=(k == nw - 1))
                for k in range(nw):
                    rhs2_of.pop((n, k), None)
            if sq == 7:
                evac(o_sb[:, oct_ * 512:(oct_ + 1) * 512], psum2[:], evac_ctr)
                evac_ctr += 1
            if s16 == 15:
                _eng_map[odma].dma_start(
                    out=out_d[:, g16 * 1024:(g16 + 1) * 1024], in_=o_sb[:])

        pos_of = {n: i for i, n in enumerate(perm)}

        def wx_sb_view(n, k):
            t, base = wx_tiles[pos_of[n] // GROUP]
            off = plans[n]["wx_off"] + k * OUT - base
            return t[:, off:off + OUT]

        for g0 in range(0, N_BOXES, GROUP):
            gi = g0 // GROUP
            emit_maps_due(gi)
            load_chunks(gi)
            if wyb_rng[gi] is not None:
                wyb_sb, wyb_base = wy_tiles[gi]["b"]
            if wy8_rng[gi] is not None:
                wy8_sb, wy8_base = wy_tiles[gi]["8"]

            for n in perm[g0:g0 + GROUP]:
                p = plans[n]
                if p is not None:
                    for k, x0 in enumerate(p["xws"]):
                        if cur_p1 is None:
                            cur_p1 = p1pool.tile([PART, slots * 256], FP32,
                                                 name="ps1", tag="ps1")
                        si = len(cur_slots)
                        cur_slots.append((n, k))
                        base = si * 256
                        if p["fp8"]:
                            src = g_vs[(p["xp"], True)] if p["xp"] > 1 else f8_v
                            off = p["wy8_off"] - wy8_base
                            for c in range(C_LOC):
                                for j, tp in enumerate(p["tp"]):
                                    nc.tensor.matmul(
                                        out=cur_p1[:, base + c * OUT: base + (c + 1) * OUT],
                                        lhsT=src[:, c, tp, :, x0:x0 + PART],
                                        rhs=wy8_sb[:, off + j * 64: off + (j + 1) * 64]
                                            .rearrange("p (q i) -> p q i", q=2),
                                        start=(j == 0), stop=(j == len(p["tp"]) - 1),
                                        perf_mode=DR)
                        else:
                            src = g_vs[(p["xp"], False)] if p["xp"] > 1 else f_v
                            off = p["wyb_off"] - wyb_base
                            for c in range(C_LOC):
                                for j, t in enumerate(p["ts"]):
                                    nc.tensor.matmul(
                                        out=cur_p1[:, base + c * OUT: base + (c + 1) * OUT],
                                        lhsT=src[:, c, t, x0:x0 + PART],
                                        rhs=wyb_sb[:, off + j * OUT: off + (j + 1) * OUT],
                                        start=(j == 0), stop=(j == len(p["ts"]) - 1))
                        if len(cur_slots) == slots:
                            flush_p1()
                pend_s2.append(n)
                # emit stage 2 for boxes whose windows are all evacuated
                while pend_s2:
                    b = pend_s2[0]
                    pb = plans[b]
                    if pb is not None and any(
                            (b, k) not in rhs2_of for k in range(len(pb["xws"]))):
                        break
                    if pos_of[n] - pos_of[b] < s2_lag:
                        break
                    emit_stage2(b)
                    pend_s2.pop(0)
        flush_p1()
        while pend_s2:
            emit_stage2(pend_s2.pop(0))

    return _patch_serialization(nc)


# ---------------------------------------------------------------------------
# Entry point
# ---------------------------------------------------------------------------

_LAST = {}


def kernel(feature_map, boxes, output_width):
    from concourse.bass_utils import run_bass_kernel_spmd
    import ml_dtypes

    feature_map = np.asarray(feature_map, dtype=np.float32)
    boxes_np = np.asarray(boxes, dtype=np.float32)
    assert int(output_width) == OUT

    wy_all, wx_all = host_geometry(boxes_np)
    plans, perm, wyb_flat, wy8_flat, wx_flat = plan_boxes(
        boxes_np, wy_all, wx_all)
    nc = _build_program(plans, perm, wyb_flat.shape[1], wy8_flat.shape[1],
                        wx_flat.shape[1], **BUILD_KW)

    any_fp8 = any(p is not None and p["fp8"] for p in plans)
    variants = sorted({(p["xp"], p["fp8"]) for p in plans
                       if p is not None and p["xp"] > 1})
    pools_needed = sorted({xp for (xp, _) in variants})
    wyb_u = wyb_flat.astype(ml_dtypes.bfloat16)
    wx_u = wx_flat.astype(ml_dtypes.bfloat16)
    if any_fp8:
        wy8_u = wy8_flat.astype(ml_dtypes.float8_e4m3)
    bases = {xp: xpool_basis(xp) for xp in pools_needed}

    in_maps = []
    for kcore in range(N_CORES):
        # f layout [p, (c, t, x)]: y = t*128 + p
        f_k = feature_map[:, :, kcore * C_LOC:(kcore + 1) * C_LOC]  # [y, x, c]
        f_bf = f_k.astype(ml_dtypes.bfloat16).astype(np.float32)
        f_sb = np.ascontiguousarray(
            f_bf.reshape(NT, PART, W, C_LOC).transpose(1, 3, 0, 2)
        ).reshape(PART, C_LOC * NT * W).astype(ml_dtypes.bfloat16)
        m = {"f": f_sb, "wyb": wyb_u, "wx": wx_u}
        if any_fp8:
            # f8 layout [p, (c, tp, q, x)]: y = tp*256 + q*128 + p
            f8_sb = np.ascontiguousarray(
                f_bf.reshape(2, 2, PART, W, C_LOC).transpose(2, 4, 0, 1, 3)
            ).reshape(PART, C_LOC * 2 * 2 * W).astype(ml_dtypes.float8_e4m3)
            m["f8"] = f8_sb
            m["wy8"] = wy8_u
        g_ks = {xp: np.einsum("xu,yxc->yuc", bases[xp], f_bf, optimize=True)
                for xp in pools_needed}
        for (xp, isf8) in variants:
            g_k = g_ks[xp]
            nm = f"g{xp}{'f8' if isf8 else ''}"
            if isf8:
                upw = UPS_PAD[xp]
                g_pad = np.zeros((H, upw, C_LOC), np.float32)
                g_pad[:, :g_k.shape[1], :] = g_k
                m[nm] = np.ascontiguousarray(
                    g_pad.reshape(2, 2, PART, upw, C_LOC).transpose(2, 4, 0, 1, 3)
                ).reshape(PART, C_LOC * 2 * 2 * upw).astype(ml_dtypes.float8_e4m3)
            else:
                upw = g_k.shape[1]
                m[nm] = np.ascontiguousarray(
                    g_k.reshape(NT, PART, upw, C_LOC).transpose(1, 3, 0, 2)
                ).reshape(PART, C_LOC * NT * upw).astype(ml_dtypes.bfloat16)
        in_maps.append(m)

    _LAST["nc"] = nc
    _LAST["in_maps"] = in_maps
    res = run_bass_kernel_spmd(nc, in_maps, list(range(N_CORES)))

    out = np.zeros((N_BOXES, OUT, OUT, C), np.float32)
    perm_np = np.asarray(perm)
    for kcore in range(N_CORES):
        dev = np.asarray(res.results[kcore]["out"]).astype(np.float32)
        # [p, (G, g, hh, j)] with p = c_lh*32 + i, slot = 16G + g =
        # processing position; unpermute to original box order
        v = dev.reshape(4, OUT, N_BOXES // 16, 16, 2, OUT)  # c_lh, i, G, g, hh, j
        v = v.transpose(2, 3, 1, 5, 4, 0)                   # G, g, i, j, hh, c_lh
        v = v.reshape(N_BOXES, OUT, OUT, C_LOC)
        out[perm_np, :, :, kcore * C_LOC:(kcore + 1) * C_LOC] = v
    for n in range(N_BOXES):
        if plans[n] is None:
            out[n] = 0.0
    return out


def estimate_hw_ns():
    """Cost-model estimate of the per-core kernel duration (ns)."""
    from concourse.timeline_sim import TimelineSim
    nc = _LAST.get("nc")
    if nc is None:
        return -1
    sim = TimelineSim(nc)
    sim.simulate()
    return int(sim.time)


def measure_wall(n=5):
    """Wall-clock of repeated dispatches (includes axon round trips)."""
    import time
    from concourse.bass_utils import run_bass_kernel_spmd
    times = []
    for _ in range(n):
        t0 = time.perf_counter()
        run_bass_kernel_spmd(_LAST["nc"], _LAST["in_maps"], list(range(N_CORES)))
        times.append(time.perf_counter() - t0)
    return times


# revision 61
# speedup vs baseline: 1.1776x; 1.0262x over previous
"""RoIAlign (scale_and_translate, linear, antialias) Trainium2 kernel.

Channel-sharded across 8 NeuronCores: each core keeps a [512, 512, 8]
slice of the feature map resident in SBUF and computes all 512 boxes
for its 8 channels, one box at a time (no box grouping):

  stage 1 (PE):  T^T[x, (c, i)] = sum_y F[y, x, c] * Wy[y, i]
                 one matmul per (c, y-tile, x-window), out free = 32.
                 Wide-kernel boxes (ky*kx >= 12) run fp8e4 DoubleRow
                 (two 128-row y-tiles per matmul, 0.5 cycles/row); the
                 rest run bf16.
  evac:          psum_t [128, 512] (two (box, window) slots) -> SBUF
                 bf16, least-loaded assignment over DVE / ACT (GPSIMD
                 cannot read PSUM).
  stage 2 (PE):  out[(c,i), j] = sum_x T^T[x, ci] * Wx[x, j],
                 operand-swapped (lhsT = T^T chunk, moving = 32 Wx
                 cols) so out free = 32 instead of 256.
  out:           psum2 [128, 512] (8 boxes) -> bf16 staging -> DRAM.

Boxes with wide x-kernels use device-resident 2x/4x x-pooled copies of
the feature map (hat-function pooling; per-box Wx is least-squares
refit onto the hat basis, with delta columns at the image edge). This
shrinks the x-band, cutting stage-1 work and - critically - the
PSUM-evacuation volume, which is the binding engine resource.

Host side computes exact per-box dense resampling weights, extracts
nonzero bands, zero-pads to 128-row tiles/windows, and streams them as
flat [128, cols] arrays in 32-box chunks. fp8 pooled maps are padded
to multiple-of-16 widths (DoubleRow ldweights stride restriction).
"""

import numpy as np

H = 512
W = 512
C = 64
N_BOXES = 512
OUT = 32
N_CORES = 8
C_LOC = C // N_CORES  # 8 channels per core
PART = 128
NT = H // PART        # 4 y partition tiles
GROUP = 32            # boxes per weight-DMA chunk
SLOTS = 4             # (box, window) slots per psum_t tile
FP8_KPROD = 12.0      # use fp8 stage-1 when ky*kx >= this
FP8_KMIN = 2.0        # ... and both ky, kx >= this
XP2_KMIN = 10.0       # use the 2x x-pooled map when kx >= this
XP4_KMIN = 13.0       # use the 4x x-pooled map when kx >= this
WCLIP = 1e-3          # zero fitted pooled weights below this (rel) magnitude
UP2 = W // 2 + 1      # 2x-pooled x size: hats at even x + delta at x=511
UP4 = (W - 1) // 4 + 1 + 3  # 4x pooled: 128 hat nodes + 3 tail deltas = 131
UPS = {2: UP2, 4: UP4}
# fp8 DoubleRow ldweights requires the pair-dim byte stride to be a
# multiple of 16 -> pad the fp8 pooled-map widths up to a multiple of 16
UPS_PAD = {p: (u + 15) // 16 * 16 for p, u in UPS.items()}

# scheduling knobs for _build_program (tuned via TimelineSim)
BUILD_KW = dict(slots=4, p1bufs=3, rbufs=8, p2bufs=2, evac_pat="auto",
                wdma="yy", odma="y", s2_lag=16, wbufs=3, map_lead=1)


# ---------------------------------------------------------------------------
# Host-side weight computation (mirrors jax.image.scale_and_translate with
# method="linear", antialias=True)
# ---------------------------------------------------------------------------

def _compute_weight_mat(in_size, out_size, scale, translation):
    inv_scale = 1.0 / scale
    kernel_scale = max(inv_scale, 1.0)
    sample_f = (np.arange(out_size, dtype=np.float64) + 0.5) * inv_scale \
        - translation * inv_scale - 0.5
    x = np.abs(sample_f[None, :] - np.arange(in_size, dtype=np.float64)[:, None]) \
        / kernel_scale
    weights = np.maximum(0.0, 1.0 - x)
    total = weights.sum(axis=0, keepdims=True)
    weights = np.where(
        np.abs(total) > 1000.0 * float(np.finfo(np.float32).eps),
        weights / np.where(total != 0, total, 1.0),
        0.0,
    )
    valid = (sample_f >= -0.5) & (sample_f <= in_size - 0.5)
    return np.where(valid[None, :], weights, 0.0).astype(np.float32)


def host_geometry(boxes):
    """Exact per-box dense weights wy/wx [N, 512, 32] fp32."""
    boxes = np.asarray(boxes, dtype=np.float64)
    wy_all = np.zeros((N_BOXES, H, OUT), np.float32)
    wx_all = np.zeros((N_BOXES, W, OUT), np.float32)
    for n in range(N_BOXES):
        cx, cy, w, h = boxes[n]
        x0 = cx - w / 2
        y0 = cy - h / 2
        w = max(w, 1e-6)
        h = max(h, 1e-6)
        x_scale = OUT / (w * W)
        y_scale = OUT / (h * H)
        wy_all[n] = _compute_weight_mat(H, OUT, y_scale, -y0 * OUT / h)
        wx_all[n] = _compute_weight_mat(W, OUT, x_scale, -x0 * OUT / w)
    return wy_all, wx_all


def xpool_basis(p):
    """Hat basis at stride-p grid nodes plus delta columns for the tail
    pixels past the last node (which a hat grid cannot represent)."""
    nu = (W - 1) // p + 1
    extra = [x for x in range(W) if x > (nu - 1) * p]
    P = np.zeros((W, nu + len(extra)), np.float32)
    for u in range(nu):
        c = p * u
        for x in range(max(0, c - p + 1), min(W, c + p)):
            P[x, u] = 1.0 - abs(x - c) / p
    for j, x in enumerate(extra):
        P[x, :] = 0.0
        P[x, nu + j] = 1.0
    return P


def xpool_fit_mats():
    """Per pooling factor: (P, PINV) with PINV = (P^T P)^-1 P^T."""
    mats = {}
    for p in (2, 4):
        P = xpool_basis(p)
        PINV = np.linalg.solve((P.T @ P).astype(np.float64),
                               P.T.astype(np.float64))
        mats[p] = (P, PINV)
    return mats


def plan_boxes(boxes, wy_all, wx_all):
    """Per-box banded geometry + flat weight packing.

    Returns plan dicts (None for boxes with empty bands) and the packed
    flat weight arrays wyb [128, :] bf16, wy8 [128, :] fp8, wx [128, :]
    bf16 (as float32 here; cast at upload).
    """
    boxes = np.asarray(boxes, dtype=np.float64)
    ky = np.maximum(16.0 * boxes[:, 3], 1.0)  # y kernel halfwidth (px)
    kx = np.maximum(16.0 * boxes[:, 2], 1.0)

    mats = xpool_fit_mats()

    # processing order: boxes whose source map arrives earliest on the
    # input DMA queue go first (f -> f8 -> pooled bf16 -> pooled fp8), so
    # compute covers the input-upload stream instead of stalling on it.
    # Within the first class, boxes needing only low y-tiles go first so
    # PE can start right after the first F-tile DMA lands.
    def klass(n):
        f8 = (ky[n] * kx[n] >= FP8_KPROD
              and ky[n] >= FP8_KMIN and kx[n] >= FP8_KMIN)
        xp = 4 if kx[n] >= XP4_KMIN else (2 if kx[n] >= XP2_KMIN else 1)
        if xp == 1:
            return 0 if not f8 else 1
        return 2 if not f8 else 3

    def last_tile(n):
        nz = np.nonzero(wy_all[n].any(axis=1))[0]
        return (int(nz[-1]) // PART) if len(nz) else 0

    perm = sorted(range(N_BOXES),
                  key=lambda n: (klass(n),
                                 last_tile(n) if klass(n) == 0 else 0, n))

    plans = [None] * N_BOXES
    wyb_cols, wy8_cols, wx_cols = [], [], []
    for n in perm:
        ynz = np.nonzero(wy_all[n].any(axis=1))[0]
        xnz = np.nonzero(wx_all[n].any(axis=1))[0]
        if len(ynz) == 0 or len(xnz) == 0:
            continue
        r0, r1 = int(ynz[0]), int(ynz[-1]) + 1
        use_fp8 = (ky[n] * kx[n] >= FP8_KPROD
                   and ky[n] >= FP8_KMIN and kx[n] >= FP8_KMIN)
        xp = 4 if kx[n] >= XP4_KMIN else (2 if kx[n] >= XP2_KMIN else 1)

        if xp > 1:
            # least-squares fit of Wx in the pooled hat basis, clipping
            # the tiny ringing tail of the fit to keep the band compact
            P, PINV = mats[xp]
            wx_n = (PINV @ wx_all[n].astype(np.float64)).astype(np.float32)
            wx_n[np.abs(wx_n) < WCLIP * np.abs(wx_n).max()] = 0.0
            xnz = np.nonzero(wx_n.any(axis=1))[0]
            if len(xnz) == 0:
                continue
            WW = UPS[xp]
        else:
            wx_n = wx_all[n]
            WW = W
        c0, c1 = int(xnz[0]), int(xnz[-1]) + 1

        # x windows (arbitrary free-dim offset, clamped; overlap zeroed)
        nxw = (c1 - c0 + PART - 1) // PART
        xws = [min(c0 + k * PART, max(WW - PART, 0)) for k in range(nxw)]

        p = {"fp8": use_fp8, "xp": xp, "xws": xws}
        if use_fp8:
            p_lo, p_hi = r0 // (2 * PART), (r1 - 1) // (2 * PART)
            p["tp"] = list(range(p_lo, p_hi + 1))
            p["wy8_off"] = len(wy8_cols) and sum(c.shape[1] for c in wy8_cols)
            p["wy8_off"] = sum(c.shape[1] for c in wy8_cols)
            for tp in p["tp"]:
                # [128, (q, i)] with y = tp*256 + q*128 + p
                blk = np.zeros((PART, 2, OUT), np.float32)
                for q in range(2):
                    blk[:, q, :] = wy_all[n][tp * 256 + q * 128: tp * 256 + (q + 1) * 128]
                wy8_cols.append(blk.reshape(PART, 2 * OUT))
        else:
            t_lo, t_hi = r0 // PART, (r1 - 1) // PART
            p["ts"] = list(range(t_lo, t_hi + 1))
            p["wyb_off"] = sum(c.shape[1] for c in wyb_cols)
            for t in p["ts"]:
                wyb_cols.append(wy_all[n][t * PART:(t + 1) * PART].copy())

        p["wx_off"] = sum(c.shape[1] for c in wx_cols)
        prev_end = c0
        for x0 in xws:
            wxw = wx_n[x0:x0 + PART].copy()
            lo = max(prev_end - x0, 0)
            wxw[:lo] = 0.0
            prev_end = max(prev_end, x0 + PART)
            wx_cols.append(wxw)
        plans[n] = p

    def cat(cols, width):
        if not cols:
            return np.zeros((PART, width), np.float32)
        return np.concatenate(cols, axis=1)

    wyb_flat = cat(wyb_cols, OUT)
    wy8_flat = cat(wy8_cols, 2 * OUT)
    wx_flat = cat(wx_cols, OUT)
    return plans, perm, wyb_flat, wy8_flat, wx_flat


# ---------------------------------------------------------------------------
# Device program
# ---------------------------------------------------------------------------

def _split_multiwait_bir(raw: bytes) -> bytes:
    """The walrus build here accepts only one sync wait per instruction.
    Hoist extra waits onto single-wait EventSemaphore instructions inserted
    just before, on the same engine (per-engine order is preserved)."""
    import orjson

    d = orjson.loads(raw)
    ctr = 0
    for fn in d.get("functions", []):
        for bb in fn.get("blocks") or []:
            out = []
            for ins in bb["instructions"]:
                si = ins.get("sync_info")
                ws = (si or {}).get("on_wait") or []
                if len(ws) > 1:
                    for w in ws[:-1]:
                        ctr += 1
                        out.append({
                            "debug": ins.get("debug", 0),
                            "engine": ins["engine"],
                            "ins": [],
                            "outs": [],
                            "name": f"{ins['name']}-xw{ctr}",
                            "opcode": "EventSemaphore",
                            "sync_info": {"on_update": [], "on_wait": [w]},
                        })
                    si["on_wait"] = [ws[-1]]
                out.append(ins)
            bb["instructions"] = out
    return orjson.dumps(d)


def _patch_serialization(nc):
    orig = nc.to_json_bytes

    def patched():
        return _split_multiwait_bir(orig())

    nc.to_json_bytes = patched
    return nc


def _make_tc_class():
    import concourse.tile as tile
    from concourse.vector_clock import ScopedClock
    import bass_rust

    class TC(tile.TileContext):
        """TileContext with the tail drain's multi-sem wait split into
        individual single-wait instructions (this walrus rejects >1 wait
        on a CTRL instruction)."""

        def _drain_and_barrier(self, tick_clock, wait_clock):
            nc = self.nc
            probe = nc.sync.drain()
            wait_clock.add_sem_waits(
                probe.ins, ScopedClock({None: tick_clock.global_clock})
            )
            waits = list(probe.ins.sync_info.on_wait)
            probe.ins.sync_info = bass_rust.SyncInfo(on_wait=[], on_update=[])
            by_name = {hh.name: hh for hh in self.sems.allocated().values()}
            for wt in waits:
                nc.sync.wait_ge(by_name[wt.ant_name], wt.wait_value)
            nc.all_engine_barrier()
            popped = nc._tile_sem_poison_stack.pop()
            assert popped is self._sem_poison
            nc.clear_and_free_semaphores(list(self.sems.allocated().values()))
            nc.all_engine_barrier()

    return TC


def _build_program(plans, perm, wyb_cols_n, wy8_cols_n, wx_cols_n,
                   slots=SLOTS, p1bufs=2, rbufs=6, p2bufs=2, evac_mod=3,
                   evac_pat=None, wdma="gs", odma="y", s2_lag=0,
                   wbufs=2, map_lead=1):
    import concourse.bass as bass
    import concourse.mybir as mybir
    from contextlib import ExitStack

    FP32 = mybir.dt.float32
    BF16 = mybir.dt.bfloat16
    F8E4 = mybir.dt.float8e4
    DR = mybir.MatmulPerfMode.DoubleRow

    any_fp8 = any(p is not None and p["fp8"] for p in plans)
    # pooled-map variants needed: (pool factor, fp8?)
    variants = sorted({(p["xp"], p["fp8"]) for p in plans
                       if p is not None and p["xp"] > 1})

    nc = bass.Bass()
    # feature map, already in SBUF layout [128, (c, t, x)] / [128, (c, tp, q, x)]
    f_d = nc.dram_tensor("f", [PART, C_LOC * NT * W], BF16, kind="ExternalInput")
    if any_fp8:
        f8_d = nc.dram_tensor("f8", [PART, C_LOC * 2 * 2 * W], F8E4,
                              kind="ExternalInput")
    g_ds = {}
    for (xp, isf8) in variants:
        nm = f"g{xp}{'f8' if isf8 else ''}"
        if isf8:
            g_ds[(xp, True)] = nc.dram_tensor(
                nm, [PART, C_LOC * 2 * 2 * UPS_PAD[xp]], F8E4,
                kind="ExternalInput")
        else:
            g_ds[(xp, False)] = nc.dram_tensor(
                nm, [PART, C_LOC * NT * UPS[xp]], BF16, kind="ExternalInput")
    wyb_d = nc.dram_tensor("wyb", [PART, max(wyb_cols_n, OUT)], BF16,
                           kind="ExternalInput")
    if any_fp8:
        wy8_d = nc.dram_tensor("wy8", [PART, max(wy8_cols_n, 2 * OUT)], F8E4,
                               kind="ExternalInput")
    wx_d = nc.dram_tensor("wx", [PART, max(wx_cols_n, OUT)], BF16,
                          kind="ExternalInput")
    # output: box b = 16*G + g: cols G*1024 + g*64 + hh*32 + j,
    # partition p = c_lh*32 + i, channel c = hh*4 + c_lh
    NG16 = (N_BOXES + 15) // 16
    if odma == "P":
        out_d = nc.dram_tensor("out", [PART, (N_BOXES // 4) * 256], FP32,
                               kind="ExternalOutput")
    else:
        out_d = nc.dram_tensor("out", [PART, NG16 * 1024], BF16,
                               kind="ExternalOutput")

    # per-GROUP chunk column ranges (groups are processing positions)
    def group_ranges(key, width):
        rng = []
        for g0 in range(0, N_BOXES, GROUP):
            los, his = [], []
            for n in perm[g0:g0 + GROUP]:
                p = plans[n]
                if p is None or key not in p:
                    continue
                ntiles = len(p["ts"]) if key == "wyb_off" else (
                    len(p["tp"]) if key == "wy8_off" else len(p["xws"]))
                los.append(p[key])
                his.append(p[key] + ntiles * width)
            rng.append((min(los), max(his)) if los else None)
        return rng

    wyb_rng = group_ranges("wyb_off", OUT)
    wy8_rng = group_ranges("wy8_off", 2 * OUT)
    wx_rng = group_ranges("wx_off", OUT)

    TC = _make_tc_class()
    with TC(nc) as tc, ExitStack() as ctx:
        fpool = ctx.enter_context(tc.tile_pool(name="fmap", bufs=1))
        wpool = ctx.enter_context(tc.tile_pool(name="wts", bufs=wbufs))
        rpool = ctx.enter_context(tc.tile_pool(name="rhs2", bufs=rbufs))
        opool = ctx.enter_context(tc.tile_pool(name="osb", bufs=2))
        p1pool = ctx.enter_context(tc.tile_pool(name="psumT", bufs=p1bufs, space="PSUM"))
        p2pool = ctx.enter_context(tc.tile_pool(name="psum2", bufs=p2bufs, space="PSUM"))

        _eng_map = {"g": nc.gpsimd, "s": nc.scalar, "y": nc.sync}

        def _wy_eng():
            return _eng_map[wdma[0]]

        def _wx_eng():
            return _eng_map[wdma[1]]

        # weight chunk state
        wy_tiles = {}  # group index -> {"b": (tile, base), "8": (tile, base)}
        wx_tiles = {}  # group index -> (tile, base)

        def load_chunks(gi):
            if gi in wy_tiles:
                return
            wy_tiles[gi] = {}
            if wyb_rng[gi] is not None:
                lo, hi = wyb_rng[gi]
                tb = wpool.tile([PART, hi - lo], BF16, name="wyb_sb", tag="wyb")
                _wy_eng().dma_start(out=tb[:], in_=wyb_d[:, lo:hi])
                wy_tiles[gi]["b"] = (tb, lo)
            if wy8_rng[gi] is not None:
                lo, hi = wy8_rng[gi]
                t8 = wpool.tile([PART, hi - lo], F8E4, name="wy8_sb", tag="wy8")
                _wy_eng().dma_start(out=t8[:], in_=wy8_d[:, lo:hi])
                wy_tiles[gi]["8"] = (t8, lo)
            if wx_rng[gi] is not None:
                lo, hi = wx_rng[gi]
                tx = wpool.tile([PART, hi - lo], BF16, name="wx_sb", tag="wx")
                _wx_eng().dma_start(out=tx[:], in_=wx_d[:, lo:hi])
                wx_tiles[gi] = (tx, lo)

        # group-0 weights first so PE can start as soon as the first F
        # tiles land (same FIFO queue as the F DMAs below)
        load_chunks(0)

        f_sb = fpool.tile([PART, C_LOC * NT * W], BF16, name="f_sb")
        f_v = f_sb[:].rearrange("p (c t x) -> p c t x", c=C_LOC, t=NT)
        f_dv = f_d[:].rearrange("p (c t x) -> p c t x", c=C_LOC, t=NT)
        for t in range(NT):
            nc.sync.dma_start(out=f_v[:, :, t, :], in_=f_dv[:, :, t, :])

        # f8 / pooled-map uploads are deferred to just before the first
        # processing group that uses them, so the (FIFO) input queue
        # delivers weights and maps in need order instead of stalling
        # later weight chunks behind maps nobody needs yet.
        map_dmas = {}  # key -> emit thunk
        first_grp = {}  # key -> first processing group using the map

        def _key_of(p):
            if p["xp"] > 1:
                return (p["xp"], p["fp8"])
            return "f8" if p["fp8"] else None

        for pos, n in enumerate(perm):
            p = plans[n]
            if p is None:
                continue
            kkey = _key_of(p)
            if kkey is not None and kkey not in first_grp:
                first_grp[kkey] = pos // GROUP

        if any_fp8:
            f8_sb = fpool.tile([PART, C_LOC * 2 * 2 * W], F8E4, name="f8_sb")
            f8_v = f8_sb[:].rearrange("p (c tp q x) -> p c tp q x",
                                      c=C_LOC, tp=2, q=2)
            f8_dv = f8_d[:].rearrange("p (c tp q x) -> p c tp q x",
                                      c=C_LOC, tp=2, q=2)

            def _emit_f8(f8_v=f8_v, f8_dv=f8_dv):
                for tp in range(2):
                    nc.sync.dma_start(out=f8_v[:, :, tp, :, :],
                                      in_=f8_dv[:, :, tp, :, :])
            map_dmas["f8"] = _emit_f8
        g_vs = {}
        for (xp, isf8), gd in g_ds.items():
            if isf8:
                upw = UPS_PAD[xp]
                gt = fpool.tile([PART, C_LOC * 2 * 2 * upw], F8E4,
                                name=f"g{xp}f8_sb", tag=f"g{xp}f8")
                gv = gt[:].rearrange("p (c tp q x) -> p c tp q x",
                                     c=C_LOC, tp=2, q=2)

                def _emit(gv=gv, gd=gd):
                    nc.sync.dma_start(out=gv, in_=gd[:].rearrange(
                        "p (c tp q x) -> p c tp q x", c=C_LOC, tp=2, q=2))
            else:
                upw = UPS[xp]
                gt = fpool.tile([PART, C_LOC * NT * upw], BF16,
                                name=f"g{xp}_sb", tag=f"g{xp}")
                gv = gt[:].rearrange("p (c t x) -> p c t x", c=C_LOC, t=NT)

                def _emit(gv=gv, gd=gd):
                    nc.sync.dma_start(out=gv, in_=gd[:].rearrange(
                        "p (c t x) -> p c t x", c=C_LOC, t=NT))
            map_dmas[(xp, isf8)] = _emit
            g_vs[(xp, isf8)] = gv

        def emit_maps_due(gi):
            """Emit deferred map DMAs with map_lead groups of lead time."""
            for kkey, thunk in list(map_dmas.items()):
                if first_grp.get(kkey, 0) <= gi + map_lead:
                    thunk()
                    del map_dmas[kkey]
        emit_maps_due(0)

        evac_busy = [0.0, 0.0]  # DVE, ACT modeled busy ns

        def evac(out_ap, in_ap, idx):
            # GPSIMD cannot read PSUM on real HW: DVE / ACT only.
            free = in_ap.free_size()
            costs = (free * 1.0417 + 125.0, free * 0.8333 + 185.0)
            if evac_pat == "auto" or evac_pat is None:
                which = 0 if evac_busy[0] + costs[0] <= evac_busy[1] + costs[1] \
                    else 1
            else:
                which = {"v": 0, "s": 1}[evac_pat[idx % len(evac_pat)]]
            evac_busy[which] += costs[which]
            if which == 0:
                nc.vector.tensor_copy(out=out_ap, in_=in_ap)
            else:
                nc.scalar.copy(out_ap, in_ap)

        evac_ctr = 0
        # (box, window) slot stream state
        cur_p1 = None          # current psum_t tile
        cur_slots = []         # [(box, win_idx)]
        rhs2_of = {}           # (box, win) -> (tile, slot)
        pend_s2 = []           # boxes whose stage-2 is not yet emitted

        o_sb = None
        psum2 = None

        def flush_p1():
            nonlocal cur_p1, cur_slots, evac_ctr
            if cur_p1 is None:
                return
            r_t = rpool.tile([PART, slots * 256], BF16, name="r_t", tag="r")
            evac(r_t[:], cur_p1[:], evac_ctr)
            evac_ctr += 1
            for si, key in enumerate(cur_slots):
                rhs2_of[key] = (r_t, si)
            cur_p1 = None
            cur_slots = []

        def emit_stage2(n):
            """stage 2 + output for box n (requires rhs2 of all windows).
            Output slots are by processing position; host unpermutes."""
            nonlocal psum2, o_sb, evac_ctr
            p = plans[n]
            pos = pos_of[n]
            g16, s16 = pos // 16, pos % 16
            oct_, sq = s16 // 8, s16 % 8
            if s16 == 0:
                o_sb = opool.tile([PART, 1024], BF16, name="o_sb", tag="o")
            if sq == 0:
                psum2 = p2pool.tile([PART, 512], FP32, name="ps2", tag="ps2")
            if p is None:
                # write *something* finite so the tile is defined
                for hh in range(2):
                    nc.tensor.matmul(
                        out=psum2[:, sq * 64 + hh * 32: sq * 64 + hh * 32 + 32],
                        lhsT=f_v[:, 0, 0, 0:PART], rhs=f_v[:, 0, 0, 0:OUT],
                        start=True, stop=True)
            else:
                nw = len(p["xws"])
                for hh in range(2):
                    for k in range(nw):
                        r_t, si = rhs2_of[(n, k)]
                        wx_sl = wx_sb_view(n, k)
                        nc.tensor.matmul(
                            out=psum2[:, sq * 64 + hh * 32: sq * 64 + hh * 32 + 32],
                            lhsT=r_t[:, si * 256 + hh * 128: si * 256 + (hh + 1) * 128],
                            rhs=wx_sl,
                            start=(k == 0), stop=(k == nw - 1))
                for k in range(nw):
                    rhs2_of.pop((n, k), None)
            if sq == 7:
                evac(o_sb[:, oct_ * 512:(oct_ + 1) * 512], psum2[:], evac_ctr)
                evac_ctr += 1
            if s16 == 15:
                _eng_map[odma].dma_start(
                    out=out_d[:, g16 * 1024:(g16 + 1) * 1024], in_=o_sb[:])

        pos_of = {n: i for i, n in enumerate(perm)}

        def wx_sb_view(n, k):
            t, base = wx_tiles[pos_of[n] // GROUP]
            off = plans[n]["wx_off"] + k * OUT - base
            return t[:, off:off + OUT]

        NGRP = (N_BOXES + GROUP - 1) // GROUP
        for g0 in range(0, N_BOXES, GROUP):
            gi = g0 // GROUP
            emit_maps_due(gi)
            for la in range(wbufs - 1):
                if gi + la < NGRP:
                    load_chunks(gi + la)
            if wyb_rng[gi] is not None:
                wyb_sb, wyb_base = wy_tiles[gi]["b"]
            if wy8_rng[gi] is not None:
                wy8_sb, wy8_base = wy_tiles[gi]["8"]

            for n in perm[g0:g0 + GROUP]:
                p = plans[n]
                if p is not None:
                    for k, x0 in enumerate(p["xws"]):
                        if cur_p1 is None:
                            cur_p1 = p1pool.tile([PART, slots * 256], FP32,
                                                 name="ps1", tag="ps1")
                        si = len(cur_slots)
                        cur_slots.append((n, k))
                        base = si * 256
                        if p["fp8"]:
                            src = g_vs[(p["xp"], True)] if p["xp"] > 1 else f8_v
                            off = p["wy8_off"] - wy8_base
                            for c in range(C_LOC):
                                for j, tp in enumerate(p["tp"]):
                                    nc.tensor.matmul(
                                        out=cur_p1[:, base + c * OUT: base + (c + 1) * OUT],
                                        lhsT=src[:, c, tp, :, x0:x0 + PART],
                                        rhs=wy8_sb[:, off + j * 64: off + (j + 1) * 64]
                                            .rearrange("p (q i) -> p q i", q=2),
                                        start=(j == 0), stop=(j == len(p["tp"]) - 1),
                                        perf_mode=DR)
                        else:
                            src = g_vs[(p["xp"], False)] if p["xp"] > 1 else f_v
                            off = p["wyb_off"] - wyb_base
                            for c in range(C_LOC):
                                for j, t in enumerate(p["ts"]):
                                    nc.tensor.matmul(
                                        out=cur_p1[:, base + c * OUT: base + (c + 1) * OUT],
                                        lhsT=src[:, c, t, x0:x0 + PART],
                                        rhs=wyb_sb[:, off + j * OUT: off + (j + 1) * OUT],
                                        start=(j == 0), stop=(j == len(p["ts"]) - 1))
                        if len(cur_slots) == slots:
                            flush_p1()
                pend_s2.append(n)
                # emit stage 2 for boxes whose windows are all evacuated
                while pend_s2:
                    b = pend_s2[0]
                    pb = plans[b]
                    if pb is not None and any(
                            (b, k) not in rhs2_of for k in range(len(pb["xws"]))):
                        break
                    if pos_of[n] - pos_of[b] < s2_lag:
                        break
                    emit_stage2(b)
                    pend_s2.pop(0)
        flush_p1()
        while pend_s2:
            emit_stage2(pend_s2.pop(0))

    return _patch_serialization(nc)


# ---------------------------------------------------------------------------
# Entry point
# ---------------------------------------------------------------------------

_LAST = {}


def kernel(feature_map, boxes, output_width):
    from concourse.bass_utils import run_bass_kernel_spmd
    import ml_dtypes

    feature_map = np.asarray(feature_map, dtype=np.float32)
    boxes_np = np.asarray(boxes, dtype=np.float32)
    assert int(output_width) == OUT

    wy_all, wx_all = host_geometry(boxes_np)
    plans, perm, wyb_flat, wy8_flat, wx_flat = plan_boxes(
        boxes_np, wy_all, wx_all)
    nc = _build_program(plans, perm, wyb_flat.shape[1], wy8_flat.shape[1],
                        wx_flat.shape[1], **BUILD_KW)

    any_fp8 = any(p is not None and p["fp8"] for p in plans)
    variants = sorted({(p["xp"], p["fp8"]) for p in plans
                       if p is not None and p["xp"] > 1})
    pools_needed = sorted({xp for (xp, _) in variants})
    wyb_u = wyb_flat.astype(ml_dtypes.bfloat16)
    wx_u = wx_flat.astype(ml_dtypes.bfloat16)
    if any_fp8:
        wy8_u = wy8_flat.astype(ml_dtypes.float8_e4m3)
    bases = {xp: xpool_basis(xp) for xp in pools_needed}

    in_maps = []
    for kcore in range(N_CORES):
        # f layout [p, (c, t, x)]: y = t*128 + p
        f_k = feature_map[:, :, kcore * C_LOC:(kcore + 1) * C_LOC]  # [y, x, c]
        f_bf = f_k.astype(ml_dtypes.bfloat16).astype(np.float32)
        f_sb = np.ascontiguousarray(
            f_bf.reshape(NT, PART, W, C_LOC).transpose(1, 3, 0, 2)
        ).reshape(PART, C_LOC * NT * W).astype(ml_dtypes.bfloat16)
        m = {"f": f_sb, "wyb": wyb_u, "wx": wx_u}
        if any_fp8:
            # f8 layout [p, (c, tp, q, x)]: y = tp*256 + q*128 + p
            f8_sb = np.ascontiguousarray(
                f_bf.reshape(2, 2, PART, W, C_LOC).transpose(2, 4, 0, 1, 3)
            ).reshape(PART, C_LOC * 2 * 2 * W).astype(ml_dtypes.float8_e4m3)
            m["f8"] = f8_sb
            m["wy8"] = wy8_u
        g_ks = {xp: np.einsum("xu,yxc->yuc", bases[xp], f_bf, optimize=True)
                for xp in pools_needed}
        for (xp, isf8) in variants:
            g_k = g_ks[xp]
            nm = f"g{xp}{'f8' if isf8 else ''}"
            if isf8:
                upw = UPS_PAD[xp]
                g_pad = np.zeros((H, upw, C_LOC), np.float32)
                g_pad[:, :g_k.shape[1], :] = g_k
                m[nm] = np.ascontiguousarray(
                    g_pad.reshape(2, 2, PART, upw, C_LOC).transpose(2, 4, 0, 1, 3)
                ).reshape(PART, C_LOC * 2 * 2 * upw).astype(ml_dtypes.float8_e4m3)
            else:
                upw = g_k.shape[1]
                m[nm] = np.ascontiguousarray(
                    g_k.reshape(NT, PART, upw, C_LOC).transpose(1, 3, 0, 2)
                ).reshape(PART, C_LOC * NT * upw).astype(ml_dtypes.bfloat16)
        in_maps.append(m)

    _LAST["nc"] = nc
    _LAST["in_maps"] = in_maps
    res = run_bass_kernel_spmd(nc, in_maps, list(range(N_CORES)))

    out = np.zeros((N_BOXES, OUT, OUT, C), np.float32)
    perm_np = np.asarray(perm)
    for kcore in range(N_CORES):
        dev = np.asarray(res.results[kcore]["out"]).astype(np.float32)
        # [p, (G, g, hh, j)] with p = c_lh*32 + i, slot = 16G + g =
        # processing position; unpermute to original box order
        v = dev.reshape(4, OUT, N_BOXES // 16, 16, 2, OUT)  # c_lh, i, G, g, hh, j
        v = v.transpose(2, 3, 1, 5, 4, 0)                   # G, g, i, j, hh, c_lh
        v = v.reshape(N_BOXES, OUT, OUT, C_LOC)
        out[perm_np, :, :, kcore * C_LOC:(kcore + 1) * C_LOC] = v
    for n in range(N_BOXES):
        if plans[n] is None:
            out[n] = 0.0
    return out


def estimate_hw_ns():
    """Cost-model estimate of the per-core kernel duration (ns)."""
    from concourse.timeline_sim import TimelineSim
    nc = _LAST.get("nc")
    if nc is None:
        return -1
    sim = TimelineSim(nc)
    sim.simulate()
    return int(sim.time)


def measure_wall(n=5):
    """Wall-clock of repeated dispatches (includes axon round trips)."""
    import time
    from concourse.bass_utils import run_bass_kernel_spmd
    times = []
    for _ in range(n):
        t0 = time.perf_counter()
        run_bass_kernel_spmd(_LAST["nc"], _LAST["in_maps"], list(range(N_CORES)))
        times.append(time.perf_counter() - t0)
    return times


# revision 64
# speedup vs baseline: 1.1889x; 1.0096x over previous
"""RoIAlign (scale_and_translate, linear, antialias) Trainium2 kernel.

Channel-sharded across 8 NeuronCores: each core keeps a [512, 512, 8]
slice of the feature map resident in SBUF and computes all 512 boxes
for its 8 channels, one box at a time (no box grouping):

  stage 1 (PE):  T^T[x, (c, i)] = sum_y F[y, x, c] * Wy[y, i]
                 one matmul per (c, y-tile, x-window), out free = 32.
                 Wide-kernel boxes (ky*kx >= 12) run fp8e4 DoubleRow
                 (two 128-row y-tiles per matmul, 0.5 cycles/row); the
                 rest run bf16.
  evac:          psum_t [128, 512] (two (box, window) slots) -> SBUF
                 bf16, least-loaded assignment over DVE / ACT (GPSIMD
                 cannot read PSUM).
  stage 2 (PE):  out[(c,i), j] = sum_x T^T[x, ci] * Wx[x, j],
                 operand-swapped (lhsT = T^T chunk, moving = 32 Wx
                 cols) so out free = 32 instead of 256.
  out:           psum2 [128, 512] (8 boxes) -> bf16 staging -> DRAM.

Boxes with wide x-kernels use device-resident 2x/4x x-pooled copies of
the feature map (hat-function pooling; per-box Wx is least-squares
refit onto the hat basis, with delta columns at the image edge). This
shrinks the x-band, cutting stage-1 work and - critically - the
PSUM-evacuation volume, which is the binding engine resource.

Host side computes exact per-box dense resampling weights, extracts
nonzero bands, zero-pads to 128-row tiles/windows, and streams them as
flat [128, cols] arrays in 32-box chunks. fp8 pooled maps are padded
to multiple-of-16 widths (DoubleRow ldweights stride restriction).
"""

import numpy as np

H = 512
W = 512
C = 64
N_BOXES = 512
OUT = 32
N_CORES = 8
C_LOC = C // N_CORES  # 8 channels per core
PART = 128
NT = H // PART        # 4 y partition tiles
GROUP = 32            # boxes per weight-DMA chunk
SLOTS = 4             # (box, window) slots per psum_t tile
FP8_KPROD = 12.0      # use fp8 stage-1 when ky*kx >= this
FP8_KMIN = 2.0        # ... and both ky, kx >= this
XP2_KMIN = 10.0       # use the 2x x-pooled map when kx >= this
XP4_KMIN = 13.0       # use the 4x x-pooled map when kx >= this
WCLIP = 1e-3          # zero fitted pooled weights below this (rel) magnitude
UP2 = W // 2 + 1      # 2x-pooled x size: hats at even x + delta at x=511
UP4 = (W - 1) // 4 + 1 + 3  # 4x pooled: 128 hat nodes + 3 tail deltas = 131
UPS = {2: UP2, 4: UP4}
# fp8 DoubleRow ldweights requires the pair-dim byte stride to be a
# multiple of 16 -> pad the fp8 pooled-map widths up to a multiple of 16
UPS_PAD = {p: (u + 15) // 16 * 16 for p, u in UPS.items()}
INTERLEAVE_HEAD = 101  # pure-bf16 boxes before class interleaving starts

# scheduling knobs for _build_program (tuned via TimelineSim)
BUILD_KW = dict(slots=4, p1bufs=3, rbufs=8, p2bufs=2, evac_pat="auto",
                wdma="yy", odma="y", s2_lag=16, wbufs=3, map_lead=1)


# ---------------------------------------------------------------------------
# Host-side weight computation (mirrors jax.image.scale_and_translate with
# method="linear", antialias=True)
# ---------------------------------------------------------------------------

def _compute_weight_mat(in_size, out_size, scale, translation):
    inv_scale = 1.0 / scale
    kernel_scale = max(inv_scale, 1.0)
    sample_f = (np.arange(out_size, dtype=np.float64) + 0.5) * inv_scale \
        - translation * inv_scale - 0.5
    x = np.abs(sample_f[None, :] - np.arange(in_size, dtype=np.float64)[:, None]) \
        / kernel_scale
    weights = np.maximum(0.0, 1.0 - x)
    total = weights.sum(axis=0, keepdims=True)
    weights = np.where(
        np.abs(total) > 1000.0 * float(np.finfo(np.float32).eps),
        weights / np.where(total != 0, total, 1.0),
        0.0,
    )
    valid = (sample_f >= -0.5) & (sample_f <= in_size - 0.5)
    return np.where(valid[None, :], weights, 0.0).astype(np.float32)


def host_geometry(boxes):
    """Exact per-box dense weights wy/wx [N, 512, 32] fp32."""
    boxes = np.asarray(boxes, dtype=np.float64)
    wy_all = np.zeros((N_BOXES, H, OUT), np.float32)
    wx_all = np.zeros((N_BOXES, W, OUT), np.float32)
    for n in range(N_BOXES):
        cx, cy, w, h = boxes[n]
        x0 = cx - w / 2
        y0 = cy - h / 2
        w = max(w, 1e-6)
        h = max(h, 1e-6)
        x_scale = OUT / (w * W)
        y_scale = OUT / (h * H)
        wy_all[n] = _compute_weight_mat(H, OUT, y_scale, -y0 * OUT / h)
        wx_all[n] = _compute_weight_mat(W, OUT, x_scale, -x0 * OUT / w)
    return wy_all, wx_all


def xpool_basis(p):
    """Hat basis at stride-p grid nodes plus delta columns for the tail
    pixels past the last node (which a hat grid cannot represent)."""
    nu = (W - 1) // p + 1
    extra = [x for x in range(W) if x > (nu - 1) * p]
    P = np.zeros((W, nu + len(extra)), np.float32)
    for u in range(nu):
        c = p * u
        for x in range(max(0, c - p + 1), min(W, c + p)):
            P[x, u] = 1.0 - abs(x - c) / p
    for j, x in enumerate(extra):
        P[x, :] = 0.0
        P[x, nu + j] = 1.0
    return P


def xpool_fit_mats():
    """Per pooling factor: (P, PINV) with PINV = (P^T P)^-1 P^T."""
    mats = {}
    for p in (2, 4):
        P = xpool_basis(p)
        PINV = np.linalg.solve((P.T @ P).astype(np.float64),
                               P.T.astype(np.float64))
        mats[p] = (P, PINV)
    return mats


def plan_boxes(boxes, wy_all, wx_all):
    """Per-box banded geometry + flat weight packing.

    Returns plan dicts (None for boxes with empty bands) and the packed
    flat weight arrays wyb [128, :] bf16, wy8 [128, :] fp8, wx [128, :]
    bf16 (as float32 here; cast at upload).
    """
    boxes = np.asarray(boxes, dtype=np.float64)
    ky = np.maximum(16.0 * boxes[:, 3], 1.0)  # y kernel halfwidth (px)
    kx = np.maximum(16.0 * boxes[:, 2], 1.0)

    mats = xpool_fit_mats()

    # processing order: boxes whose source map arrives earliest on the
    # input DMA queue go first (f -> f8 -> pooled bf16 -> pooled fp8), so
    # compute covers the input-upload stream instead of stalling on it.
    # Within the first class, boxes needing only low y-tiles go first so
    # PE can start right after the first F-tile DMA lands.
    def klass(n):
        f8 = (ky[n] * kx[n] >= FP8_KPROD
              and ky[n] >= FP8_KMIN and kx[n] >= FP8_KMIN)
        xp = 4 if kx[n] >= XP4_KMIN else (2 if kx[n] >= XP2_KMIN else 1)
        if xp == 1:
            return 0 if not f8 else 1
        return 2 if not f8 else 3

    def last_tile(n):
        nz = np.nonzero(wy_all[n].any(axis=1))[0]
        return (int(nz[-1]) // PART) if len(nz) else 0

    by_class = {0: [], 1: [], 2: [], 3: []}
    for n in range(N_BOXES):
        by_class[klass(n)].append(n)
    by_class[0].sort(key=lambda n: (last_tile(n), n))
    # head: pure bf16 boxes to cover the f/f8 upload; then interleave the
    # PE-heavy (bf16) remainder evenly among the evac-heavy fp8 boxes so
    # neither PE nor the evac engines starves in class-segregated bursts
    NA = min(INTERLEAVE_HEAD, len(by_class[0]))
    head = by_class[0][:NA]
    pe_list = by_class[0][NA:] + by_class[2]   # bf16: PE-heavy
    ev_list = by_class[1] + by_class[3]        # fp8: evac-heavy
    total = len(pe_list) + len(ev_list)
    merged = []
    pi = ei = 0
    acc = 0.0
    r = len(pe_list) / max(total, 1)
    for _ in range(total):
        acc += r
        if acc >= 1.0 and pi < len(pe_list):
            merged.append(pe_list[pi]); pi += 1; acc -= 1.0
        elif ei < len(ev_list):
            merged.append(ev_list[ei]); ei += 1
        else:
            merged.append(pe_list[pi]); pi += 1
    perm = head + merged

    plans = [None] * N_BOXES
    wyb_cols, wy8_cols, wx_cols = [], [], []
    for n in perm:
        ynz = np.nonzero(wy_all[n].any(axis=1))[0]
        xnz = np.nonzero(wx_all[n].any(axis=1))[0]
        if len(ynz) == 0 or len(xnz) == 0:
            continue
        r0, r1 = int(ynz[0]), int(ynz[-1]) + 1
        use_fp8 = (ky[n] * kx[n] >= FP8_KPROD
                   and ky[n] >= FP8_KMIN and kx[n] >= FP8_KMIN)
        xp = 4 if kx[n] >= XP4_KMIN else (2 if kx[n] >= XP2_KMIN else 1)

        if xp > 1:
            # least-squares fit of Wx in the pooled hat basis, clipping
            # the tiny ringing tail of the fit to keep the band compact
            P, PINV = mats[xp]
            wx_n = (PINV @ wx_all[n].astype(np.float64)).astype(np.float32)
            wx_n[np.abs(wx_n) < WCLIP * np.abs(wx_n).max()] = 0.0
            xnz = np.nonzero(wx_n.any(axis=1))[0]
            if len(xnz) == 0:
                continue
            WW = UPS[xp]
        else:
            wx_n = wx_all[n]
            WW = W
        c0, c1 = int(xnz[0]), int(xnz[-1]) + 1

        # x windows (arbitrary free-dim offset, clamped; overlap zeroed)
        nxw = (c1 - c0 + PART - 1) // PART
        xws = [min(c0 + k * PART, max(WW - PART, 0)) for k in range(nxw)]

        p = {"fp8": use_fp8, "xp": xp, "xws": xws}
        if use_fp8:
            p_lo, p_hi = r0 // (2 * PART), (r1 - 1) // (2 * PART)
            p["tp"] = list(range(p_lo, p_hi + 1))
            p["wy8_off"] = len(wy8_cols) and sum(c.shape[1] for c in wy8_cols)
            p["wy8_off"] = sum(c.shape[1] for c in wy8_cols)
            for tp in p["tp"]:
                # [128, (q, i)] with y = tp*256 + q*128 + p
                blk = np.zeros((PART, 2, OUT), np.float32)
                for q in range(2):
                    blk[:, q, :] = wy_all[n][tp * 256 + q * 128: tp * 256 + (q + 1) * 128]
                wy8_cols.append(blk.reshape(PART, 2 * OUT))
        else:
            t_lo, t_hi = r0 // PART, (r1 - 1) // PART
            p["ts"] = list(range(t_lo, t_hi + 1))
            p["wyb_off"] = sum(c.shape[1] for c in wyb_cols)
            for t in p["ts"]:
                wyb_cols.append(wy_all[n][t * PART:(t + 1) * PART].copy())

        p["wx_off"] = sum(c.shape[1] for c in wx_cols)
        prev_end = c0
        for x0 in xws:
            wxw = wx_n[x0:x0 + PART].copy()
            lo = max(prev_end - x0, 0)
            wxw[:lo] = 0.0
            prev_end = max(prev_end, x0 + PART)
            wx_cols.append(wxw)
        plans[n] = p

    def cat(cols, width):
        if not cols:
            return np.zeros((PART, width), np.float32)
        return np.concatenate(cols, axis=1)

    wyb_flat = cat(wyb_cols, OUT)
    wy8_flat = cat(wy8_cols, 2 * OUT)
    wx_flat = cat(wx_cols, OUT)
    return plans, perm, wyb_flat, wy8_flat, wx_flat


# ---------------------------------------------------------------------------
# Device program
# ---------------------------------------------------------------------------

def _split_multiwait_bir(raw: bytes) -> bytes:
    """The walrus build here accepts only one sync wait per instruction.
    Hoist extra waits onto single-wait EventSemaphore instructions inserted
    just before, on the same engine (per-engine order is preserved)."""
    import orjson

    d = orjson.loads(raw)
    ctr = 0
    for fn in d.get("functions", []):
        for bb in fn.get("blocks") or []:
            out = []
            for ins in bb["instructions"]:
                si = ins.get("sync_info")
                ws = (si or {}).get("on_wait") or []
                if len(ws) > 1:
                    for w in ws[:-1]:
                        ctr += 1
                        out.append({
                            "debug": ins.get("debug", 0),
                            "engine": ins["engine"],
                            "ins": [],
                            "outs": [],
                            "name": f"{ins['name']}-xw{ctr}",
                            "opcode": "EventSemaphore",
                            "sync_info": {"on_update": [], "on_wait": [w]},
                        })
                    si["on_wait"] = [ws[-1]]
                out.append(ins)
            bb["instructions"] = out
    return orjson.dumps(d)


def _patch_serialization(nc):
    orig = nc.to_json_bytes

    def patched():
        return _split_multiwait_bir(orig())

    nc.to_json_bytes = patched
    return nc


def _make_tc_class():
    import concourse.tile as tile
    from concourse.vector_clock import ScopedClock
    import bass_rust

    class TC(tile.TileContext):
        """TileContext with the tail drain's multi-sem wait split into
        individual single-wait instructions (this walrus rejects >1 wait
        on a CTRL instruction)."""

        def _drain_and_barrier(self, tick_clock, wait_clock):
            nc = self.nc
            probe = nc.sync.drain()
            wait_clock.add_sem_waits(
                probe.ins, ScopedClock({None: tick_clock.global_clock})
            )
            waits = list(probe.ins.sync_info.on_wait)
            probe.ins.sync_info = bass_rust.SyncInfo(on_wait=[], on_update=[])
            by_name = {hh.name: hh for hh in self.sems.allocated().values()}
            for wt in waits:
                nc.sync.wait_ge(by_name[wt.ant_name], wt.wait_value)
            nc.all_engine_barrier()
            popped = nc._tile_sem_poison_stack.pop()
            assert popped is self._sem_poison
            nc.clear_and_free_semaphores(list(self.sems.allocated().values()))
            nc.all_engine_barrier()

    return TC


def _build_program(plans, perm, wyb_cols_n, wy8_cols_n, wx_cols_n,
                   slots=SLOTS, p1bufs=2, rbufs=6, p2bufs=2, evac_mod=3,
                   evac_pat=None, wdma="gs", odma="y", s2_lag=0,
                   wbufs=2, map_lead=1):
    import concourse.bass as bass
    import concourse.mybir as mybir
    from contextlib import ExitStack

    FP32 = mybir.dt.float32
    BF16 = mybir.dt.bfloat16
    F8E4 = mybir.dt.float8e4
    DR = mybir.MatmulPerfMode.DoubleRow

    any_fp8 = any(p is not None and p["fp8"] for p in plans)
    # pooled-map variants needed: (pool factor, fp8?)
    variants = sorted({(p["xp"], p["fp8"]) for p in plans
                       if p is not None and p["xp"] > 1})

    nc = bass.Bass()
    # feature map, already in SBUF layout [128, (c, t, x)] / [128, (c, tp, q, x)]
    f_d = nc.dram_tensor("f", [PART, C_LOC * NT * W], BF16, kind="ExternalInput")
    if any_fp8:
        f8_d = nc.dram_tensor("f8", [PART, C_LOC * 2 * 2 * W], F8E4,
                              kind="ExternalInput")
    g_ds = {}
    for (xp, isf8) in variants:
        nm = f"g{xp}{'f8' if isf8 else ''}"
        if isf8:
            g_ds[(xp, True)] = nc.dram_tensor(
                nm, [PART, C_LOC * 2 * 2 * UPS_PAD[xp]], F8E4,
                kind="ExternalInput")
        else:
            g_ds[(xp, False)] = nc.dram_tensor(
                nm, [PART, C_LOC * NT * UPS[xp]], BF16, kind="ExternalInput")
    wyb_d = nc.dram_tensor("wyb", [PART, max(wyb_cols_n, OUT)], BF16,
                           kind="ExternalInput")
    if any_fp8:
        wy8_d = nc.dram_tensor("wy8", [PART, max(wy8_cols_n, 2 * OUT)], F8E4,
                               kind="ExternalInput")
    wx_d = nc.dram_tensor("wx", [PART, max(wx_cols_n, OUT)], BF16,
                          kind="ExternalInput")
    # output: box b = 16*G + g: cols G*1024 + g*64 + hh*32 + j,
    # partition p = c_lh*32 + i, channel c = hh*4 + c_lh
    NG16 = (N_BOXES + 15) // 16
    if odma == "P":
        out_d = nc.dram_tensor("out", [PART, (N_BOXES // 4) * 256], FP32,
                               kind="ExternalOutput")
    else:
        out_d = nc.dram_tensor("out", [PART, NG16 * 1024], BF16,
                               kind="ExternalOutput")

    # per-GROUP chunk column ranges (groups are processing positions)
    def group_ranges(key, width):
        rng = []
        for g0 in range(0, N_BOXES, GROUP):
            los, his = [], []
            for n in perm[g0:g0 + GROUP]:
                p = plans[n]
                if p is None or key not in p:
                    continue
                ntiles = len(p["ts"]) if key == "wyb_off" else (
                    len(p["tp"]) if key == "wy8_off" else len(p["xws"]))
                los.append(p[key])
                his.append(p[key] + ntiles * width)
            rng.append((min(los), max(his)) if los else None)
        return rng

    wyb_rng = group_ranges("wyb_off", OUT)
    wy8_rng = group_ranges("wy8_off", 2 * OUT)
    wx_rng = group_ranges("wx_off", OUT)

    TC = _make_tc_class()
    with TC(nc) as tc, ExitStack() as ctx:
        fpool = ctx.enter_context(tc.tile_pool(name="fmap", bufs=1))
        wpool = ctx.enter_context(tc.tile_pool(name="wts", bufs=wbufs))
        rpool = ctx.enter_context(tc.tile_pool(name="rhs2", bufs=rbufs))
        opool = ctx.enter_context(tc.tile_pool(name="osb", bufs=2))
        p1pool = ctx.enter_context(tc.tile_pool(name="psumT", bufs=p1bufs, space="PSUM"))
        p2pool = ctx.enter_context(tc.tile_pool(name="psum2", bufs=p2bufs, space="PSUM"))

        _eng_map = {"g": nc.gpsimd, "s": nc.scalar, "y": nc.sync}

        def _wy_eng():
            return _eng_map[wdma[0]]

        def _wx_eng():
            return _eng_map[wdma[1]]

        # weight chunk state
        wy_tiles = {}  # group index -> {"b": (tile, base), "8": (tile, base)}
        wx_tiles = {}  # group index -> (tile, base)

        def load_chunks(gi):
            if gi in wy_tiles:
                return
            wy_tiles[gi] = {}
            if wyb_rng[gi] is not None:
                lo, hi = wyb_rng[gi]
                tb = wpool.tile([PART, hi - lo], BF16, name="wyb_sb", tag="wyb")
                _wy_eng().dma_start(out=tb[:], in_=wyb_d[:, lo:hi])
                wy_tiles[gi]["b"] = (tb, lo)
            if wy8_rng[gi] is not None:
                lo, hi = wy8_rng[gi]
                t8 = wpool.tile([PART, hi - lo], F8E4, name="wy8_sb", tag="wy8")
                _wy_eng().dma_start(out=t8[:], in_=wy8_d[:, lo:hi])
                wy_tiles[gi]["8"] = (t8, lo)
            if wx_rng[gi] is not None:
                lo, hi = wx_rng[gi]
                tx = wpool.tile([PART, hi - lo], BF16, name="wx_sb", tag="wx")
                _wx_eng().dma_start(out=tx[:], in_=wx_d[:, lo:hi])
                wx_tiles[gi] = (tx, lo)

        # group-0 weights first so PE can start as soon as the first F
        # tiles land (same FIFO queue as the F DMAs below)
        load_chunks(0)

        f_sb = fpool.tile([PART, C_LOC * NT * W], BF16, name="f_sb")
        f_v = f_sb[:].rearrange("p (c t x) -> p c t x", c=C_LOC, t=NT)
        f_dv = f_d[:].rearrange("p (c t x) -> p c t x", c=C_LOC, t=NT)
        for t in range(NT):
            nc.sync.dma_start(out=f_v[:, :, t, :], in_=f_dv[:, :, t, :])

        # f8 / pooled-map uploads are deferred to just before the first
        # processing group that uses them, so the (FIFO) input queue
        # delivers weights and maps in need order instead of stalling
        # later weight chunks behind maps nobody needs yet.
        map_dmas = {}  # key -> emit thunk
        first_grp = {}  # key -> first processing group using the map

        def _key_of(p):
            if p["xp"] > 1:
                return (p["xp"], p["fp8"])
            return "f8" if p["fp8"] else None

        for pos, n in enumerate(perm):
            p = plans[n]
            if p is None:
                continue
            kkey = _key_of(p)
            if kkey is not None and kkey not in first_grp:
                first_grp[kkey] = pos // GROUP

        if any_fp8:
            f8_sb = fpool.tile([PART, C_LOC * 2 * 2 * W], F8E4, name="f8_sb")
            f8_v = f8_sb[:].rearrange("p (c tp q x) -> p c tp q x",
                                      c=C_LOC, tp=2, q=2)
            f8_dv = f8_d[:].rearrange("p (c tp q x) -> p c tp q x",
                                      c=C_LOC, tp=2, q=2)

            def _emit_f8(f8_v=f8_v, f8_dv=f8_dv):
                for tp in range(2):
                    nc.sync.dma_start(out=f8_v[:, :, tp, :, :],
                                      in_=f8_dv[:, :, tp, :, :])
            map_dmas["f8"] = _emit_f8
        g_vs = {}
        for (xp, isf8), gd in g_ds.items():
            if isf8:
                upw = UPS_PAD[xp]
                gt = fpool.tile([PART, C_LOC * 2 * 2 * upw], F8E4,
                                name=f"g{xp}f8_sb", tag=f"g{xp}f8")
                gv = gt[:].rearrange("p (c tp q x) -> p c tp q x",
                                     c=C_LOC, tp=2, q=2)

                def _emit(gv=gv, gd=gd):
                    nc.sync.dma_start(out=gv, in_=gd[:].rearrange(
                        "p (c tp q x) -> p c tp q x", c=C_LOC, tp=2, q=2))
            else:
                upw = UPS[xp]
                gt = fpool.tile([PART, C_LOC * NT * upw], BF16,
                                name=f"g{xp}_sb", tag=f"g{xp}")
                gv = gt[:].rearrange("p (c t x) -> p c t x", c=C_LOC, t=NT)

                def _emit(gv=gv, gd=gd):
                    nc.sync.dma_start(out=gv, in_=gd[:].rearrange(
                        "p (c t x) -> p c t x", c=C_LOC, t=NT))
            map_dmas[(xp, isf8)] = _emit
            g_vs[(xp, isf8)] = gv

        def emit_maps_due(gi):
            """Emit deferred map DMAs with map_lead groups of lead time."""
            for kkey, thunk in list(map_dmas.items()):
                if first_grp.get(kkey, 0) <= gi + map_lead:
                    thunk()
                    del map_dmas[kkey]
        emit_maps_due(0)

        evac_busy = [0.0, 0.0]  # DVE, ACT modeled busy ns

        def evac(out_ap, in_ap, idx):
            # GPSIMD cannot read PSUM on real HW: DVE / ACT only.
            free = in_ap.free_size()
            costs = (free * 1.0417 + 125.0, free * 0.8333 + 185.0)
            if evac_pat == "auto" or evac_pat is None:
                which = 0 if evac_busy[0] + costs[0] <= evac_busy[1] + costs[1] \
                    else 1
            else:
                which = {"v": 0, "s": 1}[evac_pat[idx % len(evac_pat)]]
            evac_busy[which] += costs[which]
            if which == 0:
                nc.vector.tensor_copy(out=out_ap, in_=in_ap)
            else:
                nc.scalar.copy(out_ap, in_ap)

        evac_ctr = 0
        # (box, window) slot stream state
        cur_p1 = None          # current psum_t tile
        cur_slots = []         # [(box, win_idx)]
        rhs2_of = {}           # (box, win) -> (tile, slot)
        pend_s2 = []           # boxes whose stage-2 is not yet emitted

        o_sb = None
        psum2 = None

        def flush_p1():
            nonlocal cur_p1, cur_slots, evac_ctr
            if cur_p1 is None:
                return
            r_t = rpool.tile([PART, slots * 256], BF16, name="r_t", tag="r")
            evac(r_t[:], cur_p1[:], evac_ctr)
            evac_ctr += 1
            for si, key in enumerate(cur_slots):
                rhs2_of[key] = (r_t, si)
            cur_p1 = None
            cur_slots = []

        def emit_stage2(n):
            """stage 2 + output for box n (requires rhs2 of all windows).
            Output slots are by processing position; host unpermutes."""
            nonlocal psum2, o_sb, evac_ctr
            p = plans[n]
            pos = pos_of[n]
            g16, s16 = pos // 16, pos % 16
            oct_, sq = s16 // 8, s16 % 8
            if s16 == 0:
                o_sb = opool.tile([PART, 1024], BF16, name="o_sb", tag="o")
            if sq == 0:
                psum2 = p2pool.tile([PART, 512], FP32, name="ps2", tag="ps2")
            if p is None:
                # write *something* finite so the tile is defined
                for hh in range(2):
                    nc.tensor.matmul(
                        out=psum2[:, sq * 64 + hh * 32: sq * 64 + hh * 32 + 32],
                        lhsT=f_v[:, 0, 0, 0:PART], rhs=f_v[:, 0, 0, 0:OUT],
                        start=True, stop=True)
            else:
                nw = len(p["xws"])
                for hh in range(2):
                    for k in range(nw):
                        r_t, si = rhs2_of[(n, k)]
                        wx_sl = wx_sb_view(n, k)
                        nc.tensor.matmul(
                            out=psum2[:, sq * 64 + hh * 32: sq * 64 + hh * 32 + 32],
                            lhsT=r_t[:, si * 256 + hh * 128: si * 256 + (hh + 1) * 128],
                            rhs=wx_sl,
                            start=(k == 0), stop=(k == nw - 1))
                for k in range(nw):
                    rhs2_of.pop((n, k), None)
            if sq == 7:
                evac(o_sb[:, oct_ * 512:(oct_ + 1) * 512], psum2[:], evac_ctr)
                evac_ctr += 1
            if s16 == 15:
                _eng_map[odma].dma_start(
                    out=out_d[:, g16 * 1024:(g16 + 1) * 1024], in_=o_sb[:])

        pos_of = {n: i for i, n in enumerate(perm)}

        def wx_sb_view(n, k):
            t, base = wx_tiles[pos_of[n] // GROUP]
            off = plans[n]["wx_off"] + k * OUT - base
            return t[:, off:off + OUT]

        NGRP = (N_BOXES + GROUP - 1) // GROUP
        for g0 in range(0, N_BOXES, GROUP):
            gi = g0 // GROUP
            emit_maps_due(gi)
            for la in range(wbufs - 1):
                if gi + la < NGRP:
                    load_chunks(gi + la)
            if wyb_rng[gi] is not None:
                wyb_sb, wyb_base = wy_tiles[gi]["b"]
            if wy8_rng[gi] is not None:
                wy8_sb, wy8_base = wy_tiles[gi]["8"]

            for n in perm[g0:g0 + GROUP]:
                p = plans[n]
                if p is not None:
                    for k, x0 in enumerate(p["xws"]):
                        if cur_p1 is None:
                            cur_p1 = p1pool.tile([PART, slots * 256], FP32,
                                                 name="ps1", tag="ps1")
                        si = len(cur_slots)
                        cur_slots.append((n, k))
                        base = si * 256
                        if p["fp8"]:
                            src = g_vs[(p["xp"], True)] if p["xp"] > 1 else f8_v
                            off = p["wy8_off"] - wy8_base
                            for c in range(C_LOC):
                                for j, tp in enumerate(p["tp"]):
                                    nc.tensor.matmul(
                                        out=cur_p1[:, base + c * OUT: base + (c + 1) * OUT],
                                        lhsT=src[:, c, tp, :, x0:x0 + PART],
                                        rhs=wy8_sb[:, off + j * 64: off + (j + 1) * 64]
                                            .rearrange("p (q i) -> p q i", q=2),
                                        start=(j == 0), stop=(j == len(p["tp"]) - 1),
                                        perf_mode=DR)
                        else:
                            src = g_vs[(p["xp"], False)] if p["xp"] > 1 else f_v
                            off = p["wyb_off"] - wyb_base
                            for c in range(C_LOC):
                                for j, t in enumerate(p["ts"]):
                                    nc.tensor.matmul(
                                        out=cur_p1[:, base + c * OUT: base + (c + 1) * OUT],
                                        lhsT=src[:, c, t, x0:x0 + PART],
                                        rhs=wyb_sb[:, off + j * OUT: off + (j + 1) * OUT],
                                        start=(j == 0), stop=(j == len(p["ts"]) - 1))
                        if len(cur_slots) == slots:
                            flush_p1()
                pend_s2.append(n)
                # emit stage 2 for boxes whose windows are all evacuated
                while pend_s2:
                    b = pend_s2[0]
                    pb = plans[b]
                    if pb is not None and any(
                            (b, k) not in rhs2_of for k in range(len(pb["xws"]))):
                        break
                    if pos_of[n] - pos_of[b] < s2_lag:
                        break
                    emit_stage2(b)
                    pend_s2.pop(0)
        flush_p1()
        while pend_s2:
            emit_stage2(pend_s2.pop(0))

    return _patch_serialization(nc)


# ---------------------------------------------------------------------------
# Entry point
# ---------------------------------------------------------------------------

_LAST = {}


def kernel(feature_map, boxes, output_width):
    from concourse.bass_utils import run_bass_kernel_spmd
    import ml_dtypes

    feature_map = np.asarray(feature_map, dtype=np.float32)
    boxes_np = np.asarray(boxes, dtype=np.float32)
    assert int(output_width) == OUT

    wy_all, wx_all = host_geometry(boxes_np)
    plans, perm, wyb_flat, wy8_flat, wx_flat = plan_boxes(
        boxes_np, wy_all, wx_all)
    nc = _build_program(plans, perm, wyb_flat.shape[1], wy8_flat.shape[1],
                        wx_flat.shape[1], **BUILD_KW)

    any_fp8 = any(p is not None and p["fp8"] for p in plans)
    variants = sorted({(p["xp"], p["fp8"]) for p in plans
                       if p is not None and p["xp"] > 1})
    pools_needed = sorted({xp for (xp, _) in variants})
    wyb_u = wyb_flat.astype(ml_dtypes.bfloat16)
    wx_u = wx_flat.astype(ml_dtypes.bfloat16)
    if any_fp8:
        wy8_u = wy8_flat.astype(ml_dtypes.float8_e4m3)
    bases = {xp: xpool_basis(xp) for xp in pools_needed}

    in_maps = []
    for kcore in range(N_CORES):
        # f layout [p, (c, t, x)]: y = t*128 + p
        f_k = feature_map[:, :, kcore * C_LOC:(kcore + 1) * C_LOC]  # [y, x, c]
        f_bf = f_k.astype(ml_dtypes.bfloat16).astype(np.float32)
        f_sb = np.ascontiguousarray(
            f_bf.reshape(NT, PART, W, C_LOC).transpose(1, 3, 0, 2)
        ).reshape(PART, C_LOC * NT * W).astype(ml_dtypes.bfloat16)
        m = {"f": f_sb, "wyb": wyb_u, "wx": wx_u}
        if any_fp8:
            # f8 layout [p, (c, tp, q, x)]: y = tp*256 + q*128 + p
            f8_sb = np.ascontiguousarray(
                f_bf.reshape(2, 2, PART, W, C_LOC).transpose(2, 4, 0, 1, 3)
            ).reshape(PART, C_LOC * 2 * 2 * W).astype(ml_dtypes.float8_e4m3)
            m["f8"] = f8_sb
            m["wy8"] = wy8_u
        g_ks = {xp: np.einsum("xu,yxc->yuc", bases[xp], f_bf, optimize=True)
                for xp in pools_needed}
        for (xp, isf8) in variants:
            g_k = g_ks[xp]
            nm = f"g{xp}{'f8' if isf8 else ''}"
            if isf8:
                upw = UPS_PAD[xp]
                g_pad = np.zeros((H, upw, C_LOC), np.float32)
                g_pad[:, :g_k.shape[1], :] = g_k
                m[nm] = np.ascontiguousarray(
                    g_pad.reshape(2, 2, PART, upw, C_LOC).transpose(2, 4, 0, 1, 3)
                ).reshape(PART, C_LOC * 2 * 2 * upw).astype(ml_dtypes.float8_e4m3)
            else:
                upw = g_k.shape[1]
                m[nm] = np.ascontiguousarray(
                    g_k.reshape(NT, PART, upw, C_LOC).transpose(1, 3, 0, 2)
                ).reshape(PART, C_LOC * NT * upw).astype(ml_dtypes.bfloat16)
        in_maps.append(m)

    _LAST["nc"] = nc
    _LAST["in_maps"] = in_maps
    res = run_bass_kernel_spmd(nc, in_maps, list(range(N_CORES)))

    out = np.zeros((N_BOXES, OUT, OUT, C), np.float32)
    perm_np = np.asarray(perm)
    for kcore in range(N_CORES):
        dev = np.asarray(res.results[kcore]["out"]).astype(np.float32)
        # [p, (G, g, hh, j)] with p = c_lh*32 + i, slot = 16G + g =
        # processing position; unpermute to original box order
        v = dev.reshape(4, OUT, N_BOXES // 16, 16, 2, OUT)  # c_lh, i, G, g, hh, j
        v = v.transpose(2, 3, 1, 5, 4, 0)                   # G, g, i, j, hh, c_lh
        v = v.reshape(N_BOXES, OUT, OUT, C_LOC)
        out[perm_np, :, :, kcore * C_LOC:(kcore + 1) * C_LOC] = v
    for n in range(N_BOXES):
        if plans[n] is None:
            out[n] = 0.0
    return out


def estimate_hw_ns():
    """Cost-model estimate of the per-core kernel duration (ns)."""
    from concourse.timeline_sim import TimelineSim
    nc = _LAST.get("nc")
    if nc is None:
        return -1
    sim = TimelineSim(nc)
    sim.simulate()
    return int(sim.time)


def measure_wall(n=5):
    """Wall-clock of repeated dispatches (includes axon round trips)."""
    import time
    from concourse.bass_utils import run_bass_kernel_spmd
    times = []
    for _ in range(n):
        t0 = time.perf_counter()
        run_bass_kernel_spmd(_LAST["nc"], _LAST["in_maps"], list(range(N_CORES)))
        times.append(time.perf_counter() - t0)
    return times


# revision 67
# speedup vs baseline: 1.1924x; 1.0029x over previous
"""RoIAlign (scale_and_translate, linear, antialias) Trainium2 kernel.

Channel-sharded across 8 NeuronCores: each core keeps a [512, 512, 8]
slice of the feature map resident in SBUF and computes all 512 boxes
for its 8 channels, one box at a time (no box grouping):

  stage 1 (PE):  T^T[x, (c, i)] = sum_y F[y, x, c] * Wy[y, i]
                 one matmul per (c, y-tile, x-window), out free = 32.
                 Wide-kernel boxes (ky*kx >= 12) run fp8e4 DoubleRow
                 (two 128-row y-tiles per matmul, 0.5 cycles/row); the
                 rest run bf16.
  evac:          psum_t [128, 512] (two (box, window) slots) -> SBUF
                 bf16, least-loaded assignment over DVE / ACT (GPSIMD
                 cannot read PSUM).
  stage 2 (PE):  out[(c,i), j] = sum_x T^T[x, ci] * Wx[x, j],
                 operand-swapped (lhsT = T^T chunk, moving = 32 Wx
                 cols) so out free = 32 instead of 256.
  out:           psum2 [128, 512] (8 boxes) -> bf16 staging -> DRAM.

Boxes with wide x-kernels use device-resident 2x/4x x-pooled copies of
the feature map (hat-function pooling; per-box Wx is least-squares
refit onto the hat basis, with delta columns at the image edge). This
shrinks the x-band, cutting stage-1 work and - critically - the
PSUM-evacuation volume, which is the binding engine resource.

Host side computes exact per-box dense resampling weights, extracts
nonzero bands, zero-pads to 128-row tiles/windows, and streams them as
flat [128, cols] arrays in 32-box chunks. fp8 pooled maps are padded
to multiple-of-16 widths (DoubleRow ldweights stride restriction).
"""

import numpy as np

H = 512
W = 512
C = 64
N_BOXES = 512
OUT = 32
N_CORES = 8
C_LOC = C // N_CORES  # 8 channels per core
PART = 128
NT = H // PART        # 4 y partition tiles
GROUP = 32            # boxes per weight-DMA chunk
SLOTS = 4             # (box, window) slots per psum_t tile
FP8_KPROD = 12.0      # use fp8 stage-1 when ky*kx >= this
FP8_KMIN = 2.0        # ... and both ky, kx >= this
XP2_KMIN = 10.0       # use the 2x x-pooled map when kx >= this
XP4_KMIN = 13.0       # use the 4x x-pooled map when kx >= this
WCLIP = 1e-3          # zero fitted pooled weights below this (rel) magnitude
UP2 = W // 2 + 1      # 2x-pooled x size: hats at even x + delta at x=511
UP4 = (W - 1) // 4 + 1 + 3  # 4x pooled: 128 hat nodes + 3 tail deltas = 131
UPS = {2: UP2, 4: UP4}
# fp8 DoubleRow ldweights requires the pair-dim byte stride to be a
# multiple of 16 -> pad the fp8 pooled-map widths up to a multiple of 16
UPS_PAD = {p: (u + 15) // 16 * 16 for p, u in UPS.items()}
INTERLEAVE_HEAD = 101  # pure-bf16 boxes before class interleaving starts

# scheduling knobs for _build_program (tuned via TimelineSim)
BUILD_KW = dict(slots=4, p1bufs=3, rbufs=8, p2bufs=2, evac_pat="auto",
                wdma="yy", odma="y", s2_lag=16, wbufs=3, map_lead=1)


# ---------------------------------------------------------------------------
# Host-side weight computation (mirrors jax.image.scale_and_translate with
# method="linear", antialias=True)
# ---------------------------------------------------------------------------

def _compute_weight_mat(in_size, out_size, scale, translation):
    inv_scale = 1.0 / scale
    kernel_scale = max(inv_scale, 1.0)
    sample_f = (np.arange(out_size, dtype=np.float64) + 0.5) * inv_scale \
        - translation * inv_scale - 0.5
    x = np.abs(sample_f[None, :] - np.arange(in_size, dtype=np.float64)[:, None]) \
        / kernel_scale
    weights = np.maximum(0.0, 1.0 - x)
    total = weights.sum(axis=0, keepdims=True)
    weights = np.where(
        np.abs(total) > 1000.0 * float(np.finfo(np.float32).eps),
        weights / np.where(total != 0, total, 1.0),
        0.0,
    )
    valid = (sample_f >= -0.5) & (sample_f <= in_size - 0.5)
    return np.where(valid[None, :], weights, 0.0).astype(np.float32)


def host_geometry(boxes):
    """Exact per-box dense weights wy/wx [N, 512, 32] fp32."""
    boxes = np.asarray(boxes, dtype=np.float64)
    wy_all = np.zeros((N_BOXES, H, OUT), np.float32)
    wx_all = np.zeros((N_BOXES, W, OUT), np.float32)
    for n in range(N_BOXES):
        cx, cy, w, h = boxes[n]
        x0 = cx - w / 2
        y0 = cy - h / 2
        w = max(w, 1e-6)
        h = max(h, 1e-6)
        x_scale = OUT / (w * W)
        y_scale = OUT / (h * H)
        wy_all[n] = _compute_weight_mat(H, OUT, y_scale, -y0 * OUT / h)
        wx_all[n] = _compute_weight_mat(W, OUT, x_scale, -x0 * OUT / w)
    return wy_all, wx_all


def xpool_basis(p):
    """Hat basis at stride-p grid nodes plus delta columns for the tail
    pixels past the last node (which a hat grid cannot represent)."""
    nu = (W - 1) // p + 1
    extra = [x for x in range(W) if x > (nu - 1) * p]
    P = np.zeros((W, nu + len(extra)), np.float32)
    for u in range(nu):
        c = p * u
        for x in range(max(0, c - p + 1), min(W, c + p)):
            P[x, u] = 1.0 - abs(x - c) / p
    for j, x in enumerate(extra):
        P[x, :] = 0.0
        P[x, nu + j] = 1.0
    return P


def xpool_fit_mats():
    """Per pooling factor: (P, PINV) with PINV = (P^T P)^-1 P^T."""
    mats = {}
    for p in (2, 4):
        P = xpool_basis(p)
        PINV = np.linalg.solve((P.T @ P).astype(np.float64),
                               P.T.astype(np.float64))
        mats[p] = (P, PINV)
    return mats


def plan_boxes(boxes, wy_all, wx_all):
    """Per-box banded geometry + flat weight packing.

    Returns plan dicts (None for boxes with empty bands) and the packed
    flat weight arrays wyb [128, :] bf16, wy8 [128, :] fp8, wx [128, :]
    bf16 (as float32 here; cast at upload).
    """
    boxes = np.asarray(boxes, dtype=np.float64)
    ky = np.maximum(16.0 * boxes[:, 3], 1.0)  # y kernel halfwidth (px)
    kx = np.maximum(16.0 * boxes[:, 2], 1.0)

    mats = xpool_fit_mats()

    # processing order: boxes whose source map arrives earliest on the
    # input DMA queue go first (f -> f8 -> pooled bf16 -> pooled fp8), so
    # compute covers the input-upload stream instead of stalling on it.
    # Within the first class, boxes needing only low y-tiles go first so
    # PE can start right after the first F-tile DMA lands.
    def klass(n):
        f8 = (ky[n] * kx[n] >= FP8_KPROD
              and ky[n] >= FP8_KMIN and kx[n] >= FP8_KMIN)
        xp = 4 if kx[n] >= XP4_KMIN else (2 if kx[n] >= XP2_KMIN else 1)
        if xp == 1:
            return 0 if not f8 else 1
        return 2 if not f8 else 3

    def last_tile(n):
        nz = np.nonzero(wy_all[n].any(axis=1))[0]
        return (int(nz[-1]) // PART) if len(nz) else 0

    by_class = {0: [], 1: [], 2: [], 3: []}
    for n in range(N_BOXES):
        by_class[klass(n)].append(n)
    by_class[0].sort(key=lambda n: (last_tile(n), n))
    # head: pure bf16 boxes to cover the f/f8 upload; then interleave the
    # PE-heavy (bf16) remainder evenly among the evac-heavy fp8 boxes so
    # neither PE nor the evac engines starves in class-segregated bursts
    NA = min(INTERLEAVE_HEAD, len(by_class[0]))
    head = by_class[0][:NA]
    pe_list = by_class[0][NA:] + by_class[2]   # bf16: PE-heavy
    ev_list = by_class[1] + by_class[3]        # fp8: evac-heavy
    total = len(pe_list) + len(ev_list)
    merged = []
    pi = ei = 0
    acc = 0.0
    r = len(pe_list) / max(total, 1)
    for _ in range(total):
        acc += r
        if acc >= 1.0 and pi < len(pe_list):
            merged.append(pe_list[pi]); pi += 1; acc -= 1.0
        elif ei < len(ev_list):
            merged.append(ev_list[ei]); ei += 1
        else:
            merged.append(pe_list[pi]); pi += 1
    perm = head + merged

    plans = [None] * N_BOXES
    wyb_cols, wy8_cols, wx_cols = [], [], []
    for n in perm:
        ynz = np.nonzero(wy_all[n].any(axis=1))[0]
        xnz = np.nonzero(wx_all[n].any(axis=1))[0]
        if len(ynz) == 0 or len(xnz) == 0:
            continue
        r0, r1 = int(ynz[0]), int(ynz[-1]) + 1
        use_fp8 = (ky[n] * kx[n] >= FP8_KPROD
                   and ky[n] >= FP8_KMIN and kx[n] >= FP8_KMIN)
        xp = 4 if kx[n] >= XP4_KMIN else (2 if kx[n] >= XP2_KMIN else 1)

        if xp > 1:
            # least-squares fit of Wx in the pooled hat basis, clipping
            # the tiny ringing tail of the fit to keep the band compact
            P, PINV = mats[xp]
            wx_n = (PINV @ wx_all[n].astype(np.float64)).astype(np.float32)
            wx_n[np.abs(wx_n) < WCLIP * np.abs(wx_n).max()] = 0.0
            xnz = np.nonzero(wx_n.any(axis=1))[0]
            if len(xnz) == 0:
                continue
            WW = UPS[xp]
        else:
            wx_n = wx_all[n]
            WW = W
        c0, c1 = int(xnz[0]), int(xnz[-1]) + 1

        # x windows (arbitrary free-dim offset, clamped; overlap zeroed)
        nxw = (c1 - c0 + PART - 1) // PART
        xws = [min(c0 + k * PART, max(WW - PART, 0)) for k in range(nxw)]

        p = {"fp8": use_fp8, "xp": xp, "xws": xws}
        if use_fp8:
            p_lo, p_hi = r0 // (2 * PART), (r1 - 1) // (2 * PART)
            p["tp"] = list(range(p_lo, p_hi + 1))
            p["wy8_off"] = len(wy8_cols) and sum(c.shape[1] for c in wy8_cols)
            p["wy8_off"] = sum(c.shape[1] for c in wy8_cols)
            for tp in p["tp"]:
                # [128, (q, i)] with y = tp*256 + q*128 + p
                blk = np.zeros((PART, 2, OUT), np.float32)
                for q in range(2):
                    blk[:, q, :] = wy_all[n][tp * 256 + q * 128: tp * 256 + (q + 1) * 128]
                wy8_cols.append(blk.reshape(PART, 2 * OUT))
        else:
            t_lo, t_hi = r0 // PART, (r1 - 1) // PART
            p["ts"] = list(range(t_lo, t_hi + 1))
            p["wyb_off"] = sum(c.shape[1] for c in wyb_cols)
            for t in p["ts"]:
                wyb_cols.append(wy_all[n][t * PART:(t + 1) * PART].copy())

        p["wx_off"] = sum(c.shape[1] for c in wx_cols)
        prev_end = c0
        for x0 in xws:
            wxw = wx_n[x0:x0 + PART].copy()
            lo = max(prev_end - x0, 0)
            wxw[:lo] = 0.0
            prev_end = max(prev_end, x0 + PART)
            wx_cols.append(wxw)
        plans[n] = p

    def cat(cols, width):
        if not cols:
            return np.zeros((PART, width), np.float32)
        return np.concatenate(cols, axis=1)

    wyb_flat = cat(wyb_cols, OUT)
    wy8_flat = cat(wy8_cols, 2 * OUT)
    wx_flat = cat(wx_cols, OUT)
    return plans, perm, wyb_flat, wy8_flat, wx_flat


# ---------------------------------------------------------------------------
# Device program
# ---------------------------------------------------------------------------

def _split_multiwait_bir(raw: bytes) -> bytes:
    """The walrus build here accepts only one sync wait per instruction.
    Hoist extra waits onto single-wait EventSemaphore instructions inserted
    just before, on the same engine (per-engine order is preserved)."""
    import orjson

    d = orjson.loads(raw)
    ctr = 0
    for fn in d.get("functions", []):
        for bb in fn.get("blocks") or []:
            out = []
            for ins in bb["instructions"]:
                si = ins.get("sync_info")
                ws = (si or {}).get("on_wait") or []
                if len(ws) > 1:
                    for w in ws[:-1]:
                        ctr += 1
                        out.append({
                            "debug": ins.get("debug", 0),
                            "engine": ins["engine"],
                            "ins": [],
                            "outs": [],
                            "name": f"{ins['name']}-xw{ctr}",
                            "opcode": "EventSemaphore",
                            "sync_info": {"on_update": [], "on_wait": [w]},
                        })
                    si["on_wait"] = [ws[-1]]
                out.append(ins)
            bb["instructions"] = out
    return orjson.dumps(d)


def _patch_serialization(nc):
    orig = nc.to_json_bytes

    def patched():
        return _split_multiwait_bir(orig())

    nc.to_json_bytes = patched
    return nc


def _make_tc_class():
    import concourse.tile as tile
    from concourse.vector_clock import ScopedClock
    import bass_rust

    class TC(tile.TileContext):
        """TileContext with the tail drain's multi-sem wait split into
        individual single-wait instructions (this walrus rejects >1 wait
        on a CTRL instruction)."""

        def _drain_and_barrier(self, tick_clock, wait_clock):
            nc = self.nc
            probe = nc.sync.drain()
            wait_clock.add_sem_waits(
                probe.ins, ScopedClock({None: tick_clock.global_clock})
            )
            waits = list(probe.ins.sync_info.on_wait)
            probe.ins.sync_info = bass_rust.SyncInfo(on_wait=[], on_update=[])
            by_name = {hh.name: hh for hh in self.sems.allocated().values()}
            for wt in waits:
                nc.sync.wait_ge(by_name[wt.ant_name], wt.wait_value)
            nc.all_engine_barrier()
            popped = nc._tile_sem_poison_stack.pop()
            assert popped is self._sem_poison
            nc.clear_and_free_semaphores(list(self.sems.allocated().values()))
            nc.all_engine_barrier()

    return TC


def _build_program(plans, perm, wyb_cols_n, wy8_cols_n, wx_cols_n,
                   slots=SLOTS, p1bufs=2, rbufs=6, p2bufs=2, evac_mod=3,
                   evac_pat=None, wdma="gs", odma="y", s2_lag=0,
                   wbufs=2, map_lead=1):
    import concourse.bass as bass
    import concourse.mybir as mybir
    from contextlib import ExitStack

    FP32 = mybir.dt.float32
    BF16 = mybir.dt.bfloat16
    F8E4 = mybir.dt.float8e4
    DR = mybir.MatmulPerfMode.DoubleRow

    any_fp8 = any(p is not None and p["fp8"] for p in plans)
    # pooled-map variants needed: (pool factor, fp8?)
    variants = sorted({(p["xp"], p["fp8"]) for p in plans
                       if p is not None and p["xp"] > 1})

    nc = bass.Bass()
    # feature map, already in SBUF layout [128, (c, t, x)] / [128, (c, tp, q, x)]
    f_d = nc.dram_tensor("f", [PART, C_LOC * NT * W], BF16, kind="ExternalInput")
    if any_fp8:
        f8_d = nc.dram_tensor("f8", [PART, C_LOC * 2 * 2 * W], F8E4,
                              kind="ExternalInput")
    g_ds = {}
    for (xp, isf8) in variants:
        nm = f"g{xp}{'f8' if isf8 else ''}"
        if isf8:
            g_ds[(xp, True)] = nc.dram_tensor(
                nm, [PART, C_LOC * 2 * 2 * UPS_PAD[xp]], F8E4,
                kind="ExternalInput")
        else:
            g_ds[(xp, False)] = nc.dram_tensor(
                nm, [PART, C_LOC * NT * UPS[xp]], BF16, kind="ExternalInput")
    wyb_d = nc.dram_tensor("wyb", [PART, max(wyb_cols_n, OUT)], BF16,
                           kind="ExternalInput")
    if any_fp8:
        wy8_d = nc.dram_tensor("wy8", [PART, max(wy8_cols_n, 2 * OUT)], F8E4,
                               kind="ExternalInput")
    wx_d = nc.dram_tensor("wx", [PART, max(wx_cols_n, OUT)], BF16,
                          kind="ExternalInput")
    # output: box b = 16*G + g: cols G*1024 + g*64 + hh*32 + j,
    # partition p = c_lh*32 + i, channel c = hh*4 + c_lh
    NG16 = (N_BOXES + 15) // 16
    if odma == "P":
        out_d = nc.dram_tensor("out", [PART, (N_BOXES // 4) * 256], FP32,
                               kind="ExternalOutput")
    else:
        out_d = nc.dram_tensor("out", [PART, NG16 * 1024], BF16,
                               kind="ExternalOutput")

    # per-GROUP chunk column ranges (groups are processing positions)
    def group_ranges(key, width):
        rng = []
        for g0 in range(0, N_BOXES, GROUP):
            los, his = [], []
            for n in perm[g0:g0 + GROUP]:
                p = plans[n]
                if p is None or key not in p:
                    continue
                ntiles = len(p["ts"]) if key == "wyb_off" else (
                    len(p["tp"]) if key == "wy8_off" else len(p["xws"]))
                los.append(p[key])
                his.append(p[key] + ntiles * width)
            rng.append((min(los), max(his)) if los else None)
        return rng

    wyb_rng = group_ranges("wyb_off", OUT)
    wy8_rng = group_ranges("wy8_off", 2 * OUT)
    wx_rng = group_ranges("wx_off", OUT)

    TC = _make_tc_class()
    with TC(nc) as tc, ExitStack() as ctx:
        fpool = ctx.enter_context(tc.tile_pool(name="fmap", bufs=1))
        wpool = ctx.enter_context(tc.tile_pool(name="wts", bufs=wbufs))
        rpool = ctx.enter_context(tc.tile_pool(name="rhs2", bufs=rbufs))
        opool = ctx.enter_context(tc.tile_pool(name="osb", bufs=2))
        p1pool = ctx.enter_context(tc.tile_pool(name="psumT", bufs=p1bufs, space="PSUM"))
        p2pool = ctx.enter_context(tc.tile_pool(name="psum2", bufs=p2bufs, space="PSUM"))

        _eng_map = {"g": nc.gpsimd, "s": nc.scalar, "y": nc.sync}

        def _wy_eng():
            return _eng_map[wdma[0]]

        def _wx_eng():
            return _eng_map[wdma[1]]

        # weight chunk state
        wy_tiles = {}  # group index -> {"b": (tile, base), "8": (tile, base)}
        wx_tiles = {}  # group index -> (tile, base)

        def load_chunks(gi):
            if gi in wy_tiles:
                return
            wy_tiles[gi] = {}
            if wyb_rng[gi] is not None:
                lo, hi = wyb_rng[gi]
                tb = wpool.tile([PART, hi - lo], BF16, name="wyb_sb", tag="wyb")
                _wy_eng().dma_start(out=tb[:], in_=wyb_d[:, lo:hi])
                wy_tiles[gi]["b"] = (tb, lo)
            if wy8_rng[gi] is not None:
                lo, hi = wy8_rng[gi]
                t8 = wpool.tile([PART, hi - lo], F8E4, name="wy8_sb", tag="wy8")
                _wy_eng().dma_start(out=t8[:], in_=wy8_d[:, lo:hi])
                wy_tiles[gi]["8"] = (t8, lo)
            if wx_rng[gi] is not None:
                lo, hi = wx_rng[gi]
                tx = wpool.tile([PART, hi - lo], BF16, name="wx_sb", tag="wx")
                _wx_eng().dma_start(out=tx[:], in_=wx_d[:, lo:hi])
                wx_tiles[gi] = (tx, lo)

        f_sb = fpool.tile([PART, C_LOC * NT * W], BF16, name="f_sb")
        f_v = f_sb[:].rearrange("p (c t x) -> p c t x", c=C_LOC, t=NT)
        f_dv = f_d[:].rearrange("p (c t x) -> p c t x", c=C_LOC, t=NT)
        # queue order: first F tile, then group-0 weights (both needed by
        # the first boxes), then the remaining F tiles
        nc.sync.dma_start(out=f_v[:, :, 0, :], in_=f_dv[:, :, 0, :])
        load_chunks(0)
        for t in range(1, NT):
            nc.sync.dma_start(out=f_v[:, :, t, :], in_=f_dv[:, :, t, :])

        # f8 / pooled-map uploads are deferred to just before the first
        # processing group that uses them, so the (FIFO) input queue
        # delivers weights and maps in need order instead of stalling
        # later weight chunks behind maps nobody needs yet.
        map_dmas = {}  # key -> emit thunk
        first_grp = {}  # key -> first processing group using the map

        def _key_of(p):
            if p["xp"] > 1:
                return (p["xp"], p["fp8"])
            return "f8" if p["fp8"] else None

        for pos, n in enumerate(perm):
            p = plans[n]
            if p is None:
                continue
            kkey = _key_of(p)
            if kkey is not None and kkey not in first_grp:
                first_grp[kkey] = pos // GROUP

        if any_fp8:
            f8_sb = fpool.tile([PART, C_LOC * 2 * 2 * W], F8E4, name="f8_sb")
            f8_v = f8_sb[:].rearrange("p (c tp q x) -> p c tp q x",
                                      c=C_LOC, tp=2, q=2)
            f8_dv = f8_d[:].rearrange("p (c tp q x) -> p c tp q x",
                                      c=C_LOC, tp=2, q=2)

            def _emit_f8(f8_v=f8_v, f8_dv=f8_dv):
                for tp in range(2):
                    nc.sync.dma_start(out=f8_v[:, :, tp, :, :],
                                      in_=f8_dv[:, :, tp, :, :])
            map_dmas["f8"] = _emit_f8
        g_vs = {}
        for (xp, isf8), gd in g_ds.items():
            if isf8:
                upw = UPS_PAD[xp]
                gt = fpool.tile([PART, C_LOC * 2 * 2 * upw], F8E4,
                                name=f"g{xp}f8_sb", tag=f"g{xp}f8")
                gv = gt[:].rearrange("p (c tp q x) -> p c tp q x",
                                     c=C_LOC, tp=2, q=2)

                def _emit(gv=gv, gd=gd):
                    nc.sync.dma_start(out=gv, in_=gd[:].rearrange(
                        "p (c tp q x) -> p c tp q x", c=C_LOC, tp=2, q=2))
            else:
                upw = UPS[xp]
                gt = fpool.tile([PART, C_LOC * NT * upw], BF16,
                                name=f"g{xp}_sb", tag=f"g{xp}")
                gv = gt[:].rearrange("p (c t x) -> p c t x", c=C_LOC, t=NT)

                def _emit(gv=gv, gd=gd):
                    nc.sync.dma_start(out=gv, in_=gd[:].rearrange(
                        "p (c t x) -> p c t x", c=C_LOC, t=NT))
            map_dmas[(xp, isf8)] = _emit
            g_vs[(xp, isf8)] = gv

        def emit_maps_due(gi):
            """Emit deferred map DMAs with map_lead groups of lead time."""
            for kkey, thunk in list(map_dmas.items()):
                if first_grp.get(kkey, 0) <= gi + map_lead:
                    thunk()
                    del map_dmas[kkey]
        emit_maps_due(0)

        evac_busy = [0.0, 0.0]  # DVE, ACT modeled busy ns

        def evac(out_ap, in_ap, idx):
            # GPSIMD cannot read PSUM on real HW: DVE / ACT only.
            free = in_ap.free_size()
            costs = (free * 1.0417 + 125.0, free * 0.8333 + 185.0)
            if evac_pat == "auto" or evac_pat is None:
                which = 0 if evac_busy[0] + costs[0] <= evac_busy[1] + costs[1] \
                    else 1
            else:
                which = {"v": 0, "s": 1}[evac_pat[idx % len(evac_pat)]]
            evac_busy[which] += costs[which]
            if which == 0:
                nc.vector.tensor_copy(out=out_ap, in_=in_ap)
            else:
                nc.scalar.copy(out_ap, in_ap)

        evac_ctr = 0
        # (box, window) slot stream state
        cur_p1 = None          # current psum_t tile
        cur_slots = []         # [(box, win_idx)]
        rhs2_of = {}           # (box, win) -> (tile, slot)
        pend_s2 = []           # boxes whose stage-2 is not yet emitted

        o_sb = None
        psum2 = None

        def flush_p1():
            nonlocal cur_p1, cur_slots, evac_ctr
            if cur_p1 is None:
                return
            r_t = rpool.tile([PART, slots * 256], BF16, name="r_t", tag="r")
            evac(r_t[:], cur_p1[:], evac_ctr)
            evac_ctr += 1
            for si, key in enumerate(cur_slots):
                rhs2_of[key] = (r_t, si)
            cur_p1 = None
            cur_slots = []

        def emit_stage2(n):
            """stage 2 + output for box n (requires rhs2 of all windows).
            Output slots are by processing position; host unpermutes."""
            nonlocal psum2, o_sb, evac_ctr
            p = plans[n]
            pos = pos_of[n]
            g16, s16 = pos // 16, pos % 16
            oct_, sq = s16 // 8, s16 % 8
            if s16 == 0:
                o_sb = opool.tile([PART, 1024], BF16, name="o_sb", tag="o")
            if sq == 0:
                psum2 = p2pool.tile([PART, 512], FP32, name="ps2", tag="ps2")
            if p is None:
                # write *something* finite so the tile is defined
                for hh in range(2):
                    nc.tensor.matmul(
                        out=psum2[:, sq * 64 + hh * 32: sq * 64 + hh * 32 + 32],
                        lhsT=f_v[:, 0, 0, 0:PART], rhs=f_v[:, 0, 0, 0:OUT],
                        start=True, stop=True)
            else:
                nw = len(p["xws"])
                for hh in range(2):
                    for k in range(nw):
                        r_t, si = rhs2_of[(n, k)]
                        wx_sl = wx_sb_view(n, k)
                        nc.tensor.matmul(
                            out=psum2[:, sq * 64 + hh * 32: sq * 64 + hh * 32 + 32],
                            lhsT=r_t[:, si * 256 + hh * 128: si * 256 + (hh + 1) * 128],
                            rhs=wx_sl,
                            start=(k == 0), stop=(k == nw - 1))
                for k in range(nw):
                    rhs2_of.pop((n, k), None)
            if sq == 7:
                evac(o_sb[:, oct_ * 512:(oct_ + 1) * 512], psum2[:], evac_ctr)
                evac_ctr += 1
            if s16 == 15:
                _eng_map[odma].dma_start(
                    out=out_d[:, g16 * 1024:(g16 + 1) * 1024], in_=o_sb[:])

        pos_of = {n: i for i, n in enumerate(perm)}

        def wx_sb_view(n, k):
            t, base = wx_tiles[pos_of[n] // GROUP]
            off = plans[n]["wx_off"] + k * OUT - base
            return t[:, off:off + OUT]

        NGRP = (N_BOXES + GROUP - 1) // GROUP
        for g0 in range(0, N_BOXES, GROUP):
            gi = g0 // GROUP
            emit_maps_due(gi)
            for la in range(wbufs - 1):
                if gi + la < NGRP:
                    load_chunks(gi + la)
            if wyb_rng[gi] is not None:
                wyb_sb, wyb_base = wy_tiles[gi]["b"]
            if wy8_rng[gi] is not None:
                wy8_sb, wy8_base = wy_tiles[gi]["8"]

            for n in perm[g0:g0 + GROUP]:
                p = plans[n]
                if p is not None:
                    for k, x0 in enumerate(p["xws"]):
                        if cur_p1 is None:
                            cur_p1 = p1pool.tile([PART, slots * 256], FP32,
                                                 name="ps1", tag="ps1")
                        si = len(cur_slots)
                        cur_slots.append((n, k))
                        base = si * 256
                        if p["fp8"]:
                            src = g_vs[(p["xp"], True)] if p["xp"] > 1 else f8_v
                            off = p["wy8_off"] - wy8_base
                            for c in range(C_LOC):
                                for j, tp in enumerate(p["tp"]):
                                    nc.tensor.matmul(
                                        out=cur_p1[:, base + c * OUT: base + (c + 1) * OUT],
                                        lhsT=src[:, c, tp, :, x0:x0 + PART],
                                        rhs=wy8_sb[:, off + j * 64: off + (j + 1) * 64]
                                            .rearrange("p (q i) -> p q i", q=2),
                                        start=(j == 0), stop=(j == len(p["tp"]) - 1),
                                        perf_mode=DR)
                        else:
                            src = g_vs[(p["xp"], False)] if p["xp"] > 1 else f_v
                            off = p["wyb_off"] - wyb_base
                            for c in range(C_LOC):
                                for j, t in enumerate(p["ts"]):
                                    nc.tensor.matmul(
                                        out=cur_p1[:, base + c * OUT: base + (c + 1) * OUT],
                                        lhsT=src[:, c, t, x0:x0 + PART],
                                        rhs=wyb_sb[:, off + j * OUT: off + (j + 1) * OUT],
                                        start=(j == 0), stop=(j == len(p["ts"]) - 1))
                        if len(cur_slots) == slots:
                            flush_p1()
                pend_s2.append(n)
                # emit stage 2 for boxes whose windows are all evacuated
                while pend_s2:
                    b = pend_s2[0]
                    pb = plans[b]
                    if pb is not None and any(
                            (b, k) not in rhs2_of for k in range(len(pb["xws"]))):
                        break
                    if pos_of[n] - pos_of[b] < s2_lag:
                        break
                    emit_stage2(b)
                    pend_s2.pop(0)
        flush_p1()
        while pend_s2:
            emit_stage2(pend_s2.pop(0))

    return _patch_serialization(nc)


# ---------------------------------------------------------------------------
# Entry point
# ---------------------------------------------------------------------------

_LAST = {}


def kernel(feature_map, boxes, output_width):
    from concourse.bass_utils import run_bass_kernel_spmd
    import ml_dtypes

    feature_map = np.asarray(feature_map, dtype=np.float32)
    boxes_np = np.asarray(boxes, dtype=np.float32)
    assert int(output_width) == OUT

    wy_all, wx_all = host_geometry(boxes_np)
    plans, perm, wyb_flat, wy8_flat, wx_flat = plan_boxes(
        boxes_np, wy_all, wx_all)
    nc = _build_program(plans, perm, wyb_flat.shape[1], wy8_flat.shape[1],
                        wx_flat.shape[1], **BUILD_KW)

    any_fp8 = any(p is not None and p["fp8"] for p in plans)
    variants = sorted({(p["xp"], p["fp8"]) for p in plans
                       if p is not None and p["xp"] > 1})
    pools_needed = sorted({xp for (xp, _) in variants})
    wyb_u = wyb_flat.astype(ml_dtypes.bfloat16)
    wx_u = wx_flat.astype(ml_dtypes.bfloat16)
    if any_fp8:
        wy8_u = wy8_flat.astype(ml_dtypes.float8_e4m3)
    bases = {xp: xpool_basis(xp) for xp in pools_needed}

    in_maps = []
    for kcore in range(N_CORES):
        # f layout [p, (c, t, x)]: y = t*128 + p
        f_k = feature_map[:, :, kcore * C_LOC:(kcore + 1) * C_LOC]  # [y, x, c]
        f_bf = f_k.astype(ml_dtypes.bfloat16).astype(np.float32)
        f_sb = np.ascontiguousarray(
            f_bf.reshape(NT, PART, W, C_LOC).transpose(1, 3, 0, 2)
        ).reshape(PART, C_LOC * NT * W).astype(ml_dtypes.bfloat16)
        m = {"f": f_sb, "wyb": wyb_u, "wx": wx_u}
        if any_fp8:
            # f8 layout [p, (c, tp, q, x)]: y = tp*256 + q*128 + p
            f8_sb = np.ascontiguousarray(
                f_bf.reshape(2, 2, PART, W, C_LOC).transpose(2, 4, 0, 1, 3)
            ).reshape(PART, C_LOC * 2 * 2 * W).astype(ml_dtypes.float8_e4m3)
            m["f8"] = f8_sb
            m["wy8"] = wy8_u
        g_ks = {xp: np.einsum("xu,yxc->yuc", bases[xp], f_bf, optimize=True)
                for xp in pools_needed}
        for (xp, isf8) in variants:
            g_k = g_ks[xp]
            nm = f"g{xp}{'f8' if isf8 else ''}"
            if isf8:
                upw = UPS_PAD[xp]
                g_pad = np.zeros((H, upw, C_LOC), np.float32)
                g_pad[:, :g_k.shape[1], :] = g_k
                m[nm] = np.ascontiguousarray(
                    g_pad.reshape(2, 2, PART, upw, C_LOC).transpose(2, 4, 0, 1, 3)
                ).reshape(PART, C_LOC * 2 * 2 * upw).astype(ml_dtypes.float8_e4m3)
            else:
                upw = g_k.shape[1]
                m[nm] = np.ascontiguousarray(
                    g_k.reshape(NT, PART, upw, C_LOC).transpose(1, 3, 0, 2)
                ).reshape(PART, C_LOC * NT * upw).astype(ml_dtypes.bfloat16)
        in_maps.append(m)

    _LAST["nc"] = nc
    _LAST["in_maps"] = in_maps
    res = run_bass_kernel_spmd(nc, in_maps, list(range(N_CORES)))

    out = np.zeros((N_BOXES, OUT, OUT, C), np.float32)
    perm_np = np.asarray(perm)
    for kcore in range(N_CORES):
        dev = np.asarray(res.results[kcore]["out"]).astype(np.float32)
        # [p, (G, g, hh, j)] with p = c_lh*32 + i, slot = 16G + g =
        # processing position; unpermute to original box order
        v = dev.reshape(4, OUT, N_BOXES // 16, 16, 2, OUT)  # c_lh, i, G, g, hh, j
        v = v.transpose(2, 3, 1, 5, 4, 0)                   # G, g, i, j, hh, c_lh
        v = v.reshape(N_BOXES, OUT, OUT, C_LOC)
        out[perm_np, :, :, kcore * C_LOC:(kcore + 1) * C_LOC] = v
    for n in range(N_BOXES):
        if plans[n] is None:
            out[n] = 0.0
    return out


def estimate_hw_ns():
    """Cost-model estimate of the per-core kernel duration (ns)."""
    from concourse.timeline_sim import TimelineSim
    nc = _LAST.get("nc")
    if nc is None:
        return -1
    sim = TimelineSim(nc)
    sim.simulate()
    return int(sim.time)


def measure_wall(n=5):
    """Wall-clock of repeated dispatches (includes axon round trips)."""
    import time
    from concourse.bass_utils import run_bass_kernel_spmd
    times = []
    for _ in range(n):
        t0 = time.perf_counter()
        run_bass_kernel_spmd(_LAST["nc"], _LAST["in_maps"], list(range(N_CORES)))
        times.append(time.perf_counter() - t0)
    return times


# revision 68
# speedup vs baseline: 1.1928x; 1.0003x over previous
"""RoIAlign (scale_and_translate, linear, antialias) Trainium2 kernel.

Channel-sharded across 8 NeuronCores: each core keeps a [512, 512, 8]
slice of the feature map resident in SBUF and computes all 512 boxes
for its 8 channels, one box at a time (no box grouping):

  stage 1 (PE):  T^T[x, (c, i)] = sum_y F[y, x, c] * Wy[y, i]
                 one matmul per (c, y-tile, x-window), out free = 32.
                 Wide-kernel boxes (ky*kx >= 12) run fp8e4 DoubleRow
                 (two 128-row y-tiles per matmul, 0.5 cycles/row); the
                 rest run bf16.
  evac:          psum_t [128, 512] (two (box, window) slots) -> SBUF
                 bf16, least-loaded assignment over DVE / ACT (GPSIMD
                 cannot read PSUM).
  stage 2 (PE):  out[(c,i), j] = sum_x T^T[x, ci] * Wx[x, j],
                 operand-swapped (lhsT = T^T chunk, moving = 32 Wx
                 cols) so out free = 32 instead of 256.
  out:           psum2 [128, 512] (8 boxes) -> bf16 staging -> DRAM.

Boxes with wide x-kernels use device-resident 2x/4x x-pooled copies of
the feature map (hat-function pooling; per-box Wx is least-squares
refit onto the hat basis, with delta columns at the image edge). This
shrinks the x-band, cutting stage-1 work and - critically - the
PSUM-evacuation volume, which is the binding engine resource.

Host side computes exact per-box dense resampling weights, extracts
nonzero bands, zero-pads to 128-row tiles/windows, and streams them as
flat [128, cols] arrays in 32-box chunks. fp8 pooled maps are padded
to multiple-of-16 widths (DoubleRow ldweights stride restriction).
"""

import numpy as np

H = 512
W = 512
C = 64
N_BOXES = 512
OUT = 32
N_CORES = 8
C_LOC = C // N_CORES  # 8 channels per core
PART = 128
NT = H // PART        # 4 y partition tiles
GROUP = 32            # boxes per weight-DMA chunk
SLOTS = 4             # (box, window) slots per psum_t tile
FP8_KPROD = 12.0      # use fp8 stage-1 when ky*kx >= this
FP8_KMIN = 2.0        # ... and both ky, kx >= this
XP2_KMIN = 10.0       # use the 2x x-pooled map when kx >= this
XP4_KMIN = 13.0       # use the 4x x-pooled map when kx >= this
WCLIP = 1e-3          # zero fitted pooled weights below this (rel) magnitude
UP2 = W // 2 + 1      # 2x-pooled x size: hats at even x + delta at x=511
UP4 = (W - 1) // 4 + 1 + 3  # 4x pooled: 128 hat nodes + 3 tail deltas = 131
UPS = {2: UP2, 4: UP4}
# fp8 DoubleRow ldweights requires the pair-dim byte stride to be a
# multiple of 16 -> pad the fp8 pooled-map widths up to a multiple of 16
UPS_PAD = {p: (u + 15) // 16 * 16 for p, u in UPS.items()}
INTERLEAVE_HEAD = 101  # pure-bf16 boxes before class interleaving starts

# scheduling knobs for _build_program (tuned via TimelineSim)
BUILD_KW = dict(slots=4, p1bufs=3, rbufs=8, p2bufs=2, evac_pat="auto",
                wdma="yy", odma="y", s2_lag=16, wbufs=3, map_lead=1)


# ---------------------------------------------------------------------------
# Host-side weight computation (mirrors jax.image.scale_and_translate with
# method="linear", antialias=True)
# ---------------------------------------------------------------------------

def _compute_weight_mat(in_size, out_size, scale, translation):
    inv_scale = 1.0 / scale
    kernel_scale = max(inv_scale, 1.0)
    sample_f = (np.arange(out_size, dtype=np.float64) + 0.5) * inv_scale \
        - translation * inv_scale - 0.5
    x = np.abs(sample_f[None, :] - np.arange(in_size, dtype=np.float64)[:, None]) \
        / kernel_scale
    weights = np.maximum(0.0, 1.0 - x)
    total = weights.sum(axis=0, keepdims=True)
    weights = np.where(
        np.abs(total) > 1000.0 * float(np.finfo(np.float32).eps),
        weights / np.where(total != 0, total, 1.0),
        0.0,
    )
    valid = (sample_f >= -0.5) & (sample_f <= in_size - 0.5)
    return np.where(valid[None, :], weights, 0.0).astype(np.float32)


def host_geometry(boxes):
    """Exact per-box dense weights wy/wx [N, 512, 32] fp32."""
    boxes = np.asarray(boxes, dtype=np.float64)
    wy_all = np.zeros((N_BOXES, H, OUT), np.float32)
    wx_all = np.zeros((N_BOXES, W, OUT), np.float32)
    for n in range(N_BOXES):
        cx, cy, w, h = boxes[n]
        x0 = cx - w / 2
        y0 = cy - h / 2
        w = max(w, 1e-6)
        h = max(h, 1e-6)
        x_scale = OUT / (w * W)
        y_scale = OUT / (h * H)
        wy_all[n] = _compute_weight_mat(H, OUT, y_scale, -y0 * OUT / h)
        wx_all[n] = _compute_weight_mat(W, OUT, x_scale, -x0 * OUT / w)
    return wy_all, wx_all


def xpool_basis(p):
    """Hat basis at stride-p grid nodes plus delta columns for the tail
    pixels past the last node (which a hat grid cannot represent)."""
    nu = (W - 1) // p + 1
    extra = [x for x in range(W) if x > (nu - 1) * p]
    P = np.zeros((W, nu + len(extra)), np.float32)
    for u in range(nu):
        c = p * u
        for x in range(max(0, c - p + 1), min(W, c + p)):
            P[x, u] = 1.0 - abs(x - c) / p
    for j, x in enumerate(extra):
        P[x, :] = 0.0
        P[x, nu + j] = 1.0
    return P


def xpool_fit_mats():
    """Per pooling factor: (P, PINV) with PINV = (P^T P)^-1 P^T."""
    mats = {}
    for p in (2, 4):
        P = xpool_basis(p)
        PINV = np.linalg.solve((P.T @ P).astype(np.float64),
                               P.T.astype(np.float64))
        mats[p] = (P, PINV)
    return mats


def plan_boxes(boxes, wy_all, wx_all):
    """Per-box banded geometry + flat weight packing.

    Returns plan dicts (None for boxes with empty bands) and the packed
    flat weight arrays wyb [128, :] bf16, wy8 [128, :] fp8, wx [128, :]
    bf16 (as float32 here; cast at upload).
    """
    boxes = np.asarray(boxes, dtype=np.float64)
    ky = np.maximum(16.0 * boxes[:, 3], 1.0)  # y kernel halfwidth (px)
    kx = np.maximum(16.0 * boxes[:, 2], 1.0)

    mats = xpool_fit_mats()

    # processing order: boxes whose source map arrives earliest on the
    # input DMA queue go first (f -> f8 -> pooled bf16 -> pooled fp8), so
    # compute covers the input-upload stream instead of stalling on it.
    # Within the first class, boxes needing only low y-tiles go first so
    # PE can start right after the first F-tile DMA lands.
    def klass(n):
        f8 = (ky[n] * kx[n] >= FP8_KPROD
              and ky[n] >= FP8_KMIN and kx[n] >= FP8_KMIN)
        xp = 4 if kx[n] >= XP4_KMIN else (2 if kx[n] >= XP2_KMIN else 1)
        if xp == 1:
            return 0 if not f8 else 1
        return 2 if not f8 else 3

    def last_tile(n):
        nz = np.nonzero(wy_all[n].any(axis=1))[0]
        return (int(nz[-1]) // PART) if len(nz) else 0

    by_class = {0: [], 1: [], 2: [], 3: []}
    for n in range(N_BOXES):
        by_class[klass(n)].append(n)
    by_class[0].sort(key=lambda n: (last_tile(n), n))
    # head: pure bf16 boxes to cover the f/f8 upload; then interleave the
    # PE-heavy (bf16) remainder evenly among the evac-heavy fp8 boxes so
    # neither PE nor the evac engines starves in class-segregated bursts
    NA = min(INTERLEAVE_HEAD, len(by_class[0]))
    head = by_class[0][:NA]
    pe_list = by_class[0][NA:] + by_class[2]   # bf16: PE-heavy
    ev_list = by_class[1] + by_class[3]        # fp8: evac-heavy
    total = len(pe_list) + len(ev_list)
    merged = []
    pi = ei = 0
    acc = 0.0
    r = len(pe_list) / max(total, 1)
    for _ in range(total):
        acc += r
        if acc >= 1.0 and pi < len(pe_list):
            merged.append(pe_list[pi]); pi += 1; acc -= 1.0
        elif ei < len(ev_list):
            merged.append(ev_list[ei]); ei += 1
        else:
            merged.append(pe_list[pi]); pi += 1
    perm = head + merged

    plans = [None] * N_BOXES
    wyb_cols, wy8_cols, wx_cols = [], [], []
    for n in perm:
        ynz = np.nonzero(wy_all[n].any(axis=1))[0]
        xnz = np.nonzero(wx_all[n].any(axis=1))[0]
        if len(ynz) == 0 or len(xnz) == 0:
            continue
        r0, r1 = int(ynz[0]), int(ynz[-1]) + 1
        use_fp8 = (ky[n] * kx[n] >= FP8_KPROD
                   and ky[n] >= FP8_KMIN and kx[n] >= FP8_KMIN)
        xp = 4 if kx[n] >= XP4_KMIN else (2 if kx[n] >= XP2_KMIN else 1)

        if xp > 1:
            # least-squares fit of Wx in the pooled hat basis, clipping
            # the tiny ringing tail of the fit to keep the band compact
            P, PINV = mats[xp]
            wx_n = (PINV @ wx_all[n].astype(np.float64)).astype(np.float32)
            wx_n[np.abs(wx_n) < WCLIP * np.abs(wx_n).max()] = 0.0
            xnz = np.nonzero(wx_n.any(axis=1))[0]
            if len(xnz) == 0:
                continue
            WW = UPS[xp]
        else:
            wx_n = wx_all[n]
            WW = W
        c0, c1 = int(xnz[0]), int(xnz[-1]) + 1

        # x windows (arbitrary free-dim offset, clamped; overlap zeroed)
        nxw = (c1 - c0 + PART - 1) // PART
        xws = [min(c0 + k * PART, max(WW - PART, 0)) for k in range(nxw)]

        p = {"fp8": use_fp8, "xp": xp, "xws": xws}
        if use_fp8:
            p_lo, p_hi = r0 // (2 * PART), (r1 - 1) // (2 * PART)
            p["tp"] = list(range(p_lo, p_hi + 1))
            p["wy8_off"] = len(wy8_cols) and sum(c.shape[1] for c in wy8_cols)
            p["wy8_off"] = sum(c.shape[1] for c in wy8_cols)
            for tp in p["tp"]:
                # [128, (q, i)] with y = tp*256 + q*128 + p
                blk = np.zeros((PART, 2, OUT), np.float32)
                for q in range(2):
                    blk[:, q, :] = wy_all[n][tp * 256 + q * 128: tp * 256 + (q + 1) * 128]
                wy8_cols.append(blk.reshape(PART, 2 * OUT))
        else:
            t_lo, t_hi = r0 // PART, (r1 - 1) // PART
            p["ts"] = list(range(t_lo, t_hi + 1))
            p["wyb_off"] = sum(c.shape[1] for c in wyb_cols)
            for t in p["ts"]:
                wyb_cols.append(wy_all[n][t * PART:(t + 1) * PART].copy())

        p["wx_off"] = sum(c.shape[1] for c in wx_cols)
        prev_end = c0
        for x0 in xws:
            wxw = wx_n[x0:x0 + PART].copy()
            lo = max(prev_end - x0, 0)
            wxw[:lo] = 0.0
            prev_end = max(prev_end, x0 + PART)
            wx_cols.append(wxw)
        plans[n] = p

    def cat(cols, width):
        if not cols:
            return np.zeros((PART, width), np.float32)
        return np.concatenate(cols, axis=1)

    wyb_flat = cat(wyb_cols, OUT)
    wy8_flat = cat(wy8_cols, 2 * OUT)
    wx_flat = cat(wx_cols, OUT)
    return plans, perm, wyb_flat, wy8_flat, wx_flat


# ---------------------------------------------------------------------------
# Device program
# ---------------------------------------------------------------------------

def _split_multiwait_bir(raw: bytes) -> bytes:
    """The walrus build here accepts only one sync wait per instruction.
    Hoist extra waits onto single-wait EventSemaphore instructions inserted
    just before, on the same engine (per-engine order is preserved)."""
    import orjson

    d = orjson.loads(raw)
    ctr = 0
    for fn in d.get("functions", []):
        for bb in fn.get("blocks") or []:
            out = []
            for ins in bb["instructions"]:
                si = ins.get("sync_info")
                ws = (si or {}).get("on_wait") or []
                if len(ws) > 1:
                    for w in ws[:-1]:
                        ctr += 1
                        out.append({
                            "debug": ins.get("debug", 0),
                            "engine": ins["engine"],
                            "ins": [],
                            "outs": [],
                            "name": f"{ins['name']}-xw{ctr}",
                            "opcode": "EventSemaphore",
                            "sync_info": {"on_update": [], "on_wait": [w]},
                        })
                    si["on_wait"] = [ws[-1]]
                out.append(ins)
            bb["instructions"] = out
    return orjson.dumps(d)


def _patch_serialization(nc):
    orig = nc.to_json_bytes

    def patched():
        return _split_multiwait_bir(orig())

    nc.to_json_bytes = patched
    return nc


def _make_tc_class():
    import concourse.tile as tile
    from concourse.vector_clock import ScopedClock
    import bass_rust

    class TC(tile.TileContext):
        """TileContext with the tail drain's multi-sem wait split into
        individual single-wait instructions (this walrus rejects >1 wait
        on a CTRL instruction)."""

        def _drain_and_barrier(self, tick_clock, wait_clock):
            nc = self.nc
            probe = nc.sync.drain()
            wait_clock.add_sem_waits(
                probe.ins, ScopedClock({None: tick_clock.global_clock})
            )
            waits = list(probe.ins.sync_info.on_wait)
            probe.ins.sync_info = bass_rust.SyncInfo(on_wait=[], on_update=[])
            by_name = {hh.name: hh for hh in self.sems.allocated().values()}
            for wt in waits:
                nc.sync.wait_ge(by_name[wt.ant_name], wt.wait_value)
            nc.all_engine_barrier()
            popped = nc._tile_sem_poison_stack.pop()
            assert popped is self._sem_poison
            nc.clear_and_free_semaphores(list(self.sems.allocated().values()))
            nc.all_engine_barrier()

    return TC


def _build_program(plans, perm, wyb_cols_n, wy8_cols_n, wx_cols_n,
                   slots=SLOTS, p1bufs=2, rbufs=6, p2bufs=2, evac_mod=3,
                   evac_pat=None, wdma="gs", odma="y", s2_lag=0,
                   wbufs=2, map_lead=1):
    import concourse.bass as bass
    import concourse.mybir as mybir
    from contextlib import ExitStack

    FP32 = mybir.dt.float32
    BF16 = mybir.dt.bfloat16
    F8E4 = mybir.dt.float8e4
    DR = mybir.MatmulPerfMode.DoubleRow

    any_fp8 = any(p is not None and p["fp8"] for p in plans)
    # pooled-map variants needed: (pool factor, fp8?)
    variants = sorted({(p["xp"], p["fp8"]) for p in plans
                       if p is not None and p["xp"] > 1})

    nc = bass.Bass()
    # feature map, already in SBUF layout [128, (c, t, x)] / [128, (c, tp, q, x)]
    f_d = nc.dram_tensor("f", [PART, C_LOC * NT * W], BF16, kind="ExternalInput")
    if any_fp8:
        f8_d = nc.dram_tensor("f8", [PART, C_LOC * 2 * 2 * W], F8E4,
                              kind="ExternalInput")
    g_ds = {}
    for (xp, isf8) in variants:
        nm = f"g{xp}{'f8' if isf8 else ''}"
        if isf8:
            g_ds[(xp, True)] = nc.dram_tensor(
                nm, [PART, C_LOC * 2 * 2 * UPS_PAD[xp]], F8E4,
                kind="ExternalInput")
        else:
            g_ds[(xp, False)] = nc.dram_tensor(
                nm, [PART, C_LOC * NT * UPS[xp]], BF16, kind="ExternalInput")
    wyb_d = nc.dram_tensor("wyb", [PART, max(wyb_cols_n, OUT)], BF16,
                           kind="ExternalInput")
    if any_fp8:
        wy8_d = nc.dram_tensor("wy8", [PART, max(wy8_cols_n, 2 * OUT)], F8E4,
                               kind="ExternalInput")
    wx_d = nc.dram_tensor("wx", [PART, max(wx_cols_n, OUT)], BF16,
                          kind="ExternalInput")
    # output: box b = 16*G + g: cols G*1024 + g*64 + hh*32 + j,
    # partition p = c_lh*32 + i, channel c = hh*4 + c_lh
    NG16 = (N_BOXES + 15) // 16
    if odma == "P":
        out_d = nc.dram_tensor("out", [PART, (N_BOXES // 4) * 256], FP32,
                               kind="ExternalOutput")
    else:
        out_d = nc.dram_tensor("out", [PART, NG16 * 1024], BF16,
                               kind="ExternalOutput")

    # per-GROUP chunk column ranges (groups are processing positions)
    def group_ranges(key, width):
        rng = []
        for g0 in range(0, N_BOXES, GROUP):
            los, his = [], []
            for n in perm[g0:g0 + GROUP]:
                p = plans[n]
                if p is None or key not in p:
                    continue
                ntiles = len(p["ts"]) if key == "wyb_off" else (
                    len(p["tp"]) if key == "wy8_off" else len(p["xws"]))
                los.append(p[key])
                his.append(p[key] + ntiles * width)
            rng.append((min(los), max(his)) if los else None)
        return rng

    wyb_rng = group_ranges("wyb_off", OUT)
    wy8_rng = group_ranges("wy8_off", 2 * OUT)
    wx_rng = group_ranges("wx_off", OUT)

    TC = _make_tc_class()
    with TC(nc) as tc, ExitStack() as ctx:
        fpool = ctx.enter_context(tc.tile_pool(name="fmap", bufs=1))
        wpool = ctx.enter_context(tc.tile_pool(name="wts", bufs=wbufs))
        rpool = ctx.enter_context(tc.tile_pool(name="rhs2", bufs=rbufs))
        opool = ctx.enter_context(tc.tile_pool(name="osb", bufs=2))
        p1pool = ctx.enter_context(tc.tile_pool(name="psumT", bufs=p1bufs, space="PSUM"))
        p2pool = ctx.enter_context(tc.tile_pool(name="psum2", bufs=p2bufs, space="PSUM"))

        _eng_map = {"g": nc.gpsimd, "s": nc.scalar, "y": nc.sync}

        def _wy_eng():
            return _eng_map[wdma[0]]

        def _wx_eng():
            return _eng_map[wdma[1]]

        # weight chunk state
        wy_tiles = {}  # group index -> {"b": (tile, base), "8": (tile, base)}
        wx_tiles = {}  # group index -> (tile, base)

        def load_chunks(gi):
            if gi in wy_tiles:
                return
            wy_tiles[gi] = {}
            if wyb_rng[gi] is not None:
                lo, hi = wyb_rng[gi]
                tb = wpool.tile([PART, hi - lo], BF16, name="wyb_sb", tag="wyb")
                _wy_eng().dma_start(out=tb[:], in_=wyb_d[:, lo:hi])
                wy_tiles[gi]["b"] = (tb, lo)
            if wy8_rng[gi] is not None:
                lo, hi = wy8_rng[gi]
                t8 = wpool.tile([PART, hi - lo], F8E4, name="wy8_sb", tag="wy8")
                _wy_eng().dma_start(out=t8[:], in_=wy8_d[:, lo:hi])
                wy_tiles[gi]["8"] = (t8, lo)
            if wx_rng[gi] is not None:
                lo, hi = wx_rng[gi]
                tx = wpool.tile([PART, hi - lo], BF16, name="wx_sb", tag="wx")
                _wx_eng().dma_start(out=tx[:], in_=wx_d[:, lo:hi])
                wx_tiles[gi] = (tx, lo)

        f_sb = fpool.tile([PART, C_LOC * NT * W], BF16, name="f_sb")
        f_v = f_sb[:].rearrange("p (c t x) -> p c t x", c=C_LOC, t=NT)
        f_dv = f_d[:].rearrange("p (c t x) -> p c t x", c=C_LOC, t=NT)
        # queue order: first channels of the first F tile, then group-0
        # weights (both needed by the very first matmuls), then the rest
        nc.sync.dma_start(out=f_v[:, 0:4, 0, :], in_=f_dv[:, 0:4, 0, :])
        load_chunks(0)
        nc.sync.dma_start(out=f_v[:, 4:8, 0, :], in_=f_dv[:, 4:8, 0, :])
        for t in range(1, NT):
            nc.sync.dma_start(out=f_v[:, :, t, :], in_=f_dv[:, :, t, :])

        # f8 / pooled-map uploads are deferred to just before the first
        # processing group that uses them, so the (FIFO) input queue
        # delivers weights and maps in need order instead of stalling
        # later weight chunks behind maps nobody needs yet.
        map_dmas = {}  # key -> emit thunk
        first_grp = {}  # key -> first processing group using the map

        def _key_of(p):
            if p["xp"] > 1:
                return (p["xp"], p["fp8"])
            return "f8" if p["fp8"] else None

        for pos, n in enumerate(perm):
            p = plans[n]
            if p is None:
                continue
            kkey = _key_of(p)
            if kkey is not None and kkey not in first_grp:
                first_grp[kkey] = pos // GROUP

        if any_fp8:
            f8_sb = fpool.tile([PART, C_LOC * 2 * 2 * W], F8E4, name="f8_sb")
            f8_v = f8_sb[:].rearrange("p (c tp q x) -> p c tp q x",
                                      c=C_LOC, tp=2, q=2)
            f8_dv = f8_d[:].rearrange("p (c tp q x) -> p c tp q x",
                                      c=C_LOC, tp=2, q=2)

            def _emit_f8(f8_v=f8_v, f8_dv=f8_dv):
                for tp in range(2):
                    nc.sync.dma_start(out=f8_v[:, :, tp, :, :],
                                      in_=f8_dv[:, :, tp, :, :])
            map_dmas["f8"] = _emit_f8
        g_vs = {}
        for (xp, isf8), gd in g_ds.items():
            if isf8:
                upw = UPS_PAD[xp]
                gt = fpool.tile([PART, C_LOC * 2 * 2 * upw], F8E4,
                                name=f"g{xp}f8_sb", tag=f"g{xp}f8")
                gv = gt[:].rearrange("p (c tp q x) -> p c tp q x",
                                     c=C_LOC, tp=2, q=2)

                def _emit(gv=gv, gd=gd):
                    nc.sync.dma_start(out=gv, in_=gd[:].rearrange(
                        "p (c tp q x) -> p c tp q x", c=C_LOC, tp=2, q=2))
            else:
                upw = UPS[xp]
                gt = fpool.tile([PART, C_LOC * NT * upw], BF16,
                                name=f"g{xp}_sb", tag=f"g{xp}")
                gv = gt[:].rearrange("p (c t x) -> p c t x", c=C_LOC, t=NT)

                def _emit(gv=gv, gd=gd):
                    nc.sync.dma_start(out=gv, in_=gd[:].rearrange(
                        "p (c t x) -> p c t x", c=C_LOC, t=NT))
            map_dmas[(xp, isf8)] = _emit
            g_vs[(xp, isf8)] = gv

        def emit_maps_due(gi):
            """Emit deferred map DMAs with map_lead groups of lead time."""
            for kkey, thunk in list(map_dmas.items()):
                if first_grp.get(kkey, 0) <= gi + map_lead:
                    thunk()
                    del map_dmas[kkey]
        emit_maps_due(0)

        evac_busy = [0.0, 0.0]  # DVE, ACT modeled busy ns

        def evac(out_ap, in_ap, idx):
            # GPSIMD cannot read PSUM on real HW: DVE / ACT only.
            free = in_ap.free_size()
            costs = (free * 1.0417 + 125.0, free * 0.8333 + 185.0)
            if evac_pat == "auto" or evac_pat is None:
                which = 0 if evac_busy[0] + costs[0] <= evac_busy[1] + costs[1] \
                    else 1
            else:
                which = {"v": 0, "s": 1}[evac_pat[idx % len(evac_pat)]]
            evac_busy[which] += costs[which]
            if which == 0:
                nc.vector.tensor_copy(out=out_ap, in_=in_ap)
            else:
                nc.scalar.copy(out_ap, in_ap)

        evac_ctr = 0
        # (box, window) slot stream state
        cur_p1 = None          # current psum_t tile
        cur_slots = []         # [(box, win_idx)]
        rhs2_of = {}           # (box, win) -> (tile, slot)
        pend_s2 = []           # boxes whose stage-2 is not yet emitted

        o_sb = None
        psum2 = None

        def flush_p1():
            nonlocal cur_p1, cur_slots, evac_ctr
            if cur_p1 is None:
                return
            r_t = rpool.tile([PART, slots * 256], BF16, name="r_t", tag="r")
            evac(r_t[:], cur_p1[:], evac_ctr)
            evac_ctr += 1
            for si, key in enumerate(cur_slots):
                rhs2_of[key] = (r_t, si)
            cur_p1 = None
            cur_slots = []

        def emit_stage2(n):
            """stage 2 + output for box n (requires rhs2 of all windows).
            Output slots are by processing position; host unpermutes."""
            nonlocal psum2, o_sb, evac_ctr
            p = plans[n]
            pos = pos_of[n]
            g16, s16 = pos // 16, pos % 16
            oct_, sq = s16 // 8, s16 % 8
            if s16 == 0:
                o_sb = opool.tile([PART, 1024], BF16, name="o_sb", tag="o")
            if sq == 0:
                psum2 = p2pool.tile([PART, 512], FP32, name="ps2", tag="ps2")
            if p is None:
                # write *something* finite so the tile is defined
                for hh in range(2):
                    nc.tensor.matmul(
                        out=psum2[:, sq * 64 + hh * 32: sq * 64 + hh * 32 + 32],
                        lhsT=f_v[:, 0, 0, 0:PART], rhs=f_v[:, 0, 0, 0:OUT],
                        start=True, stop=True)
            else:
                nw = len(p["xws"])
                for hh in range(2):
                    for k in range(nw):
                        r_t, si = rhs2_of[(n, k)]
                        wx_sl = wx_sb_view(n, k)
                        nc.tensor.matmul(
                            out=psum2[:, sq * 64 + hh * 32: sq * 64 + hh * 32 + 32],
                            lhsT=r_t[:, si * 256 + hh * 128: si * 256 + (hh + 1) * 128],
                            rhs=wx_sl,
                            start=(k == 0), stop=(k == nw - 1))
                for k in range(nw):
                    rhs2_of.pop((n, k), None)
            if sq == 7:
                evac(o_sb[:, oct_ * 512:(oct_ + 1) * 512], psum2[:], evac_ctr)
                evac_ctr += 1
            if s16 == 15:
                _eng_map[odma].dma_start(
                    out=out_d[:, g16 * 1024:(g16 + 1) * 1024], in_=o_sb[:])

        pos_of = {n: i for i, n in enumerate(perm)}

        def wx_sb_view(n, k):
            t, base = wx_tiles[pos_of[n] // GROUP]
            off = plans[n]["wx_off"] + k * OUT - base
            return t[:, off:off + OUT]

        NGRP = (N_BOXES + GROUP - 1) // GROUP
        for g0 in range(0, N_BOXES, GROUP):
            gi = g0 // GROUP
            emit_maps_due(gi)
            for la in range(wbufs - 1):
                if gi + la < NGRP:
                    load_chunks(gi + la)
            if wyb_rng[gi] is not None:
                wyb_sb, wyb_base = wy_tiles[gi]["b"]
            if wy8_rng[gi] is not None:
                wy8_sb, wy8_base = wy_tiles[gi]["8"]

            for n in perm[g0:g0 + GROUP]:
                p = plans[n]
                if p is not None:
                    for k, x0 in enumerate(p["xws"]):
                        if cur_p1 is None:
                            cur_p1 = p1pool.tile([PART, slots * 256], FP32,
                                                 name="ps1", tag="ps1")
                        si = len(cur_slots)
                        cur_slots.append((n, k))
                        base = si * 256
                        if p["fp8"]:
                            src = g_vs[(p["xp"], True)] if p["xp"] > 1 else f8_v
                            off = p["wy8_off"] - wy8_base
                            for c in range(C_LOC):
                                for j, tp in enumerate(p["tp"]):
                                    nc.tensor.matmul(
                                        out=cur_p1[:, base + c * OUT: base + (c + 1) * OUT],
                                        lhsT=src[:, c, tp, :, x0:x0 + PART],
                                        rhs=wy8_sb[:, off + j * 64: off + (j + 1) * 64]
                                            .rearrange("p (q i) -> p q i", q=2),
                                        start=(j == 0), stop=(j == len(p["tp"]) - 1),
                                        perf_mode=DR)
                        else:
                            src = g_vs[(p["xp"], False)] if p["xp"] > 1 else f_v
                            off = p["wyb_off"] - wyb_base
                            for c in range(C_LOC):
                                for j, t in enumerate(p["ts"]):
                                    nc.tensor.matmul(
                                        out=cur_p1[:, base + c * OUT: base + (c + 1) * OUT],
                                        lhsT=src[:, c, t, x0:x0 + PART],
                                        rhs=wyb_sb[:, off + j * OUT: off + (j + 1) * OUT],
                                        start=(j == 0), stop=(j == len(p["ts"]) - 1))
                        if len(cur_slots) == slots:
                            flush_p1()
                pend_s2.append(n)
                # emit stage 2 for boxes whose windows are all evacuated
                while pend_s2:
                    b = pend_s2[0]
                    pb = plans[b]
                    if pb is not None and any(
                            (b, k) not in rhs2_of for k in range(len(pb["xws"]))):
                        break
                    if pos_of[n] - pos_of[b] < s2_lag:
                        break
                    emit_stage2(b)
                    pend_s2.pop(0)
        flush_p1()
        while pend_s2:
            emit_stage2(pend_s2.pop(0))

    return _patch_serialization(nc)


# ---------------------------------------------------------------------------
# Entry point
# ---------------------------------------------------------------------------

_LAST = {}


def kernel(feature_map, boxes, output_width):
    from concourse.bass_utils import run_bass_kernel_spmd
    import ml_dtypes

    feature_map = np.asarray(feature_map, dtype=np.float32)
    boxes_np = np.asarray(boxes, dtype=np.float32)
    assert int(output_width) == OUT

    wy_all, wx_all = host_geometry(boxes_np)
    plans, perm, wyb_flat, wy8_flat, wx_flat = plan_boxes(
        boxes_np, wy_all, wx_all)
    nc = _build_program(plans, perm, wyb_flat.shape[1], wy8_flat.shape[1],
                        wx_flat.shape[1], **BUILD_KW)

    any_fp8 = any(p is not None and p["fp8"] for p in plans)
    variants = sorted({(p["xp"], p["fp8"]) for p in plans
                       if p is not None and p["xp"] > 1})
    pools_needed = sorted({xp for (xp, _) in variants})
    wyb_u = wyb_flat.astype(ml_dtypes.bfloat16)
    wx_u = wx_flat.astype(ml_dtypes.bfloat16)
    if any_fp8:
        wy8_u = wy8_flat.astype(ml_dtypes.float8_e4m3)
    bases = {xp: xpool_basis(xp) for xp in pools_needed}

    in_maps = []
    for kcore in range(N_CORES):
        # f layout [p, (c, t, x)]: y = t*128 + p
        f_k = feature_map[:, :, kcore * C_LOC:(kcore + 1) * C_LOC]  # [y, x, c]
        f_bf = f_k.astype(ml_dtypes.bfloat16).astype(np.float32)
        f_sb = np.ascontiguousarray(
            f_bf.reshape(NT, PART, W, C_LOC).transpose(1, 3, 0, 2)
        ).reshape(PART, C_LOC * NT * W).astype(ml_dtypes.bfloat16)
        m = {"f": f_sb, "wyb": wyb_u, "wx": wx_u}
        if any_fp8:
            # f8 layout [p, (c, tp, q, x)]: y = tp*256 + q*128 + p
            f8_sb = np.ascontiguousarray(
                f_bf.reshape(2, 2, PART, W, C_LOC).transpose(2, 4, 0, 1, 3)
            ).reshape(PART, C_LOC * 2 * 2 * W).astype(ml_dtypes.float8_e4m3)
            m["f8"] = f8_sb
            m["wy8"] = wy8_u
        g_ks = {xp: np.einsum("xu,yxc->yuc", bases[xp], f_bf, optimize=True)
                for xp in pools_needed}
        for (xp, isf8) in variants:
            g_k = g_ks[xp]
            nm = f"g{xp}{'f8' if isf8 else ''}"
            if isf8:
                upw = UPS_PAD[xp]
                g_pad = np.zeros((H, upw, C_LOC), np.float32)
                g_pad[:, :g_k.shape[1], :] = g_k
                m[nm] = np.ascontiguousarray(
                    g_pad.reshape(2, 2, PART, upw, C_LOC).transpose(2, 4, 0, 1, 3)
                ).reshape(PART, C_LOC * 2 * 2 * upw).astype(ml_dtypes.float8_e4m3)
            else:
                upw = g_k.shape[1]
                m[nm] = np.ascontiguousarray(
                    g_k.reshape(NT, PART, upw, C_LOC).transpose(1, 3, 0, 2)
                ).reshape(PART, C_LOC * NT * upw).astype(ml_dtypes.bfloat16)
        in_maps.append(m)

    _LAST["nc"] = nc
    _LAST["in_maps"] = in_maps
    res = run_bass_kernel_spmd(nc, in_maps, list(range(N_CORES)))

    out = np.zeros((N_BOXES, OUT, OUT, C), np.float32)
    perm_np = np.asarray(perm)
    for kcore in range(N_CORES):
        dev = np.asarray(res.results[kcore]["out"]).astype(np.float32)
        # [p, (G, g, hh, j)] with p = c_lh*32 + i, slot = 16G + g =
        # processing position; unpermute to original box order
        v = dev.reshape(4, OUT, N_BOXES // 16, 16, 2, OUT)  # c_lh, i, G, g, hh, j
        v = v.transpose(2, 3, 1, 5, 4, 0)                   # G, g, i, j, hh, c_lh
        v = v.reshape(N_BOXES, OUT, OUT, C_LOC)
        out[perm_np, :, :, kcore * C_LOC:(kcore + 1) * C_LOC] = v
    for n in range(N_BOXES):
        if plans[n] is None:
            out[n] = 0.0
    return out


def estimate_hw_ns():
    """Cost-model estimate of the per-core kernel duration (ns)."""
    from concourse.timeline_sim import TimelineSim
    nc = _LAST.get("nc")
    if nc is None:
        return -1
    sim = TimelineSim(nc)
    sim.simulate()
    return int(sim.time)


def measure_wall(n=5):
    """Wall-clock of repeated dispatches (includes axon round trips)."""
    import time
    from concourse.bass_utils import run_bass_kernel_spmd
    times = []
    for _ in range(n):
        t0 = time.perf_counter()
        run_bass_kernel_spmd(_LAST["nc"], _LAST["in_maps"], list(range(N_CORES)))
        times.append(time.perf_counter() - t0)
    return times


# revision 70
# speedup vs baseline: 1.2229x; 1.0253x over previous
"""RoIAlign (scale_and_translate, linear, antialias) Trainium2 kernel.

Channel-sharded across 8 NeuronCores: each core keeps a [512, 512, 8]
slice of the feature map resident in SBUF and computes all 512 boxes
for its 8 channels, one box at a time (no box grouping):

  stage 1 (PE):  T^T[x, (c, i)] = sum_y F[y, x, c] * Wy[y, i]
                 one matmul per (c, y-tile, x-window), out free = 32.
                 Wide-kernel boxes (ky*kx >= 12) run fp8e4 DoubleRow
                 (two 128-row y-tiles per matmul, 0.5 cycles/row); the
                 rest run bf16.
  evac:          psum_t [128, 512] (two (box, window) slots) -> SBUF
                 bf16, least-loaded assignment over DVE / ACT (GPSIMD
                 cannot read PSUM).
  stage 2 (PE):  out[(c,i), j] = sum_x T^T[x, ci] * Wx[x, j],
                 operand-swapped (lhsT = T^T chunk, moving = 32 Wx
                 cols) so out free = 32 instead of 256.
  out:           psum2 [128, 512] (8 boxes) -> bf16 staging -> DRAM.

Boxes with wide x-kernels use device-resident 2x/4x x-pooled copies of
the feature map (hat-function pooling; per-box Wx is least-squares
refit onto the hat basis, with delta columns at the image edge). This
shrinks the x-band, cutting stage-1 work and - critically - the
PSUM-evacuation volume, which is the binding engine resource.

Host side computes exact per-box dense resampling weights, extracts
nonzero bands, zero-pads to 128-row tiles/windows, and streams them as
flat [128, cols] arrays in 32-box chunks. fp8 pooled maps are padded
to multiple-of-16 widths (DoubleRow ldweights stride restriction).
"""

import numpy as np

H = 512
W = 512
C = 64
N_BOXES = 512
OUT = 32
N_CORES = 8
C_LOC = C // N_CORES  # 8 channels per core
PART = 128
NT = H // PART        # 4 y partition tiles
GROUP = 32            # boxes per weight-DMA chunk
SLOTS = 4             # (box, window) slots per psum_t tile
FP8_KPROD = 12.0      # use fp8 stage-1 when ky*kx >= this
FP8_KMIN = 2.0        # ... and both ky, kx >= this
XP2_KMIN = 9.0        # use the 2x x-pooled map when kx >= this
XP4_KMIN = 13.0       # use the 4x x-pooled map when kx >= this
WCLIP = 1e-3          # zero fitted pooled weights below this (rel) magnitude
UP2 = W // 2 + 1      # 2x-pooled x size: hats at even x + delta at x=511
UP4 = (W - 1) // 4 + 1 + 3  # 4x pooled: 128 hat nodes + 3 tail deltas = 131
UPS = {2: UP2, 4: UP4}
# fp8 DoubleRow ldweights requires the pair-dim byte stride to be a
# multiple of 16 -> pad the fp8 pooled-map widths up to a multiple of 16
UPS_PAD = {p: (u + 15) // 16 * 16 for p, u in UPS.items()}
INTERLEAVE_HEAD = 96  # pure-bf16 boxes before class interleaving starts

# scheduling knobs for _build_program (tuned via TimelineSim)
BUILD_KW = dict(slots=4, p1bufs=3, rbufs=8, p2bufs=2, evac_pat="auto",
                wdma="yy", odma="y", s2_lag=16, wbufs=3, map_lead=1)


# ---------------------------------------------------------------------------
# Host-side weight computation (mirrors jax.image.scale_and_translate with
# method="linear", antialias=True)
# ---------------------------------------------------------------------------

def _compute_weight_mat(in_size, out_size, scale, translation):
    inv_scale = 1.0 / scale
    kernel_scale = max(inv_scale, 1.0)
    sample_f = (np.arange(out_size, dtype=np.float64) + 0.5) * inv_scale \
        - translation * inv_scale - 0.5
    x = np.abs(sample_f[None, :] - np.arange(in_size, dtype=np.float64)[:, None]) \
        / kernel_scale
    weights = np.maximum(0.0, 1.0 - x)
    total = weights.sum(axis=0, keepdims=True)
    weights = np.where(
        np.abs(total) > 1000.0 * float(np.finfo(np.float32).eps),
        weights / np.where(total != 0, total, 1.0),
        0.0,
    )
    valid = (sample_f >= -0.5) & (sample_f <= in_size - 0.5)
    return np.where(valid[None, :], weights, 0.0).astype(np.float32)


def host_geometry(boxes):
    """Exact per-box dense weights wy/wx [N, 512, 32] fp32."""
    boxes = np.asarray(boxes, dtype=np.float64)
    wy_all = np.zeros((N_BOXES, H, OUT), np.float32)
    wx_all = np.zeros((N_BOXES, W, OUT), np.float32)
    for n in range(N_BOXES):
        cx, cy, w, h = boxes[n]
        x0 = cx - w / 2
        y0 = cy - h / 2
        w = max(w, 1e-6)
        h = max(h, 1e-6)
        x_scale = OUT / (w * W)
        y_scale = OUT / (h * H)
        wy_all[n] = _compute_weight_mat(H, OUT, y_scale, -y0 * OUT / h)
        wx_all[n] = _compute_weight_mat(W, OUT, x_scale, -x0 * OUT / w)
    return wy_all, wx_all


def xpool_basis(p):
    """Hat basis at stride-p grid nodes plus delta columns for the tail
    pixels past the last node (which a hat grid cannot represent)."""
    nu = (W - 1) // p + 1
    extra = [x for x in range(W) if x > (nu - 1) * p]
    P = np.zeros((W, nu + len(extra)), np.float32)
    for u in range(nu):
        c = p * u
        for x in range(max(0, c - p + 1), min(W, c + p)):
            P[x, u] = 1.0 - abs(x - c) / p
    for j, x in enumerate(extra):
        P[x, :] = 0.0
        P[x, nu + j] = 1.0
    return P


def xpool_fit_mats():
    """Per pooling factor: (P, PINV) with PINV = (P^T P)^-1 P^T."""
    mats = {}
    for p in (2, 4):
        P = xpool_basis(p)
        PINV = np.linalg.solve((P.T @ P).astype(np.float64),
                               P.T.astype(np.float64))
        mats[p] = (P, PINV)
    return mats


def plan_boxes(boxes, wy_all, wx_all):
    """Per-box banded geometry + flat weight packing.

    Returns plan dicts (None for boxes with empty bands) and the packed
    flat weight arrays wyb [128, :] bf16, wy8 [128, :] fp8, wx [128, :]
    bf16 (as float32 here; cast at upload).
    """
    boxes = np.asarray(boxes, dtype=np.float64)
    ky = np.maximum(16.0 * boxes[:, 3], 1.0)  # y kernel halfwidth (px)
    kx = np.maximum(16.0 * boxes[:, 2], 1.0)

    mats = xpool_fit_mats()

    # processing order: boxes whose source map arrives earliest on the
    # input DMA queue go first (f -> f8 -> pooled bf16 -> pooled fp8), so
    # compute covers the input-upload stream instead of stalling on it.
    # Within the first class, boxes needing only low y-tiles go first so
    # PE can start right after the first F-tile DMA lands.
    def klass(n):
        f8 = (ky[n] * kx[n] >= FP8_KPROD
              and ky[n] >= FP8_KMIN and kx[n] >= FP8_KMIN)
        xp = 4 if kx[n] >= XP4_KMIN else (2 if kx[n] >= XP2_KMIN else 1)
        if xp == 1:
            return 0 if not f8 else 1
        return 2 if not f8 else 3

    def last_tile(n):
        nz = np.nonzero(wy_all[n].any(axis=1))[0]
        return (int(nz[-1]) // PART) if len(nz) else 0

    by_class = {0: [], 1: [], 2: [], 3: []}
    for n in range(N_BOXES):
        by_class[klass(n)].append(n)
    by_class[0].sort(key=lambda n: (last_tile(n), n))
    # head: pure bf16 boxes to cover the f/f8 upload; then interleave the
    # PE-heavy (bf16) remainder evenly among the evac-heavy fp8 boxes so
    # neither PE nor the evac engines starves in class-segregated bursts
    NA = min(INTERLEAVE_HEAD, len(by_class[0]))
    head = by_class[0][:NA]
    pe_list = by_class[0][NA:] + by_class[2]   # bf16: PE-heavy
    ev_list = by_class[1] + by_class[3]        # fp8: evac-heavy
    total = len(pe_list) + len(ev_list)
    merged = []
    pi = ei = 0
    acc = 0.0
    r = len(pe_list) / max(total, 1)
    for _ in range(total):
        acc += r
        if acc >= 1.0 and pi < len(pe_list):
            merged.append(pe_list[pi]); pi += 1; acc -= 1.0
        elif ei < len(ev_list):
            merged.append(ev_list[ei]); ei += 1
        else:
            merged.append(pe_list[pi]); pi += 1
    perm = head + merged

    plans = [None] * N_BOXES
    wyb_cols, wy8_cols, wx_cols = [], [], []
    for n in perm:
        ynz = np.nonzero(wy_all[n].any(axis=1))[0]
        xnz = np.nonzero(wx_all[n].any(axis=1))[0]
        if len(ynz) == 0 or len(xnz) == 0:
            continue
        r0, r1 = int(ynz[0]), int(ynz[-1]) + 1
        use_fp8 = (ky[n] * kx[n] >= FP8_KPROD
                   and ky[n] >= FP8_KMIN and kx[n] >= FP8_KMIN)
        xp = 4 if kx[n] >= XP4_KMIN else (2 if kx[n] >= XP2_KMIN else 1)

        if xp > 1:
            # least-squares fit of Wx in the pooled hat basis, clipping
            # the tiny ringing tail of the fit to keep the band compact
            P, PINV = mats[xp]
            wx_n = (PINV @ wx_all[n].astype(np.float64)).astype(np.float32)
            wx_n[np.abs(wx_n) < WCLIP * np.abs(wx_n).max()] = 0.0
            xnz = np.nonzero(wx_n.any(axis=1))[0]
            if len(xnz) == 0:
                continue
            WW = UPS[xp]
        else:
            wx_n = wx_all[n]
            WW = W
        c0, c1 = int(xnz[0]), int(xnz[-1]) + 1

        # x windows (arbitrary free-dim offset, clamped; overlap zeroed)
        nxw = (c1 - c0 + PART - 1) // PART
        xws = [min(c0 + k * PART, max(WW - PART, 0)) for k in range(nxw)]

        p = {"fp8": use_fp8, "xp": xp, "xws": xws}
        if use_fp8:
            p_lo, p_hi = r0 // (2 * PART), (r1 - 1) // (2 * PART)
            p["tp"] = list(range(p_lo, p_hi + 1))
            p["wy8_off"] = len(wy8_cols) and sum(c.shape[1] for c in wy8_cols)
            p["wy8_off"] = sum(c.shape[1] for c in wy8_cols)
            for tp in p["tp"]:
                # [128, (q, i)] with y = tp*256 + q*128 + p
                blk = np.zeros((PART, 2, OUT), np.float32)
                for q in range(2):
                    blk[:, q, :] = wy_all[n][tp * 256 + q * 128: tp * 256 + (q + 1) * 128]
                wy8_cols.append(blk.reshape(PART, 2 * OUT))
        else:
            t_lo, t_hi = r0 // PART, (r1 - 1) // PART
            p["ts"] = list(range(t_lo, t_hi + 1))
            p["wyb_off"] = sum(c.shape[1] for c in wyb_cols)
            for t in p["ts"]:
                wyb_cols.append(wy_all[n][t * PART:(t + 1) * PART].copy())

        p["wx_off"] = sum(c.shape[1] for c in wx_cols)
        prev_end = c0
        for x0 in xws:
            wxw = wx_n[x0:x0 + PART].copy()
            lo = max(prev_end - x0, 0)
            wxw[:lo] = 0.0
            prev_end = max(prev_end, x0 + PART)
            wx_cols.append(wxw)
        plans[n] = p

    def cat(cols, width):
        if not cols:
            return np.zeros((PART, width), np.float32)
        return np.concatenate(cols, axis=1)

    wyb_flat = cat(wyb_cols, OUT)
    wy8_flat = cat(wy8_cols, 2 * OUT)
    wx_flat = cat(wx_cols, OUT)
    return plans, perm, wyb_flat, wy8_flat, wx_flat


# ---------------------------------------------------------------------------
# Device program
# ---------------------------------------------------------------------------

def _split_multiwait_bir(raw: bytes) -> bytes:
    """The walrus build here accepts only one sync wait per instruction.
    Hoist extra waits onto single-wait EventSemaphore instructions inserted
    just before, on the same engine (per-engine order is preserved)."""
    import orjson

    d = orjson.loads(raw)
    ctr = 0
    for fn in d.get("functions", []):
        for bb in fn.get("blocks") or []:
            out = []
            for ins in bb["instructions"]:
                si = ins.get("sync_info")
                ws = (si or {}).get("on_wait") or []
                if len(ws) > 1:
                    for w in ws[:-1]:
                        ctr += 1
                        out.append({
                            "debug": ins.get("debug", 0),
                            "engine": ins["engine"],
                            "ins": [],
                            "outs": [],
                            "name": f"{ins['name']}-xw{ctr}",
                            "opcode": "EventSemaphore",
                            "sync_info": {"on_update": [], "on_wait": [w]},
                        })
                    si["on_wait"] = [ws[-1]]
                out.append(ins)
            bb["instructions"] = out
    return orjson.dumps(d)


def _patch_serialization(nc):
    orig = nc.to_json_bytes

    def patched():
        return _split_multiwait_bir(orig())

    nc.to_json_bytes = patched
    return nc


def _make_tc_class():
    import concourse.tile as tile
    from concourse.vector_clock import ScopedClock
    import bass_rust

    class TC(tile.TileContext):
        """TileContext with the tail drain's multi-sem wait split into
        individual single-wait instructions (this walrus rejects >1 wait
        on a CTRL instruction)."""

        def _drain_and_barrier(self, tick_clock, wait_clock):
            nc = self.nc
            probe = nc.sync.drain()
            wait_clock.add_sem_waits(
                probe.ins, ScopedClock({None: tick_clock.global_clock})
            )
            waits = list(probe.ins.sync_info.on_wait)
            probe.ins.sync_info = bass_rust.SyncInfo(on_wait=[], on_update=[])
            by_name = {hh.name: hh for hh in self.sems.allocated().values()}
            for wt in waits:
                nc.sync.wait_ge(by_name[wt.ant_name], wt.wait_value)
            nc.all_engine_barrier()
            popped = nc._tile_sem_poison_stack.pop()
            assert popped is self._sem_poison
            nc.clear_and_free_semaphores(list(self.sems.allocated().values()))
            nc.all_engine_barrier()

    return TC


def _build_program(plans, perm, wyb_cols_n, wy8_cols_n, wx_cols_n,
                   slots=SLOTS, p1bufs=2, rbufs=6, p2bufs=2, evac_mod=3,
                   evac_pat=None, wdma="gs", odma="y", s2_lag=0,
                   wbufs=2, map_lead=1):
    import concourse.bass as bass
    import concourse.mybir as mybir
    from contextlib import ExitStack

    FP32 = mybir.dt.float32
    BF16 = mybir.dt.bfloat16
    F8E4 = mybir.dt.float8e4
    DR = mybir.MatmulPerfMode.DoubleRow

    any_fp8 = any(p is not None and p["fp8"] for p in plans)
    # pooled-map variants needed: (pool factor, fp8?)
    variants = sorted({(p["xp"], p["fp8"]) for p in plans
                       if p is not None and p["xp"] > 1})

    nc = bass.Bass()
    # feature map, already in SBUF layout [128, (c, t, x)] / [128, (c, tp, q, x)]
    f_d = nc.dram_tensor("f", [PART, C_LOC * NT * W], BF16, kind="ExternalInput")
    if any_fp8:
        f8_d = nc.dram_tensor("f8", [PART, C_LOC * 2 * 2 * W], F8E4,
                              kind="ExternalInput")
    g_ds = {}
    for (xp, isf8) in variants:
        nm = f"g{xp}{'f8' if isf8 else ''}"
        if isf8:
            g_ds[(xp, True)] = nc.dram_tensor(
                nm, [PART, C_LOC * 2 * 2 * UPS_PAD[xp]], F8E4,
                kind="ExternalInput")
        else:
            g_ds[(xp, False)] = nc.dram_tensor(
                nm, [PART, C_LOC * NT * UPS[xp]], BF16, kind="ExternalInput")
    wyb_d = nc.dram_tensor("wyb", [PART, max(wyb_cols_n, OUT)], BF16,
                           kind="ExternalInput")
    if any_fp8:
        wy8_d = nc.dram_tensor("wy8", [PART, max(wy8_cols_n, 2 * OUT)], F8E4,
                               kind="ExternalInput")
    wx_d = nc.dram_tensor("wx", [PART, max(wx_cols_n, OUT)], BF16,
                          kind="ExternalInput")
    # output: box b = 16*G + g: cols G*1024 + g*64 + hh*32 + j,
    # partition p = c_lh*32 + i, channel c = hh*4 + c_lh
    NG16 = (N_BOXES + 15) // 16
    if odma == "P":
        out_d = nc.dram_tensor("out", [PART, (N_BOXES // 4) * 256], FP32,
                               kind="ExternalOutput")
    else:
        out_d = nc.dram_tensor("out", [PART, NG16 * 1024], BF16,
                               kind="ExternalOutput")

    # per-GROUP chunk column ranges (groups are processing positions)
    def group_ranges(key, width):
        rng = []
        for g0 in range(0, N_BOXES, GROUP):
            los, his = [], []
            for n in perm[g0:g0 + GROUP]:
                p = plans[n]
                if p is None or key not in p:
                    continue
                ntiles = len(p["ts"]) if key == "wyb_off" else (
                    len(p["tp"]) if key == "wy8_off" else len(p["xws"]))
                los.append(p[key])
                his.append(p[key] + ntiles * width)
            rng.append((min(los), max(his)) if los else None)
        return rng

    wyb_rng = group_ranges("wyb_off", OUT)
    wy8_rng = group_ranges("wy8_off", 2 * OUT)
    wx_rng = group_ranges("wx_off", OUT)

    TC = _make_tc_class()
    with TC(nc) as tc, ExitStack() as ctx:
        fpool = ctx.enter_context(tc.tile_pool(name="fmap", bufs=1))
        wpool = ctx.enter_context(tc.tile_pool(name="wts", bufs=wbufs))
        rpool = ctx.enter_context(tc.tile_pool(name="rhs2", bufs=rbufs))
        opool = ctx.enter_context(tc.tile_pool(name="osb", bufs=2))
        p1pool = ctx.enter_context(tc.tile_pool(name="psumT", bufs=p1bufs, space="PSUM"))
        p2pool = ctx.enter_context(tc.tile_pool(name="psum2", bufs=p2bufs, space="PSUM"))

        _eng_map = {"g": nc.gpsimd, "s": nc.scalar, "y": nc.sync}

        def _wy_eng():
            return _eng_map[wdma[0]]

        def _wx_eng():
            return _eng_map[wdma[1]]

        # weight chunk state
        wy_tiles = {}  # group index -> {"b": (tile, base), "8": (tile, base)}
        wx_tiles = {}  # group index -> (tile, base)

        def load_chunks(gi):
            if gi in wy_tiles:
                return
            wy_tiles[gi] = {}
            if wyb_rng[gi] is not None:
                lo, hi = wyb_rng[gi]
                tb = wpool.tile([PART, hi - lo], BF16, name="wyb_sb", tag="wyb")
                _wy_eng().dma_start(out=tb[:], in_=wyb_d[:, lo:hi])
                wy_tiles[gi]["b"] = (tb, lo)
            if wy8_rng[gi] is not None:
                lo, hi = wy8_rng[gi]
                t8 = wpool.tile([PART, hi - lo], F8E4, name="wy8_sb", tag="wy8")
                _wy_eng().dma_start(out=t8[:], in_=wy8_d[:, lo:hi])
                wy_tiles[gi]["8"] = (t8, lo)
            if wx_rng[gi] is not None:
                lo, hi = wx_rng[gi]
                tx = wpool.tile([PART, hi - lo], BF16, name="wx_sb", tag="wx")
                _wx_eng().dma_start(out=tx[:], in_=wx_d[:, lo:hi])
                wx_tiles[gi] = (tx, lo)

        f_sb = fpool.tile([PART, C_LOC * NT * W], BF16, name="f_sb")
        f_v = f_sb[:].rearrange("p (c t x) -> p c t x", c=C_LOC, t=NT)
        f_dv = f_d[:].rearrange("p (c t x) -> p c t x", c=C_LOC, t=NT)
        # queue order: first channels of the first F tile, then group-0
        # weights (both needed by the very first matmuls), then the rest
        nc.sync.dma_start(out=f_v[:, 0:4, 0, :], in_=f_dv[:, 0:4, 0, :])
        load_chunks(0)
        nc.sync.dma_start(out=f_v[:, 4:8, 0, :], in_=f_dv[:, 4:8, 0, :])
        for t in range(1, NT):
            nc.sync.dma_start(out=f_v[:, :, t, :], in_=f_dv[:, :, t, :])

        # f8 / pooled-map uploads are deferred to just before the first
        # processing group that uses them, so the (FIFO) input queue
        # delivers weights and maps in need order instead of stalling
        # later weight chunks behind maps nobody needs yet.
        map_dmas = {}  # key -> emit thunk
        first_grp = {}  # key -> first processing group using the map

        def _key_of(p):
            if p["xp"] > 1:
                return (p["xp"], p["fp8"])
            return "f8" if p["fp8"] else None

        for pos, n in enumerate(perm):
            p = plans[n]
            if p is None:
                continue
            kkey = _key_of(p)
            if kkey is not None and kkey not in first_grp:
                first_grp[kkey] = pos // GROUP

        if any_fp8:
            f8_sb = fpool.tile([PART, C_LOC * 2 * 2 * W], F8E4, name="f8_sb")
            f8_v = f8_sb[:].rearrange("p (c tp q x) -> p c tp q x",
                                      c=C_LOC, tp=2, q=2)
            f8_dv = f8_d[:].rearrange("p (c tp q x) -> p c tp q x",
                                      c=C_LOC, tp=2, q=2)

            def _emit_f8(f8_v=f8_v, f8_dv=f8_dv):
                for tp in range(2):
                    nc.sync.dma_start(out=f8_v[:, :, tp, :, :],
                                      in_=f8_dv[:, :, tp, :, :])
            map_dmas["f8"] = _emit_f8
        g_vs = {}
        for (xp, isf8), gd in g_ds.items():
            if isf8:
                upw = UPS_PAD[xp]
                gt = fpool.tile([PART, C_LOC * 2 * 2 * upw], F8E4,
                                name=f"g{xp}f8_sb", tag=f"g{xp}f8")
                gv = gt[:].rearrange("p (c tp q x) -> p c tp q x",
                                     c=C_LOC, tp=2, q=2)

                def _emit(gv=gv, gd=gd):
                    nc.sync.dma_start(out=gv, in_=gd[:].rearrange(
                        "p (c tp q x) -> p c tp q x", c=C_LOC, tp=2, q=2))
            else:
                upw = UPS[xp]
                gt = fpool.tile([PART, C_LOC * NT * upw], BF16,
                                name=f"g{xp}_sb", tag=f"g{xp}")
                gv = gt[:].rearrange("p (c t x) -> p c t x", c=C_LOC, t=NT)

                def _emit(gv=gv, gd=gd):
                    nc.sync.dma_start(out=gv, in_=gd[:].rearrange(
                        "p (c t x) -> p c t x", c=C_LOC, t=NT))
            map_dmas[(xp, isf8)] = _emit
            g_vs[(xp, isf8)] = gv

        def emit_maps_due(gi):
            """Emit deferred map DMAs with map_lead groups of lead time."""
            for kkey, thunk in list(map_dmas.items()):
                if first_grp.get(kkey, 0) <= gi + map_lead:
                    thunk()
                    del map_dmas[kkey]
        emit_maps_due(0)

        evac_busy = [0.0, 0.0]  # DVE, ACT modeled busy ns

        def evac(out_ap, in_ap, idx):
            # GPSIMD cannot read PSUM on real HW: DVE / ACT only.
            free = in_ap.free_size()
            costs = (free * 1.0417 + 125.0, free * 0.8333 + 185.0)
            if evac_pat == "auto" or evac_pat is None:
                which = 0 if evac_busy[0] + costs[0] <= evac_busy[1] + costs[1] \
                    else 1
            else:
                which = {"v": 0, "s": 1}[evac_pat[idx % len(evac_pat)]]
            evac_busy[which] += costs[which]
            if which == 0:
                nc.vector.tensor_copy(out=out_ap, in_=in_ap)
            else:
                nc.scalar.copy(out_ap, in_ap)

        evac_ctr = 0
        # (box, window) slot stream state
        cur_p1 = None          # current psum_t tile
        cur_slots = []         # [(box, win_idx)]
        rhs2_of = {}           # (box, win) -> (tile, slot)
        pend_s2 = []           # boxes whose stage-2 is not yet emitted

        o_sb = None
        psum2 = None

        def flush_p1():
            nonlocal cur_p1, cur_slots, evac_ctr
            if cur_p1 is None:
                return
            r_t = rpool.tile([PART, slots * 256], BF16, name="r_t", tag="r")
            evac(r_t[:], cur_p1[:], evac_ctr)
            evac_ctr += 1
            for si, key in enumerate(cur_slots):
                rhs2_of[key] = (r_t, si)
            cur_p1 = None
            cur_slots = []

        def emit_stage2(n):
            """stage 2 + output for box n (requires rhs2 of all windows).
            Output slots are by processing position; host unpermutes."""
            nonlocal psum2, o_sb, evac_ctr
            p = plans[n]
            pos = pos_of[n]
            g16, s16 = pos // 16, pos % 16
            oct_, sq = s16 // 8, s16 % 8
            if s16 == 0:
                o_sb = opool.tile([PART, 1024], BF16, name="o_sb", tag="o")
            if sq == 0:
                psum2 = p2pool.tile([PART, 512], FP32, name="ps2", tag="ps2")
            if p is None:
                # write *something* finite so the tile is defined
                for hh in range(2):
                    nc.tensor.matmul(
                        out=psum2[:, sq * 64 + hh * 32: sq * 64 + hh * 32 + 32],
                        lhsT=f_v[:, 0, 0, 0:PART], rhs=f_v[:, 0, 0, 0:OUT],
                        start=True, stop=True)
            else:
                nw = len(p["xws"])
                for hh in range(2):
                    for k in range(nw):
                        r_t, si = rhs2_of[(n, k)]
                        wx_sl = wx_sb_view(n, k)
                        nc.tensor.matmul(
                            out=psum2[:, sq * 64 + hh * 32: sq * 64 + hh * 32 + 32],
                            lhsT=r_t[:, si * 256 + hh * 128: si * 256 + (hh + 1) * 128],
                            rhs=wx_sl,
                            start=(k == 0), stop=(k == nw - 1))
                for k in range(nw):
                    rhs2_of.pop((n, k), None)
            if sq == 7:
                evac(o_sb[:, oct_ * 512:(oct_ + 1) * 512], psum2[:], evac_ctr)
                evac_ctr += 1
            if s16 == 15:
                _eng_map[odma].dma_start(
                    out=out_d[:, g16 * 1024:(g16 + 1) * 1024], in_=o_sb[:])

        pos_of = {n: i for i, n in enumerate(perm)}

        def wx_sb_view(n, k):
            t, base = wx_tiles[pos_of[n] // GROUP]
            off = plans[n]["wx_off"] + k * OUT - base
            return t[:, off:off + OUT]

        NGRP = (N_BOXES + GROUP - 1) // GROUP
        for g0 in range(0, N_BOXES, GROUP):
            gi = g0 // GROUP
            emit_maps_due(gi)
            for la in range(wbufs - 1):
                if gi + la < NGRP:
                    load_chunks(gi + la)
            if wyb_rng[gi] is not None:
                wyb_sb, wyb_base = wy_tiles[gi]["b"]
            if wy8_rng[gi] is not None:
                wy8_sb, wy8_base = wy_tiles[gi]["8"]

            for n in perm[g0:g0 + GROUP]:
                p = plans[n]
                if p is not None:
                    for k, x0 in enumerate(p["xws"]):
                        if cur_p1 is None:
                            cur_p1 = p1pool.tile([PART, slots * 256], FP32,
                                                 name="ps1", tag="ps1")
                        si = len(cur_slots)
                        cur_slots.append((n, k))
                        base = si * 256
                        if p["fp8"]:
                            src = g_vs[(p["xp"], True)] if p["xp"] > 1 else f8_v
                            off = p["wy8_off"] - wy8_base
                            for c in range(C_LOC):
                                for j, tp in enumerate(p["tp"]):
                                    nc.tensor.matmul(
                                        out=cur_p1[:, base + c * OUT: base + (c + 1) * OUT],
                                        lhsT=src[:, c, tp, :, x0:x0 + PART],
                                        rhs=wy8_sb[:, off + j * 64: off + (j + 1) * 64]
                                            .rearrange("p (q i) -> p q i", q=2),
                                        start=(j == 0), stop=(j == len(p["tp"]) - 1),
                                        perf_mode=DR)
                        else:
                            src = g_vs[(p["xp"], False)] if p["xp"] > 1 else f_v
                            off = p["wyb_off"] - wyb_base
                            for c in range(C_LOC):
                                for j, t in enumerate(p["ts"]):
                                    nc.tensor.matmul(
                                        out=cur_p1[:, base + c * OUT: base + (c + 1) * OUT],
                                        lhsT=src[:, c, t, x0:x0 + PART],
                                        rhs=wyb_sb[:, off + j * OUT: off + (j + 1) * OUT],
                                        start=(j == 0), stop=(j == len(p["ts"]) - 1))
                        if len(cur_slots) == slots:
                            flush_p1()
                pend_s2.append(n)
                # emit stage 2 for boxes whose windows are all evacuated
                while pend_s2:
                    b = pend_s2[0]
                    pb = plans[b]
                    if pb is not None and any(
                            (b, k) not in rhs2_of for k in range(len(pb["xws"]))):
                        break
                    if pos_of[n] - pos_of[b] < s2_lag:
                        break
                    emit_stage2(b)
                    pend_s2.pop(0)
        flush_p1()
        while pend_s2:
            emit_stage2(pend_s2.pop(0))

    return _patch_serialization(nc)


# ---------------------------------------------------------------------------
# Entry point
# ---------------------------------------------------------------------------

_LAST = {}


def kernel(feature_map, boxes, output_width):
    from concourse.bass_utils import run_bass_kernel_spmd
    import ml_dtypes

    feature_map = np.asarray(feature_map, dtype=np.float32)
    boxes_np = np.asarray(boxes, dtype=np.float32)
    assert int(output_width) == OUT

    wy_all, wx_all = host_geometry(boxes_np)
    plans, perm, wyb_flat, wy8_flat, wx_flat = plan_boxes(
        boxes_np, wy_all, wx_all)
    nc = _build_program(plans, perm, wyb_flat.shape[1], wy8_flat.shape[1],
                        wx_flat.shape[1], **BUILD_KW)

    any_fp8 = any(p is not None and p["fp8"] for p in plans)
    variants = sorted({(p["xp"], p["fp8"]) for p in plans
                       if p is not None and p["xp"] > 1})
    pools_needed = sorted({xp for (xp, _) in variants})
    wyb_u = wyb_flat.astype(ml_dtypes.bfloat16)
    wx_u = wx_flat.astype(ml_dtypes.bfloat16)
    if any_fp8:
        wy8_u = wy8_flat.astype(ml_dtypes.float8_e4m3)
    bases = {xp: xpool_basis(xp) for xp in pools_needed}

    in_maps = []
    for kcore in range(N_CORES):
        # f layout [p, (c, t, x)]: y = t*128 + p
        f_k = feature_map[:, :, kcore * C_LOC:(kcore + 1) * C_LOC]  # [y, x, c]
        f_bf = f_k.astype(ml_dtypes.bfloat16).astype(np.float32)
        f_sb = np.ascontiguousarray(
            f_bf.reshape(NT, PART, W, C_LOC).transpose(1, 3, 0, 2)
        ).reshape(PART, C_LOC * NT * W).astype(ml_dtypes.bfloat16)
        m = {"f": f_sb, "wyb": wyb_u, "wx": wx_u}
        if any_fp8:
            # f8 layout [p, (c, tp, q, x)]: y = tp*256 + q*128 + p
            f8_sb = np.ascontiguousarray(
                f_bf.reshape(2, 2, PART, W, C_LOC).transpose(2, 4, 0, 1, 3)
            ).reshape(PART, C_LOC * 2 * 2 * W).astype(ml_dtypes.float8_e4m3)
            m["f8"] = f8_sb
            m["wy8"] = wy8_u
        g_ks = {xp: np.einsum("xu,yxc->yuc", bases[xp], f_bf, optimize=True)
                for xp in pools_needed}
        for (xp, isf8) in variants:
            g_k = g_ks[xp]
            nm = f"g{xp}{'f8' if isf8 else ''}"
            if isf8:
                upw = UPS_PAD[xp]
                g_pad = np.zeros((H, upw, C_LOC), np.float32)
                g_pad[:, :g_k.shape[1], :] = g_k
                m[nm] = np.ascontiguousarray(
                    g_pad.reshape(2, 2, PART, upw, C_LOC).transpose(2, 4, 0, 1, 3)
                ).reshape(PART, C_LOC * 2 * 2 * upw).astype(ml_dtypes.float8_e4m3)
            else:
                upw = g_k.shape[1]
                m[nm] = np.ascontiguousarray(
                    g_k.reshape(NT, PART, upw, C_LOC).transpose(1, 3, 0, 2)
                ).reshape(PART, C_LOC * NT * upw).astype(ml_dtypes.bfloat16)
        in_maps.append(m)

    _LAST["nc"] = nc
    _LAST["in_maps"] = in_maps
    res = run_bass_kernel_spmd(nc, in_maps, list(range(N_CORES)))

    out = np.zeros((N_BOXES, OUT, OUT, C), np.float32)
    perm_np = np.asarray(perm)
    for kcore in range(N_CORES):
        dev = np.asarray(res.results[kcore]["out"]).astype(np.float32)
        # [p, (G, g, hh, j)] with p = c_lh*32 + i, slot = 16G + g =
        # processing position; unpermute to original box order
        v = dev.reshape(4, OUT, N_BOXES // 16, 16, 2, OUT)  # c_lh, i, G, g, hh, j
        v = v.transpose(2, 3, 1, 5, 4, 0)                   # G, g, i, j, hh, c_lh
        v = v.reshape(N_BOXES, OUT, OUT, C_LOC)
        out[perm_np, :, :, kcore * C_LOC:(kcore + 1) * C_LOC] = v
    for n in range(N_BOXES):
        if plans[n] is None:
            out[n] = 0.0
    return out


def estimate_hw_ns():
    """Cost-model estimate of the per-core kernel duration (ns)."""
    from concourse.timeline_sim import TimelineSim
    nc = _LAST.get("nc")
    if nc is None:
        return -1
    sim = TimelineSim(nc)
    sim.simulate()
    return int(sim.time)


def measure_wall(n=5):
    """Wall-clock of repeated dispatches (includes axon round trips)."""
    import time
    from concourse.bass_utils import run_bass_kernel_spmd
    times = []
    for _ in range(n):
        t0 = time.perf_counter()
        run_bass_kernel_spmd(_LAST["nc"], _LAST["in_maps"], list(range(N_CORES)))
        times.append(time.perf_counter() - t0)
    return times
